# revision 30
# baseline (speedup 1.0000x reference)
"""Trainium2 Bass kernel for nn_ODEFunc (gnn_message_passing, 8 cores).

Strategy (cost model: matmul = out-free-rows; DMA = free-dim bytes per
queue, 3 queues; collective = 15us + gathered bytes / 40GBps):
  - Batch-parallel branches: core b computes batch b's diff+adv gconv
    branches. All Chebyshev mats are built NODE-major with the support
    as the matmul *stationary* ([128,128] S^T blocks), so each x1/x2
    costs only 16 matmuls x fin rows instead of streaming the 512-wide
    support as moving data.
  - Node-major mats are packed per layer into nm stacks [128, J, 4, fin]
    (mat index J on the free dim -> no partition-start issues). The
    layer GEMM needs feature-major stationaries: batched PE transposes
    flip 8 (L1) / 2 (L2) mats per 128-row k-tile in one psum bank.
  - Layer GEMMs run with the (host-permuted) weights as *moving* data:
    out = c1/grads node-major, 64/16 rows per matmul. Biases are added
    with a ones-row rank-1 matmul into the same psum group.
  - W_d2/b_d2, W_a2/b_a2 are host-negated so tanh emits the grad sign;
    the diff 0.1 coefficient is one DVE op. Grads come out node-major,
    exactly the AllGather staging layout (no grad transposes).
  - AllGather [2,128,4,16] fp16 per core; gathered grads land straight
    in the W_f stationary gt_all[128, 40, KT] via two 3-dim strided
    DMAs (kt = m*16+f; wt is host-permuted to the same k enumeration).
    psX[40, 512]: X_diff rows 0-7, X_adv rows 32-39 (cols 8-31 are
    memset-zero lanes).
  - W_f shard (fp16, 16 MB) is split across the 3 DMA queues sized to
    each queue's idle windows (ACT's chunk is scheduled into the
    collective window via tile_wait_until).
  - GEMM half 1 finishes before half 2 starts so its gated-fusion chain
    hides under half 2's matmuls.
"""

import sys

sys.path.insert(0, "/opt/trn_rl_repo")

import numpy as np

import concourse.bass as bass
import concourse.mybir as mybir
from concourse import masks
from concourse.bass_utils import run_bass_kernel_spmd
from concourse.tile import TileContext
from concourse.vector_clock import ScopedClock

N = 512          # nodes
FL = 16          # latent
U = 64           # units
B = 8            # batch
HID = N * FL     # 8192
COEFF = 0.1
NCORES = 8
JS = HID // NCORES  # 1024 output columns per core
KT = HID // 128     # 64 contraction tiles for the W_f GEMM

f16 = mybir.dt.float16
f32 = mybir.dt.float32
AF = mybir.ActivationFunctionType
ALU = mybir.AluOpType

# smalls_f16 packed free-dim offsets (elements)
_OFF_X0M = 0             # [128, 4*16] x0 node-major
_OFF_WA1 = 64            # 3 tiles [128, 64]
_OFF_WD1 = 256           # [48(->128), 64]
_OFF_WA2 = 320           # 9 tiles [128, 16]
_OFF_WD2 = 464           # 2 tiles [128, 16]
_OFF_BF = 496            # [1, 1024]
_OFF_B1A = 1520          # [1, 64]
_OFF_B1D = 1584          # [1, 64]
_OFF_B2A = 1648          # [1, 16]
_OFF_B2D = 1664          # [1, 16]
_OFF_ONES = 1680         # [1, 128] ones
_SM16 = 1808


class PatchedTileContext(TileContext):
    """Tail drain with at most one sem wait per instruction.

    The walrus build here rejects Drain instructions carrying >2 sync
    waits ("Too many sync wait commands"). Spread the global-clock waits
    over individual SP nops ahead of the drain.
    """

    def _drain_and_barrier(self, tick_clock, wait_clock):
        nc = self.nc
        probe = nc.sync.nop(nofuse=True)
        wait_clock.add_sem_waits(
            probe.ins, ScopedClock({None: tick_clock.global_clock})
        )
        si = probe.ins.sync_info
        ws = list(si.on_wait) if si is not None else []
        if len(ws) > 1:
            probe.ins.sync_info = mybir.SyncInfo(
                on_wait=ws[:1], on_update=list(si.on_update)
            )
            for w in ws[1:]:
                n2 = nc.sync.nop(nofuse=True)
                n2.ins.sync_info = mybir.SyncInfo(on_wait=[w], on_update=[])
        nc.sync.drain()
        nc.all_engine_barrier()
        popped = nc._tile_sem_poison_stack.pop()
        assert popped is self._sem_poison
        nc.clear_and_free_semaphores(list(self.sems.allocated().values()))
        nc.all_engine_barrier()


_WAIT_LIMIT = 1


def _split_excess_waits(nc: bass.Bass) -> None:
    """Move sync waits beyond _WAIT_LIMIT onto same-engine NOPs inserted
    just before the carrying instruction (this walrus build has tiny
    setupSyncWait budgets for DMA/collective/drain instruction formats)."""
    for fn in nc.m.functions:
        for bb in fn.blocks:
            insts = bb.instructions
            i = 0
            while i < len(insts):
                inst = insts[i]
                si = inst.sync_info
                ws = list(si.on_wait) if si is not None and si.on_wait else []
                if len(ws) > _WAIT_LIMIT and type(inst).__name__ != "InstNoOp":
                    keep = ws[:_WAIT_LIMIT]
                    extra = ws[_WAIT_LIMIT:]
                    inst.sync_info = mybir.SyncInfo(
                        on_wait=keep, on_update=list(si.on_update)
                    )
                    for k, w in enumerate(extra):
                        nop = mybir.InstNoOp(
                            name=f"{inst.name}-w{k}",
                            engine=inst.engine,
                            bass_nofuse=True,
                            sync_info=mybir.SyncInfo(on_wait=[w], on_update=[]),
                        )
                        nc.register_instruction(nop, overwrite=True)
                        insts.insert(i, nop)
                        i += 1
                i += 1


def _build(collective: bool = True) -> bass.Bass:
    nc = bass.Bass(num_devices=NCORES)

    # ---- DRAM I/O (per-core values supplied via in_maps) ----
    sm16_d = nc.dram_tensor("sm16", [128, _SM16], f16, kind="ExternalInput")
    sup_d = nc.dram_tensor("supT", [3, 128, 3, 4, N], f16, kind="ExternalInput")
    wt_d = nc.dram_tensor("wt", [128, KT, JS], f16, kind="ExternalInput")
    out_d = nc.dram_tensor("out", [B, JS], f32, kind="ExternalOutput")

    with PatchedTileContext(nc) as tc:
        from contextlib import ExitStack

        with ExitStack() as ctx:
            const_p = ctx.enter_context(tc.tile_pool(name="const", bufs=1))
            sup_p = ctx.enter_context(tc.tile_pool(name="sup", bufs=1))
            sc_p = ctx.enter_context(tc.tile_pool(name="sc", bufs=1))
            fus_p = ctx.enter_context(tc.tile_pool(name="fus", bufs=1))
            fu_p = ctx.enter_context(tc.tile_pool(name="fu", bufs=3))
            acc_p = ctx.enter_context(tc.tile_pool(name="acc", bufs=3, space="PSUM"))
            psx_p = ctx.enter_context(tc.tile_pool(name="psx", bufs=2, space="PSUM"))
            tr_p = ctx.enter_context(tc.tile_pool(name="tr", bufs=2, space="PSUM"))
            dram_p = ctx.enter_context(tc.tile_pool(name="dram", bufs=1, space="DRAM"))

            # ---- constants / memsets ----
            id128 = const_p.tile([128, 128], f16, tag="id")
            masks.make_identity(nc, id128[:])
            ones40 = const_p.tile([1, 40], f16, tag="ones")
            nc.vector.memset(ones40[:], 1.0)
            # W_f-GEMM stationary [q, col, kt]; cols 8-31 are never DMA'd
            # -> zero them once, early (garbage would NaN the psum)
            gt_all = const_p.tile([128, 40, KT], f16, tag="gt")
            nc.gpsimd.memset(gt_all[:], 0.0)

            # ---- input DMAs: smalls first; supports split SP/Pool so the
            # last support lands by ~9.5us ----
            sm16 = const_p.tile([128, _SM16], f16, tag="sm16")
            nc.sync.dma_start(sm16[:], sm16_d[:])
            sup_tiles = []
            for s in range(9):
                supb = sup_p.tile([128, 4, N], f16, tag=f"sup{s}")
                sup_tiles.append(supb)
            for s in (0, 2, 4, 6, 8):
                nc.sync.dma_start(sup_tiles[s][:], sup_d[s // 3][:, s % 3])
            for s in (1, 3, 5, 7):
                nc.gpsimd.dma_start(sup_tiles[s][:], sup_d[s // 3][:, s % 3])

            # W_f shard: SP streams most of it through the branch phase;
            # ACT's chunk is scheduled into the collective window. Pool
            # carries none (it does late-branch copies + the collective).
            wt_all = const_p.tile([128, KT, JS], f16, tag="wt")
            nc.sync.dma_start(wt_all[:, 0:40, :], wt_d[:, 0:40, :])
            nc.sync.dma_start(wt_all[:, 40:48, :], wt_d[:, 40:48, :])
            with tc.tile_wait_until(0.055):
                nc.scalar.dma_start(wt_all[:, 48:64, :], wt_d[:, 48:64, :])

            # packed-small views
            x0m_all = sm16[:, _OFF_X0M : _OFF_X0M + 64]

            def x0m_ap(m):
                return sm16[:, _OFF_X0M + m * FL : _OFF_X0M + (m + 1) * FL]

            def wa1_ap(t, k=128):
                return sm16[0:k, _OFF_WA1 + t * U : _OFF_WA1 + (t + 1) * U]

            wd1_ap = sm16[0:48, _OFF_WD1 : _OFF_WD1 + U]

            def wa2_ap(t, k=128):
                return sm16[0:k, _OFF_WA2 + t * FL : _OFF_WA2 + (t + 1) * FL]

            def wd2_ap(t, k=128):
                return sm16[0:k, _OFF_WD2 + t * FL : _OFF_WD2 + (t + 1) * FL]

            def bf_ap(lo, hi):
                return sm16[0:1, _OFF_BF + lo : _OFF_BF + hi]

            b1a = sm16[0:1, _OFF_B1A : _OFF_B1A + U]
            b1d = sm16[0:1, _OFF_B1D : _OFF_B1D + U]
            b2a = sm16[0:1, _OFF_B2A : _OFF_B2A + FL]
            b2d = sm16[0:1, _OFF_B2D : _OFF_B2D + FL]
            ones128 = sm16[0:1, _OFF_ONES : _OFF_ONES + 128]

            # node-major mat stacks [128, J, m, fin]; J: 0=x0/c1, then
            # x1_s at 1+2s, x2_s at 2+2s (the reference concat order)
            nm1a = const_p.tile([128, 4, 17, FL], f16, tag="nm1a")
            nm1d = const_p.tile([128, 4, 3, FL], f16, tag="nm1d")
            nm2a = const_p.tile([128, 4, 17, U], f16, tag="nm2a")
            nm2d = const_p.tile([128, 4, 3, U], f16, tag="nm2d")

            nc.vector.tensor_copy(nm1a[:, :, 0, :], x0m_all)
            nc.scalar.copy(nm1d[:, :, 0, :], x0m_all)

            def supT_ap(s, m, j):
                # S_s^T[m-block, j-block] = stationary for out node-block j
                return sup_tiles[s][:, m, j * 128 : (j + 1) * 128]

            # copy/stt helpers: 0 = DVE, 1 = ACT (copy only), 2 = Pool
            def cp(which, dst, src):
                if which == 1:
                    nc.scalar.copy(dst, src)
                elif which == 2:
                    nc.gpsimd.tensor_copy(dst, src)
                else:
                    nc.vector.tensor_copy(dst, src)

            def cheb_nm(fin, nm, s_list, x_src, pool_tag, cp_rot):
                """x1 = S@x, x2 = 2*S@x1 - x in node-major form.

                x_src(s): [128, 4, fin] node-major input (x0 or c1).
                Writes nm[:, 1+2i, :, :] and nm[:, 2+2i, :, :] for s_list[i].
                """
                # pass A: x1 for every support
                for i, s in enumerate(s_list):
                    psb = acc_p.tile([128, 4, U], f32, tag="ps")
                    ps1 = psb[:, :, 0:fin]
                    for j in range(4):
                        for m in range(4):
                            nc.tensor.matmul(
                                ps1[:, j, :], supT_ap(s, m, j),
                                x_src(s)[:, m, :],
                                start=(m == 0), stop=(m == 3),
                                skip_group_check=True,
                            )
                    cp(cp_rot[i % len(cp_rot)], nm[s][:, :, 1 + 2 * i, :], ps1[:])
                # pass B: x2 = 2*(S@x1) - x
                for i, s in enumerate(s_list):
                    psb = acc_p.tile([128, 4, U], f32, tag="ps")
                    ps2 = psb[:, :, 0:fin]
                    for j in range(4):
                        for m in range(4):
                            nc.tensor.matmul(
                                ps2[:, j, :], supT_ap(s, m, j),
                                nm[s][:, m, 1 + 2 * i, :],
                                start=(m == 0), stop=(m == 3),
                                skip_group_check=True,
                            )
                    cp(cp_rot[(i + 1) % len(cp_rot)],
                       nm[s][:, :, 2 + 2 * i, :], ps2[:])

            def fm_transpose(nm_ap_fn, rows, tag, eng):
                """Batched FM flip: nm cols (J..J+g, fin) of each m-block ->
                fm tile [rows<=128, 512] (k = J*fin+f, node-major cols)."""
                fmb = sc_p.tile([128, N], f16, tag=tag)
                fm = fmb[0:rows, :]
                ptr = tr_p.tile([128, N], f16, tag="ptr")
                for m in range(4):
                    nc.tensor.transpose(
                        ptr[0:rows, m * 128 : (m + 1) * 128],
                        nm_ap_fn(m),
                        id128[:],
                    )
                cp(eng, fm, ptr[0:rows, :])
                return fm

            # ---- Layer 1 (fin=16) ----
            adv = list(range(8))
            cheb_nm(FL, {s: nm1a for s in adv}, adv,
                    lambda s: nm1a[:, :, 0, :], "psL1", (0, 1))
            cheb_nm(FL, {8: nm1d}, [8],
                    lambda s: nm1d[:, :, 0, :], "psL1", (1, 0))

            # FM stationaries for the L1 GEMM: adv tiles (8+8+1 mats),
            # diff tile (3 mats)
            fm1a = []
            for t in range(2):
                fm1a.append(fm_transpose(
                    lambda m, t=t: nm1a[:, m, 8 * t : 8 * t + 8, :],
                    128, f"fmA{t}", t % 2,
                ))
            fm1a.append(fm_transpose(
                lambda m: nm1a[:, m, 16, :], FL, "fmA2", 0))
            fm1d = fm_transpose(
                lambda m: nm1d[:, m, 0:3, :], 48, "fmA3", 1)

            # L1 GEMM (weights moving): c1 = tanh(xs @ W1 + b1), node-major
            pc1a = acc_p.tile([128, 4, U], f32, tag="ps")
            pc1d = acc_p.tile([128, 4, U], f32, tag="ps")
            for j in range(4):
                for t in range(3):
                    kk = 128 if t < 2 else FL
                    nc.tensor.matmul(
                        pc1a[:, j, :], fm1a[t][0:kk, j * 128 : (j + 1) * 128],
                        wa1_ap(t, kk),
                        start=(t == 0), stop=False, skip_group_check=True,
                    )
                nc.tensor.matmul(
                    pc1a[:, j, :], ones128, b1a,
                    start=False, stop=True, skip_group_check=True,
                )
                nc.tensor.matmul(
                    pc1d[:, j, :], fm1d[:, j * 128 : (j + 1) * 128], wd1_ap,
                    start=True, stop=False, skip_group_check=True,
                )
                nc.tensor.matmul(
                    pc1d[:, j, :], ones128, b1d,
                    start=False, stop=True, skip_group_check=True,
                )
            nc.scalar.activation(nm2a[:, :, 0, :], pc1a[:], AF.Tanh)
            nc.scalar.activation(nm2d[:, :, 0, :], pc1d[:], AF.Tanh)

            # ---- Layer 2 (fin=64); diff first so its grad chain overlaps ----
            cheb_nm(U, {8: nm2d}, [8],
                    lambda s: nm2d[:, :, 0, :], "psL2", (1, 0))
            cheb_nm(U, {s: nm2a for s in adv}, adv,
                    lambda s: nm2a[:, :, 0, :], "psL2", (0, 1))

            # FM stationaries for the L2 GEMM: diff (2 tiles), adv (9 tiles)
            fm2d = []
            fm2d.append(fm_transpose(
                lambda m: nm2d[:, m, 0:2, :], 128, "fmA0", 0))
            fm2d.append(fm_transpose(
                lambda m: nm2d[:, m, 2, :], U, "fmA1", 1))
            fm2a = []
            for t in range(8):
                fm2a.append(fm_transpose(
                    lambda m, t=t: nm2a[:, m, 2 * t : 2 * t + 2, :],
                    128, f"fmA{t+2}", (0, 1)[t % 2],
                ))
            fm2a.append(fm_transpose(
                lambda m: nm2a[:, m, 16, :], U, "fmA10", 1))

            # L2 GEMMs -> grads node-major [128, 4, 16] (the agin layout).
            # W2/b2 are host-negated so tanh lands the sign; diff still
            # needs the 0.1 coefficient.
            g_st = fus_p.tile([128, 2, 4, FL], f16, tag="gst")
            pgdb = acc_p.tile([128, 4, U], f32, tag="ps")
            pgd = pgdb[:, :, 0:FL]
            for j in range(4):
                for t in range(2):
                    kk = 128 if t < 1 else U
                    nc.tensor.matmul(
                        pgd[:, j, :], fm2d[t][0:kk, j * 128 : (j + 1) * 128],
                        wd2_ap(t, kk),
                        start=(t == 0), stop=False, skip_group_check=True,
                    )
                nc.tensor.matmul(
                    pgd[:, j, :], ones128, b2d,
                    start=False, stop=True, skip_group_check=True,
                )
            gd_t = sc_p.tile([128, 4, FL], f16, tag="gdt")
            nc.scalar.activation(gd_t[:], pgd[:], AF.Tanh)
            nc.vector.tensor_scalar_mul(g_st[:, 0, :, :], gd_t[:], COEFF)

            pgab = acc_p.tile([128, 4, U], f32, tag="ps")
            pga = pgab[:, :, 0:FL]
            for j in range(4):
                for t in range(9):
                    kk = 128 if t < 8 else U
                    nc.tensor.matmul(
                        pga[:, j, :], fm2a[t][0:kk, j * 128 : (j + 1) * 128],
                        wa2_ap(t, kk),
                        start=(t == 0), stop=False, skip_group_check=True,
                    )
                nc.tensor.matmul(
                    pga[:, j, :], ones128, b2a,
                    start=False, stop=True, skip_group_check=True,
                )
            nc.scalar.activation(g_st[:, 1, :, :], pga[:], AF.Tanh)

            # ---- AllGather of node-major grads: agin[r, p, m, f] ----
            agin = dram_p.tile([2, 128, 4, FL], f16)
            agout = dram_p.tile([NCORES, 2, 128, 4, FL], f16)
            nc.sync.dma_start(agin[0].rearrange("p m f -> p (m f)"),
                              g_st[:, 0].rearrange("p m f -> p (m f)"))
            nc.scalar.dma_start(agin[1].rearrange("p m f -> p (m f)"),
                                g_st[:, 1].rearrange("p m f -> p (m f)"))
            if collective:
                nc.gpsimd.collective_compute(
                    "AllGather",
                    ALU.bypass,
                    replica_groups=[list(range(NCORES))],
                    ins=[agin.opt()],
                    outs=[agout.opt()],
                )
            else:
                for r in range(NCORES):
                    nc.gpsimd.dma_start(agout[r], agin[:])

            # ---- W_f phase ----
            # Gathered grads land directly in the stationary layout:
            # gt_all[q, col, kt] with kt = m*16+f <-> k = (m*128+q)*16+f;
            # wt is host-permuted to the same enumeration. Diff grads ->
            # cols 0-7, adv -> cols 32-39 (3-dim APs, contiguous last dim).
            nc.sync.dma_start(
                gt_all[:, 0:8, :],
                agout[:, 0].rearrange("c p m f -> p c (m f)"),
            )
            nc.scalar.dma_start(
                gt_all[:, 32:40, :],
                agout[:, 1].rearrange("c p m f -> p c (m f)"),
            )

            # Half 1's GEMM completes first so its fusion chain runs under
            # half 2's GEMM; each half is 64 matmuls + a bias row.
            def fusion(ps, h):
                # only one PSUM operand allowed per DVE op -> stage X_adv
                xa = fu_p.tile([B, 512], f16, tag="fu")
                nc.scalar.copy(xa[:], ps[32 : 32 + B, :])
                ssum = fu_p.tile([B, 512], f16, tag="fu")
                nc.vector.tensor_add(ssum[:], ps[0:B, :], xa[:])
                d = fu_p.tile([B, 512], f16, tag="fu")
                nc.vector.tensor_sub(d[:], ps[0:B, :], xa[:])
                z = fu_p.tile([B, 512], f16, tag="fu")
                nc.scalar.activation(z[:], ssum[:], AF.Sigmoid)
                zd = fu_p.tile([B, 512], f16, tag="fu")
                nc.vector.tensor_mul(zd[:], z[:], d[:])
                o = fus_p.tile([B, 512], f32, tag="fo")
                nc.vector.tensor_add(o[:], zd[:], ps[32 : 32 + B, :])
                nc.sync.dma_start(out_d[:, h * 512 : (h + 1) * 512], o[:])

            def fusion_q(ps, q):
                c0 = q * 256
                sl = slice(c0, c0 + 256)
                xa = fu_p.tile([B, 256], f16, tag="fuq")
                nc.scalar.copy(xa[:], ps[32 : 32 + B, sl])
                ssum = fu_p.tile([B, 256], f16, tag="fuq")
                nc.vector.tensor_add(ssum[:], ps[0:B, sl], xa[:])
                d = fu_p.tile([B, 256], f16, tag="fuq")
                nc.vector.tensor_sub(d[:], ps[0:B, sl], xa[:])
                z = fu_p.tile([B, 256], f16, tag="fuq")
                nc.scalar.activation(z[:], ssum[:], AF.Sigmoid)
                zd = fu_p.tile([B, 256], f16, tag="fuq")
                nc.vector.tensor_mul(zd[:], z[:], d[:])
                o = fus_p.tile([B, 256], f32, tag=f"foq{q}")
                nc.vector.tensor_add(o[:], zd[:], ps[32 : 32 + B, sl])
                nc.sync.dma_start(out_d[:, 512 + c0 : 512 + c0 + 256], o[:])

            psX1 = psx_p.tile([40, 512], f32, tag="psX")
            psX2 = psx_p.tile([40, 512], f32, tag="psX")
            for kt in range(KT):
                nc.tensor.matmul(
                    psX1[:], gt_all[:, :, kt], wt_all[:, kt, 0:512],
                    start=(kt == 0), stop=False, skip_group_check=True,
                )
            nc.tensor.matmul(
                psX1[:], ones40[:], bf_ap(0, 512),
                start=False, stop=True, skip_group_check=True,
            )
            fusion(psX1, 0)
            for q, (lo, hi) in enumerate(((512, 768), (768, 1024))):
                for kt in range(KT):
                    nc.tensor.matmul(
                        psX2[:, q * 256 : (q + 1) * 256], gt_all[:, :, kt],
                        wt_all[:, kt, lo:hi],
                        start=(kt == 0), stop=False, skip_group_check=True,
                    )
                nc.tensor.matmul(
                    psX2[:, q * 256 : (q + 1) * 256], ones40[:], bf_ap(lo, hi),
                    start=False, stop=True, skip_group_check=True,
                )
                fusion_q(psX2, q)

    _split_excess_waits(nc)
    return nc


def _prep_in_maps(inputs: dict) -> list[dict]:
    y = np.asarray(inputs["y"], np.float32)
    sd = np.asarray(inputs["supports_diff"], np.float32)
    sa = np.asarray(inputs["supports_adv"], np.float32)
    W_d1 = np.asarray(inputs["W_d1"], np.float32)
    W_d2 = -np.asarray(inputs["W_d2"], np.float32)
    W_a1 = np.asarray(inputs["W_a1"], np.float32)
    W_a2 = -np.asarray(inputs["W_a2"], np.float32)
    W_f = np.asarray(inputs["W_f"], np.float32)
    b_f = np.asarray(inputs["b_f"], np.float32)


    def cheb_fold(W, fin, M):
        # mats become [x0, x1_s, y2_s=S@x1_s]: W'[x0] -= sum W[x2_s];
        # W'[y2_s] = 2 W[x2_s]
        Wf = W.reshape(fin, M, -1).copy()
        for j in range(2, M, 2):
            Wf[:, 0, :] -= Wf[:, j, :]
            Wf[:, j, :] *= 2.0
        return Wf.reshape(fin * M, -1)

    W_a1 = cheb_fold(W_a1, FL, 17)
    W_d1 = cheb_fold(W_d1, FL, 3)
    W_a2 = cheb_fold(W_a2, U, 17)
    W_d2 = cheb_fold(W_d2, U, 3)

    # supports, transposed, node-tile-major, one per tile:
    # supT[b, p, si, m, n] = S_{3b+si}.T[m*128+p, n]
    supT = np.empty((3, 128, 3, 4, N), np.float16)
    for s in range(9):
        Ssrc = sa[s] if s < 8 else sd[0]
        st = Ssrc.T.astype(np.float16)  # [m, n]
        supT[s // 3, :, s % 3] = st.reshape(4, 128, N).transpose(1, 0, 2)

    def perm_pad(W, fin, M, fout, ntiles):
        # reference row (f, m) -> packed row m*fin+f, zero-padded to tiles
        Wp = W.reshape(fin, M, fout).transpose(1, 0, 2).reshape(fin * M, fout)
        pad = np.zeros((ntiles * 128, fout), np.float16)
        pad[: fin * M] = Wp.astype(np.float16)
        return pad.reshape(ntiles, 128, fout)

    wa1 = perm_pad(W_a1, FL, 17, U, 3)
    wd1 = perm_pad(W_d1, FL, 3, U, 1)
    wa2 = perm_pad(W_a2, U, 17, FL, 9)
    wd2 = perm_pad(W_d2, U, 3, FL, 2)

    # wt[q, m*16+f, j] = W_f.T[(m*128+q)*FL + f, c*JS+j]  (kt = m*16+f)
    WT = W_f.T.astype(np.float16)  # [k_orig = n*FL+f, j_global]
    in_maps = []
    for c in range(NCORES):
        x0 = y[c].reshape(N, FL)  # [node, f]
        x0m = x0.reshape(4, 128, FL).transpose(1, 0, 2).astype(np.float16)

        sm16 = np.zeros((128, _SM16), np.float16)
        sm16[:, _OFF_X0M : _OFF_X0M + 64] = x0m.reshape(128, 64)
        sm16[:, _OFF_WA1 : _OFF_WA1 + 3 * U] = wa1.transpose(1, 0, 2).reshape(
            128, 3 * U
        )
        sm16[:, _OFF_WD1 : _OFF_WD1 + U] = wd1[0]
        sm16[:, _OFF_WA2 : _OFF_WA2 + 9 * FL] = wa2.transpose(1, 0, 2).reshape(
            128, 9 * FL
        )
        sm16[:, _OFF_WD2 : _OFF_WD2 + 2 * FL] = wd2.transpose(1, 0, 2).reshape(
            128, 2 * FL
        )
        sm16[0, _OFF_BF : _OFF_BF + JS] = b_f[c * JS : (c + 1) * JS].astype(
            np.float16
        )
        sm16[0, _OFF_B1A : _OFF_B1A + U] = np.asarray(inputs["b_a1"], np.float16)
        sm16[0, _OFF_B1D : _OFF_B1D + U] = np.asarray(inputs["b_d1"], np.float16)
        sm16[0, _OFF_B2A : _OFF_B2A + FL] = -np.asarray(
            inputs["b_a2"], np.float16
        )
        sm16[0, _OFF_B2D : _OFF_B2D + FL] = -np.asarray(
            inputs["b_d2"], np.float16
        )
        sm16[0, _OFF_ONES : _OFF_ONES + 128] = 1.0

        # [(m q f), j] -> [q, m, f, j] -> [q, m*16+f, j]
        wt = np.ascontiguousarray(
            WT[:, c * JS : (c + 1) * JS]
            .reshape(4, 128, FL, JS)
            .transpose(1, 0, 2, 3)
            .reshape(128, KT, JS)
        )
        in_maps.append({"sm16": sm16, "supT": supT, "wt": wt})
    return in_maps


_CACHE: dict = {}


def _get_nc() -> bass.Bass:
    if "nc" not in _CACHE:
        _CACHE["nc"] = _build()
    return _CACHE["nc"]


def run(inputs: dict, trace: bool = False):
    """Run on the 8 cores; returns (full_output, BassKernelResults)."""
    in_maps = _prep_in_maps(inputs)
    nc = _get_nc()
    kw = {}
    if trace:
        kw = dict(trace=True, trace_cores=list(range(NCORES)), stitch_traces=False)
    res = run_bass_kernel_spmd(nc, in_maps, core_ids=list(range(NCORES)), **kw)
    out = np.concatenate(
        [res.results[c]["out"] for c in range(NCORES)], axis=1
    ).astype(np.float32)
    return out, res


def kernel(**inputs) -> np.ndarray:
    out, _ = run(inputs)
    return out


# revision 31
# speedup vs baseline: 1.1866x; 1.1866x over previous
"""Trainium2 Bass kernel for nn_ODEFunc (gnn_message_passing, 8 cores).

Strategy (cost model: matmul = out-free-rows; DMA = free-dim bytes per
queue, 3 queues; collective = 15us + gathered bytes / 40GBps):
  - Batch-parallel branches: core b computes batch b's diff+adv gconv
    branches. All Chebyshev mats are built NODE-major with the support
    as the matmul *stationary* ([128,128] S^T blocks), so each x1/x2
    costs only 16 matmuls x fin rows instead of streaming the 512-wide
    support as moving data.
  - Node-major mats are packed per layer into nm stacks [128, J, 4, fin]
    (mat index J on the free dim -> no partition-start issues). The
    layer GEMM needs feature-major stationaries: batched PE transposes
    flip 8 (L1) / 2 (L2) mats per 128-row k-tile in one psum bank.
  - Layer GEMMs run with the (host-permuted) weights as *moving* data:
    out = c1/grads node-major, 64/16 rows per matmul. Biases are added
    with a ones-row rank-1 matmul into the same psum group.
  - W_d2/b_d2, W_a2/b_a2 are host-negated so tanh emits the grad sign;
    the diff 0.1 coefficient is one DVE op. Grads come out node-major,
    exactly the AllGather staging layout (no grad transposes).
  - AllGather [2,128,4,16] fp16 per core; gathered grads land straight
    in the W_f stationary gt_all[128, 40, KT] via two 3-dim strided
    DMAs (kt = m*16+f; wt is host-permuted to the same k enumeration).
    psX[40, 512]: X_diff rows 0-7, X_adv rows 32-39 (cols 8-31 are
    memset-zero lanes).
  - W_f shard (fp16, 16 MB) is split across the 3 DMA queues sized to
    each queue's idle windows (ACT's chunk is scheduled into the
    collective window via tile_wait_until).
  - GEMM half 1 finishes before half 2 starts so its gated-fusion chain
    hides under half 2's matmuls.
"""

import sys

sys.path.insert(0, "/opt/trn_rl_repo")

import numpy as np

import concourse.bass as bass
import concourse.mybir as mybir
from concourse import masks
from concourse.bass_utils import run_bass_kernel_spmd
from concourse.tile import TileContext
from concourse.vector_clock import ScopedClock

N = 512          # nodes
FL = 16          # latent
U = 64           # units
B = 8            # batch
HID = N * FL     # 8192
COEFF = 0.1
NCORES = 8
JS = HID // NCORES  # 1024 output columns per core
KT = HID // 128     # 64 contraction tiles for the W_f GEMM

f16 = mybir.dt.float16
f32 = mybir.dt.float32
AF = mybir.ActivationFunctionType
ALU = mybir.AluOpType

# smalls_f16 packed free-dim offsets (elements)
_OFF_X0M = 0             # [128, 4*16] x0 node-major
_OFF_WA1 = 64            # 3 tiles [128, 64]
_OFF_WD1 = 256           # [48(->128), 64]
_OFF_WA2 = 320           # 9 tiles [128, 16]
_OFF_WD2 = 464           # 2 tiles [128, 16]
_OFF_BF = 496            # [1, 1024]
_OFF_B1A = 1520          # [1, 64]
_OFF_B1D = 1584          # [1, 64]
_OFF_B2A = 1648          # [1, 16]
_OFF_B2D = 1664          # [1, 16]
_OFF_ONES = 1680         # [1, 128] ones
_SM16 = 1808


class PatchedTileContext(TileContext):
    """Tail drain with at most one sem wait per instruction.

    The walrus build here rejects Drain instructions carrying >2 sync
    waits ("Too many sync wait commands"). Spread the global-clock waits
    over individual SP nops ahead of the drain.
    """

    def _drain_and_barrier(self, tick_clock, wait_clock):
        nc = self.nc
        probe = nc.sync.nop(nofuse=True)
        wait_clock.add_sem_waits(
            probe.ins, ScopedClock({None: tick_clock.global_clock})
        )
        si = probe.ins.sync_info
        ws = list(si.on_wait) if si is not None else []
        if len(ws) > 1:
            probe.ins.sync_info = mybir.SyncInfo(
                on_wait=ws[:1], on_update=list(si.on_update)
            )
            for w in ws[1:]:
                n2 = nc.sync.nop(nofuse=True)
                n2.ins.sync_info = mybir.SyncInfo(on_wait=[w], on_update=[])
        nc.sync.drain()
        nc.all_engine_barrier()
        popped = nc._tile_sem_poison_stack.pop()
        assert popped is self._sem_poison
        nc.clear_and_free_semaphores(list(self.sems.allocated().values()))
        nc.all_engine_barrier()


_WAIT_LIMIT = 1


def _split_excess_waits(nc: bass.Bass) -> None:
    """Move sync waits beyond _WAIT_LIMIT onto same-engine NOPs inserted
    just before the carrying instruction (this walrus build has tiny
    setupSyncWait budgets for DMA/collective/drain instruction formats)."""
    for fn in nc.m.functions:
        for bb in fn.blocks:
            insts = bb.instructions
            i = 0
            while i < len(insts):
                inst = insts[i]
                si = inst.sync_info
                ws = list(si.on_wait) if si is not None and si.on_wait else []
                if len(ws) > _WAIT_LIMIT and type(inst).__name__ != "InstNoOp":
                    keep = ws[:_WAIT_LIMIT]
                    extra = ws[_WAIT_LIMIT:]
                    inst.sync_info = mybir.SyncInfo(
                        on_wait=keep, on_update=list(si.on_update)
                    )
                    for k, w in enumerate(extra):
                        nop = mybir.InstNoOp(
                            name=f"{inst.name}-w{k}",
                            engine=inst.engine,
                            bass_nofuse=True,
                            sync_info=mybir.SyncInfo(on_wait=[w], on_update=[]),
                        )
                        nc.register_instruction(nop, overwrite=True)
                        insts.insert(i, nop)
                        i += 1
                i += 1


def _build(collective: bool = True) -> bass.Bass:
    nc = bass.Bass(num_devices=NCORES)

    # ---- DRAM I/O (per-core values supplied via in_maps) ----
    sm16_d = nc.dram_tensor("sm16", [128, _SM16], f16, kind="ExternalInput")
    sup_d = nc.dram_tensor("supT", [3, 128, 3, 4, N], f16, kind="ExternalInput")
    wt_d = nc.dram_tensor("wt", [128, KT, JS], f16, kind="ExternalInput")
    out_d = nc.dram_tensor("out", [B, JS], f32, kind="ExternalOutput")

    with PatchedTileContext(nc) as tc:
        from contextlib import ExitStack

        with ExitStack() as ctx:
            const_p = ctx.enter_context(tc.tile_pool(name="const", bufs=1))
            sup_p = ctx.enter_context(tc.tile_pool(name="sup", bufs=1))
            sc_p = ctx.enter_context(tc.tile_pool(name="sc", bufs=1))
            fus_p = ctx.enter_context(tc.tile_pool(name="fus", bufs=1))
            fu_p = ctx.enter_context(tc.tile_pool(name="fu", bufs=3))
            acc_p = ctx.enter_context(tc.tile_pool(name="acc", bufs=3, space="PSUM"))
            psx_p = ctx.enter_context(tc.tile_pool(name="psx", bufs=2, space="PSUM"))
            tr_p = ctx.enter_context(tc.tile_pool(name="tr", bufs=2, space="PSUM"))
            dram_p = ctx.enter_context(tc.tile_pool(name="dram", bufs=1, space="DRAM"))

            # ---- constants / memsets ----
            id128 = const_p.tile([128, 128], f16, tag="id")
            masks.make_identity(nc, id128[:])
            ones40 = const_p.tile([1, 40], f16, tag="ones")
            nc.vector.memset(ones40[:], 1.0)
            # W_f-GEMM stationary [q, col, kt]; cols 8-31 are never DMA'd
            # -> zero them once, early (garbage would NaN the psum)
            gt_all = const_p.tile([128, 40, KT], f16, tag="gt")
            nc.gpsimd.memset(gt_all[:], 0.0)

            # ---- input DMAs: smalls first; supports split SP/Pool so the
            # last support lands by ~9.5us ----
            sm16 = const_p.tile([128, _SM16], f16, tag="sm16")
            nc.sync.dma_start(sm16[:], sm16_d[:])
            sup_tiles = []
            for s in range(9):
                supb = sup_p.tile([128, 4, N], f16, tag=f"sup{s}")
                sup_tiles.append(supb)
            for s in (0, 2, 4, 6, 8):
                nc.sync.dma_start(sup_tiles[s][:], sup_d[s // 3][:, s % 3])
            for s in (1, 3, 5, 7):
                nc.gpsimd.dma_start(sup_tiles[s][:], sup_d[s // 3][:, s % 3])

            # W_f shard: SP streams most of it through the branch phase;
            # ACT's chunk is scheduled into the collective window. Pool
            # carries none (it does late-branch copies + the collective).
            wt_all = const_p.tile([128, KT, JS], f16, tag="wt")
            nc.sync.dma_start(wt_all[:, 0:40, :], wt_d[:, 0:40, :])
            nc.sync.dma_start(wt_all[:, 40:48, :], wt_d[:, 40:48, :])
            with tc.tile_wait_until(0.055):
                nc.scalar.dma_start(wt_all[:, 48:64, :], wt_d[:, 48:64, :])

            # packed-small views
            x0m_all = sm16[:, _OFF_X0M : _OFF_X0M + 64]

            def x0m_ap(m):
                return sm16[:, _OFF_X0M + m * FL : _OFF_X0M + (m + 1) * FL]

            def wa1_ap(t, k=128):
                return sm16[0:k, _OFF_WA1 + t * U : _OFF_WA1 + (t + 1) * U]

            wd1_ap = sm16[0:48, _OFF_WD1 : _OFF_WD1 + U]

            def wa2_ap(t, k=128):
                return sm16[0:k, _OFF_WA2 + t * FL : _OFF_WA2 + (t + 1) * FL]

            def wd2_ap(t, k=128):
                return sm16[0:k, _OFF_WD2 + t * FL : _OFF_WD2 + (t + 1) * FL]

            def bf_ap(lo, hi):
                return sm16[0:1, _OFF_BF + lo : _OFF_BF + hi]

            b1a = sm16[0:1, _OFF_B1A : _OFF_B1A + U]
            b1d = sm16[0:1, _OFF_B1D : _OFF_B1D + U]
            b2a = sm16[0:1, _OFF_B2A : _OFF_B2A + FL]
            b2d = sm16[0:1, _OFF_B2D : _OFF_B2D + FL]
            ones128 = sm16[0:1, _OFF_ONES : _OFF_ONES + 128]

            # node-major mat stacks [128, J, m, fin]; J: 0=x0/c1, then
            # x1_s at 1+2s, x2_s at 2+2s (the reference concat order)
            nm1a = const_p.tile([128, 4, 17, FL], f16, tag="nm1a")
            nm1d = const_p.tile([128, 4, 3, FL], f16, tag="nm1d")
            nm2a = const_p.tile([128, 4, 17, U], f16, tag="nm2a")
            nm2d = const_p.tile([128, 4, 3, U], f16, tag="nm2d")

            nc.vector.tensor_copy(nm1a[:, :, 0, :], x0m_all)
            nc.scalar.copy(nm1d[:, :, 0, :], x0m_all)

            def supT_ap(s, m, j):
                # S_s^T[m-block, j-block] = stationary for out node-block j
                return sup_tiles[s][:, m, j * 128 : (j + 1) * 128]

            # copy/stt helpers: 0 = DVE, 1 = ACT (copy only), 2 = Pool
            def cp(which, dst, src):
                if which == 1:
                    nc.scalar.copy(dst, src)
                elif which == 2:
                    nc.gpsimd.tensor_copy(dst, src)
                else:
                    nc.vector.tensor_copy(dst, src)

            def cheb_nm(fin, nm, s_list, x_src, pool_tag, cp_rot):
                """x1 = S@x, x2 = 2*S@x1 - x in node-major form.

                x_src(s): [128, 4, fin] node-major input (x0 or c1).
                Writes nm[:, 1+2i, :, :] and nm[:, 2+2i, :, :] for s_list[i].
                """
                # pass A: x1 for every support
                for i, s in enumerate(s_list):
                    psb = acc_p.tile([128, 4, U], f32, tag="ps")
                    ps1 = psb[:, :, 0:fin]
                    for j in range(4):
                        for m in range(4):
                            nc.tensor.matmul(
                                ps1[:, j, :], supT_ap(s, m, j),
                                x_src(s)[:, m, :],
                                start=(m == 0), stop=(m == 3),
                                skip_group_check=True,
                            )
                    cp(cp_rot[i % len(cp_rot)], nm[s][:, :, 1 + 2 * i, :], ps1[:])
                # pass B: x2 = 2*(S@x1) - x
                for i, s in enumerate(s_list):
                    psb = acc_p.tile([128, 4, U], f32, tag="ps")
                    ps2 = psb[:, :, 0:fin]
                    for j in range(4):
                        for m in range(4):
                            nc.tensor.matmul(
                                ps2[:, j, :], supT_ap(s, m, j),
                                nm[s][:, m, 1 + 2 * i, :],
                                start=(m == 0), stop=(m == 3),
                                skip_group_check=True,
                            )
                    cp(cp_rot[(i + 1) % len(cp_rot)],
                       nm[s][:, :, 2 + 2 * i, :], ps2[:])

            def fm_transpose(nm_ap_fn, rows, tag, eng):
                """Batched FM flip: nm cols (J..J+g, fin) of each m-block ->
                fm tile [rows<=128, 512] (k = J*fin+f, node-major cols)."""
                fmb = sc_p.tile([128, N], f16, tag=tag)
                fm = fmb[0:rows, :]
                ptr = tr_p.tile([128, N], f16, tag="ptr")
                for m in range(4):
                    nc.tensor.transpose(
                        ptr[0:rows, m * 128 : (m + 1) * 128],
                        nm_ap_fn(m),
                        id128[:],
                    )
                cp(eng, fm, ptr[0:rows, :])
                return fm

            # ---- Layer 1 (fin=16) ----
            adv = list(range(8))
            cheb_nm(FL, {s: nm1a for s in adv}, adv,
                    lambda s: nm1a[:, :, 0, :], "psL1", (0, 1))
            cheb_nm(FL, {8: nm1d}, [8],
                    lambda s: nm1d[:, :, 0, :], "psL1", (1, 0))

            # FM stationaries for the L1 GEMM: adv tiles (8+8+1 mats),
            # diff tile (3 mats)
            fm1a = []
            for t in range(2):
                fm1a.append(fm_transpose(
                    lambda m, t=t: nm1a[:, m, 8 * t : 8 * t + 8, :],
                    128, f"fmA{t}", t % 2,
                ))
            fm1a.append(fm_transpose(
                lambda m: nm1a[:, m, 16, :], FL, "fmA2", 0))
            fm1d = fm_transpose(
                lambda m: nm1d[:, m, 0:3, :], 48, "fmA3", 1)

            # L1 GEMM (weights moving): c1 = tanh(xs @ W1 + b1), node-major
            pc1a = acc_p.tile([128, 4, U], f32, tag="ps")
            pc1d = acc_p.tile([128, 4, U], f32, tag="ps")
            for j in range(4):
                for t in range(3):
                    kk = 128 if t < 2 else FL
                    nc.tensor.matmul(
                        pc1a[:, j, :], fm1a[t][0:kk, j * 128 : (j + 1) * 128],
                        wa1_ap(t, kk),
                        start=(t == 0), stop=False, skip_group_check=True,
                    )
                nc.tensor.matmul(
                    pc1a[:, j, :], ones128, b1a,
                    start=False, stop=True, skip_group_check=True,
                )
                nc.tensor.matmul(
                    pc1d[:, j, :], fm1d[:, j * 128 : (j + 1) * 128], wd1_ap,
                    start=True, stop=False, skip_group_check=True,
                )
                nc.tensor.matmul(
                    pc1d[:, j, :], ones128, b1d,
                    start=False, stop=True, skip_group_check=True,
                )
            nc.scalar.activation(nm2a[:, :, 0, :], pc1a[:], AF.Tanh)
            nc.scalar.activation(nm2d[:, :, 0, :], pc1d[:], AF.Tanh)

            # ---- Layer 2 (fin=64); diff first so its grad chain overlaps ----
            cheb_nm(U, {8: nm2d}, [8],
                    lambda s: nm2d[:, :, 0, :], "psL2", (1, 0))
            cheb_nm(U, {s: nm2a for s in adv}, adv,
                    lambda s: nm2a[:, :, 0, :], "psL2", (0, 1))

            # FM stationaries for the L2 GEMM: diff (2 tiles), adv (9 tiles)
            fm2d = []
            fm2d.append(fm_transpose(
                lambda m: nm2d[:, m, 0:2, :], 128, "fmA0", 0))
            fm2d.append(fm_transpose(
                lambda m: nm2d[:, m, 2, :], U, "fmA1", 1))
            fm2a = []
            for t in range(8):
                fm2a.append(fm_transpose(
                    lambda m, t=t: nm2a[:, m, 2 * t : 2 * t + 2, :],
                    128, f"fmA{t+2}", (0, 1)[t % 2],
                ))
            fm2a.append(fm_transpose(
                lambda m: nm2a[:, m, 16, :], U, "fmA10", 1))

            # L2 GEMMs -> grads node-major [128, 4, 16] (the agin layout).
            # W2/b2 are host-negated so tanh lands the sign; diff still
            # needs the 0.1 coefficient.
            g_st = fus_p.tile([128, 2, 4, FL], f16, tag="gst")
            pgdb = acc_p.tile([128, 4, U], f32, tag="ps")
            pgd = pgdb[:, :, 0:FL]
            for j in range(4):
                for t in range(2):
                    kk = 128 if t < 1 else U
                    nc.tensor.matmul(
                        pgd[:, j, :], fm2d[t][0:kk, j * 128 : (j + 1) * 128],
                        wd2_ap(t, kk),
                        start=(t == 0), stop=False, skip_group_check=True,
                    )
                nc.tensor.matmul(
                    pgd[:, j, :], ones128, b2d,
                    start=False, stop=True, skip_group_check=True,
                )
            gd_t = sc_p.tile([128, 4, FL], f16, tag="gdt")
            nc.scalar.activation(gd_t[:], pgd[:], AF.Tanh)
            nc.vector.tensor_scalar_mul(g_st[:, 0, :, :], gd_t[:], COEFF)

            pgab = acc_p.tile([128, 4, U], f32, tag="ps")
            pga = pgab[:, :, 0:FL]
            for j in range(4):
                for t in range(9):
                    kk = 128 if t < 8 else U
                    nc.tensor.matmul(
                        pga[:, j, :], fm2a[t][0:kk, j * 128 : (j + 1) * 128],
                        wa2_ap(t, kk),
                        start=(t == 0), stop=False, skip_group_check=True,
                    )
                nc.tensor.matmul(
                    pga[:, j, :], ones128, b2a,
                    start=False, stop=True, skip_group_check=True,
                )
            nc.scalar.activation(g_st[:, 1, :, :], pga[:], AF.Tanh)

            # ---- AllGather of node-major grads: agin[r, p, m, f] ----
            agin = dram_p.tile([2, 128, 4, FL], f16)
            agout = dram_p.tile([NCORES, 2, 128, 4, FL], f16)
            nc.scalar.dma_start(agin[0].rearrange("p m f -> p (m f)"),
                                g_st[:, 0].rearrange("p m f -> p (m f)"))
            nc.scalar.dma_start(agin[1].rearrange("p m f -> p (m f)"),
                                g_st[:, 1].rearrange("p m f -> p (m f)"))
            if collective:
                nc.gpsimd.collective_compute(
                    "AllGather",
                    ALU.bypass,
                    replica_groups=[list(range(NCORES))],
                    ins=[agin.opt()],
                    outs=[agout.opt()],
                )
            else:
                for r in range(NCORES):
                    nc.gpsimd.dma_start(agout[r], agin[:])

            # ---- W_f phase ----
            # Gathered grads land directly in the stationary layout:
            # gt_all[q, col, kt] with kt = m*16+f <-> k = (m*128+q)*16+f;
            # wt is host-permuted to the same enumeration. Diff grads ->
            # cols 0-7, adv -> cols 32-39 (3-dim APs, contiguous last dim).
            nc.sync.dma_start(
                gt_all[:, 0:8, :],
                agout[:, 0].rearrange("c p m f -> p c (m f)"),
            )
            nc.gpsimd.dma_start(
                gt_all[:, 32:40, :],
                agout[:, 1].rearrange("c p m f -> p c (m f)"),
            )

            # Half 1's GEMM completes first so its fusion chain runs under
            # half 2's GEMM; each half is 64 matmuls + a bias row.
            def fusion(ps, h):
                # only one PSUM operand allowed per DVE op -> stage X_adv
                xa = fu_p.tile([B, 512], f16, tag="fu")
                nc.scalar.copy(xa[:], ps[32 : 32 + B, :])
                ssum = fu_p.tile([B, 512], f16, tag="fu")
                nc.vector.tensor_add(ssum[:], ps[0:B, :], xa[:])
                d = fu_p.tile([B, 512], f16, tag="fu")
                nc.vector.tensor_sub(d[:], ps[0:B, :], xa[:])
                z = fu_p.tile([B, 512], f16, tag="fu")
                nc.scalar.activation(z[:], ssum[:], AF.Sigmoid)
                zd = fu_p.tile([B, 512], f16, tag="fu")
                nc.vector.tensor_mul(zd[:], z[:], d[:])
                o = fus_p.tile([B, 512], f32, tag="fo")
                nc.vector.tensor_add(o[:], zd[:], ps[32 : 32 + B, :])
                nc.sync.dma_start(out_d[:, h * 512 : (h + 1) * 512], o[:])

            def fusion_q(ps, q):
                c0 = q * 256
                sl = slice(c0, c0 + 256)
                xa = fu_p.tile([B, 256], f16, tag="fuq")
                nc.scalar.copy(xa[:], ps[32 : 32 + B, sl])
                ssum = fu_p.tile([B, 256], f16, tag="fuq")
                nc.vector.tensor_add(ssum[:], ps[0:B, sl], xa[:])
                d = fu_p.tile([B, 256], f16, tag="fuq")
                nc.vector.tensor_sub(d[:], ps[0:B, sl], xa[:])
                z = fu_p.tile([B, 256], f16, tag="fuq")
                nc.scalar.activation(z[:], ssum[:], AF.Sigmoid)
                zd = fu_p.tile([B, 256], f16, tag="fuq")
                nc.vector.tensor_mul(zd[:], z[:], d[:])
                o = fus_p.tile([B, 256], f32, tag=f"foq{q}")
                nc.vector.tensor_add(o[:], zd[:], ps[32 : 32 + B, sl])
                nc.sync.dma_start(out_d[:, 512 + c0 : 512 + c0 + 256], o[:])

            psX1 = psx_p.tile([40, 512], f32, tag="psX")
            psX2 = psx_p.tile([40, 512], f32, tag="psX")
            for kt in range(KT):
                nc.tensor.matmul(
                    psX1[:], gt_all[:, :, kt], wt_all[:, kt, 0:512],
                    start=(kt == 0), stop=False, skip_group_check=True,
                )
            nc.tensor.matmul(
                psX1[:], ones40[:], bf_ap(0, 512),
                start=False, stop=True, skip_group_check=True,
            )
            fusion(psX1, 0)
            for q, (lo, hi) in enumerate(((512, 768), (768, 1024))):
                for kt in range(KT):
                    nc.tensor.matmul(
                        psX2[:, q * 256 : (q + 1) * 256], gt_all[:, :, kt],
                        wt_all[:, kt, lo:hi],
                        start=(kt == 0), stop=False, skip_group_check=True,
                    )
                nc.tensor.matmul(
                    psX2[:, q * 256 : (q + 1) * 256], ones40[:], bf_ap(lo, hi),
                    start=False, stop=True, skip_group_check=True,
                )
                fusion_q(psX2, q)

    _split_excess_waits(nc)
    return nc


def _prep_in_maps(inputs: dict) -> list[dict]:
    y = np.asarray(inputs["y"], np.float32)
    sd = np.asarray(inputs["supports_diff"], np.float32)
    sa = np.asarray(inputs["supports_adv"], np.float32)
    W_d1 = np.asarray(inputs["W_d1"], np.float32)
    W_d2 = -np.asarray(inputs["W_d2"], np.float32)
    W_a1 = np.asarray(inputs["W_a1"], np.float32)
    W_a2 = -np.asarray(inputs["W_a2"], np.float32)
    W_f = np.asarray(inputs["W_f"], np.float32)
    b_f = np.asarray(inputs["b_f"], np.float32)


    def cheb_fold(W, fin, M):
        # mats become [x0, x1_s, y2_s=S@x1_s]: W'[x0] -= sum W[x2_s];
        # W'[y2_s] = 2 W[x2_s]
        Wf = W.reshape(fin, M, -1).copy()
        for j in range(2, M, 2):
            Wf[:, 0, :] -= Wf[:, j, :]
            Wf[:, j, :] *= 2.0
        return Wf.reshape(fin * M, -1)

    W_a1 = cheb_fold(W_a1, FL, 17)
    W_d1 = cheb_fold(W_d1, FL, 3)
    W_a2 = cheb_fold(W_a2, U, 17)
    W_d2 = cheb_fold(W_d2, U, 3)

    # supports, transposed, node-tile-major, one per tile:
    # supT[b, p, si, m, n] = S_{3b+si}.T[m*128+p, n]
    supT = np.empty((3, 128, 3, 4, N), np.float16)
    for s in range(9):
        Ssrc = sa[s] if s < 8 else sd[0]
        st = Ssrc.T.astype(np.float16)  # [m, n]
        supT[s // 3, :, s % 3] = st.reshape(4, 128, N).transpose(1, 0, 2)

    def perm_pad(W, fin, M, fout, ntiles):
        # reference row (f, m) -> packed row m*fin+f, zero-padded to tiles
        Wp = W.reshape(fin, M, fout).transpose(1, 0, 2).reshape(fin * M, fout)
        pad = np.zeros((ntiles * 128, fout), np.float16)
        pad[: fin * M] = Wp.astype(np.float16)
        return pad.reshape(ntiles, 128, fout)

    wa1 = perm_pad(W_a1, FL, 17, U, 3)
    wd1 = perm_pad(W_d1, FL, 3, U, 1)
    wa2 = perm_pad(W_a2, U, 17, FL, 9)
    wd2 = perm_pad(W_d2, U, 3, FL, 2)

    # wt[q, m*16+f, j] = W_f.T[(m*128+q)*FL + f, c*JS+j]  (kt = m*16+f)
    WT = W_f.T.astype(np.float16)  # [k_orig = n*FL+f, j_global]
    in_maps = []
    for c in range(NCORES):
        x0 = y[c].reshape(N, FL)  # [node, f]
        x0m = x0.reshape(4, 128, FL).transpose(1, 0, 2).astype(np.float16)

        sm16 = np.zeros((128, _SM16), np.float16)
        sm16[:, _OFF_X0M : _OFF_X0M + 64] = x0m.reshape(128, 64)
        sm16[:, _OFF_WA1 : _OFF_WA1 + 3 * U] = wa1.transpose(1, 0, 2).reshape(
            128, 3 * U
        )
        sm16[:, _OFF_WD1 : _OFF_WD1 + U] = wd1[0]
        sm16[:, _OFF_WA2 : _OFF_WA2 + 9 * FL] = wa2.transpose(1, 0, 2).reshape(
            128, 9 * FL
        )
        sm16[:, _OFF_WD2 : _OFF_WD2 + 2 * FL] = wd2.transpose(1, 0, 2).reshape(
            128, 2 * FL
        )
        sm16[0, _OFF_BF : _OFF_BF + JS] = b_f[c * JS : (c + 1) * JS].astype(
            np.float16
        )
        sm16[0, _OFF_B1A : _OFF_B1A + U] = np.asarray(inputs["b_a1"], np.float16)
        sm16[0, _OFF_B1D : _OFF_B1D + U] = np.asarray(inputs["b_d1"], np.float16)
        sm16[0, _OFF_B2A : _OFF_B2A + FL] = -np.asarray(
            inputs["b_a2"], np.float16
        )
        sm16[0, _OFF_B2D : _OFF_B2D + FL] = -np.asarray(
            inputs["b_d2"], np.float16
        )
        sm16[0, _OFF_ONES : _OFF_ONES + 128] = 1.0

        # [(m q f), j] -> [q, m, f, j] -> [q, m*16+f, j]
        wt = np.ascontiguousarray(
            WT[:, c * JS : (c + 1) * JS]
            .reshape(4, 128, FL, JS)
            .transpose(1, 0, 2, 3)
            .reshape(128, KT, JS)
        )
        in_maps.append({"sm16": sm16, "supT": supT, "wt": wt})
    return in_maps


_CACHE: dict = {}


def _get_nc() -> bass.Bass:
    if "nc" not in _CACHE:
        _CACHE["nc"] = _build()
    return _CACHE["nc"]


def run(inputs: dict, trace: bool = False):
    """Run on the 8 cores; returns (full_output, BassKernelResults)."""
    in_maps = _prep_in_maps(inputs)
    nc = _get_nc()
    kw = {}
    if trace:
        kw = dict(trace=True, trace_cores=list(range(NCORES)), stitch_traces=False)
    res = run_bass_kernel_spmd(nc, in_maps, core_ids=list(range(NCORES)), **kw)
    out = np.concatenate(
        [res.results[c]["out"] for c in range(NCORES)], axis=1
    ).astype(np.float32)
    return out, res


def kernel(**inputs) -> np.ndarray:
    out, _ = run(inputs)
    return out


# revision 32
# speedup vs baseline: 1.1915x; 1.0041x over previous
"""Trainium2 Bass kernel for nn_ODEFunc (gnn_message_passing, 8 cores).

Strategy (cost model: matmul = out-free-rows; DMA = free-dim bytes per
queue, 3 queues; collective = 15us + gathered bytes / 40GBps):
  - Batch-parallel branches: core b computes batch b's diff+adv gconv
    branches. All Chebyshev mats are built NODE-major with the support
    as the matmul *stationary* ([128,128] S^T blocks), so each x1/x2
    costs only 16 matmuls x fin rows instead of streaming the 512-wide
    support as moving data.
  - Node-major mats are packed per layer into nm stacks [128, J, 4, fin]
    (mat index J on the free dim -> no partition-start issues). The
    layer GEMM needs feature-major stationaries: batched PE transposes
    flip 8 (L1) / 2 (L2) mats per 128-row k-tile in one psum bank.
  - Layer GEMMs run with the (host-permuted) weights as *moving* data:
    out = c1/grads node-major, 64/16 rows per matmul. Biases are added
    with a ones-row rank-1 matmul into the same psum group.
  - W_d2/b_d2, W_a2/b_a2 are host-negated so tanh emits the grad sign;
    the diff 0.1 coefficient is one DVE op. Grads come out node-major,
    exactly the AllGather staging layout (no grad transposes).
  - AllGather [2,128,4,16] fp16 per core; gathered grads land straight
    in the W_f stationary gt_all[128, 40, KT] via two 3-dim strided
    DMAs (kt = m*16+f; wt is host-permuted to the same k enumeration).
    psX[40, 512]: X_diff rows 0-7, X_adv rows 32-39 (cols 8-31 are
    memset-zero lanes).
  - W_f shard (fp16, 16 MB) is split across the 3 DMA queues sized to
    each queue's idle windows (ACT's chunk is scheduled into the
    collective window via tile_wait_until).
  - GEMM half 1 finishes before half 2 starts so its gated-fusion chain
    hides under half 2's matmuls.
"""

import sys

sys.path.insert(0, "/opt/trn_rl_repo")

import numpy as np

import concourse.bass as bass
import concourse.mybir as mybir
from concourse import masks
from concourse.bass_utils import run_bass_kernel_spmd
from concourse.tile import TileContext
from concourse.vector_clock import ScopedClock

N = 512          # nodes
FL = 16          # latent
U = 64           # units
B = 8            # batch
HID = N * FL     # 8192
COEFF = 0.1
NCORES = 8
JS = HID // NCORES  # 1024 output columns per core
KT = HID // 128     # 64 contraction tiles for the W_f GEMM

f16 = mybir.dt.float16
f32 = mybir.dt.float32
AF = mybir.ActivationFunctionType
ALU = mybir.AluOpType

# smalls_f16 packed free-dim offsets (elements)
_OFF_X0M = 0             # [128, 4*16] x0 node-major
_OFF_WA1 = 64            # 3 tiles [128, 64]
_OFF_WD1 = 256           # [48(->128), 64]
_OFF_WA2 = 320           # 9 tiles [128, 16]
_OFF_WD2 = 464           # 2 tiles [128, 16]
_OFF_BF = 496            # [1, 1024]
_OFF_B1A = 1520          # [1, 64]
_OFF_B1D = 1584          # [1, 64]
_OFF_B2A = 1648          # [1, 16]
_OFF_B2D = 1664          # [1, 16]
_OFF_ONES = 1680         # [1, 128] ones
_SM16 = 1808


class PatchedTileContext(TileContext):
    """Tail drain with at most one sem wait per instruction.

    The walrus build here rejects Drain instructions carrying >2 sync
    waits ("Too many sync wait commands"). Spread the global-clock waits
    over individual SP nops ahead of the drain.
    """

    def _drain_and_barrier(self, tick_clock, wait_clock):
        nc = self.nc
        probe = nc.sync.nop(nofuse=True)
        wait_clock.add_sem_waits(
            probe.ins, ScopedClock({None: tick_clock.global_clock})
        )
        si = probe.ins.sync_info
        ws = list(si.on_wait) if si is not None else []
        if len(ws) > 1:
            probe.ins.sync_info = mybir.SyncInfo(
                on_wait=ws[:1], on_update=list(si.on_update)
            )
            for w in ws[1:]:
                n2 = nc.sync.nop(nofuse=True)
                n2.ins.sync_info = mybir.SyncInfo(on_wait=[w], on_update=[])
        nc.sync.drain()
        nc.all_engine_barrier()
        popped = nc._tile_sem_poison_stack.pop()
        assert popped is self._sem_poison
        nc.clear_and_free_semaphores(list(self.sems.allocated().values()))
        nc.all_engine_barrier()


_WAIT_LIMIT = 1


def _split_excess_waits(nc: bass.Bass) -> None:
    """Move sync waits beyond _WAIT_LIMIT onto same-engine NOPs inserted
    just before the carrying instruction (this walrus build has tiny
    setupSyncWait budgets for DMA/collective/drain instruction formats)."""
    for fn in nc.m.functions:
        for bb in fn.blocks:
            insts = bb.instructions
            i = 0
            while i < len(insts):
                inst = insts[i]
                si = inst.sync_info
                ws = list(si.on_wait) if si is not None and si.on_wait else []
                if len(ws) > _WAIT_LIMIT and type(inst).__name__ != "InstNoOp":
                    keep = ws[:_WAIT_LIMIT]
                    extra = ws[_WAIT_LIMIT:]
                    inst.sync_info = mybir.SyncInfo(
                        on_wait=keep, on_update=list(si.on_update)
                    )
                    for k, w in enumerate(extra):
                        nop = mybir.InstNoOp(
                            name=f"{inst.name}-w{k}",
                            engine=inst.engine,
                            bass_nofuse=True,
                            sync_info=mybir.SyncInfo(on_wait=[w], on_update=[]),
                        )
                        nc.register_instruction(nop, overwrite=True)
                        insts.insert(i, nop)
                        i += 1
                i += 1


def _build(collective: bool = True) -> bass.Bass:
    nc = bass.Bass(num_devices=NCORES)

    # ---- DRAM I/O (per-core values supplied via in_maps) ----
    sm16_d = nc.dram_tensor("sm16", [128, _SM16], f16, kind="ExternalInput")
    sup_d = nc.dram_tensor("supT", [3, 128, 3, 4, N], f16, kind="ExternalInput")
    wt_d = nc.dram_tensor("wt", [128, KT, JS], f16, kind="ExternalInput")
    out_d = nc.dram_tensor("out", [B, JS], f32, kind="ExternalOutput")

    with PatchedTileContext(nc) as tc:
        from contextlib import ExitStack

        with ExitStack() as ctx:
            const_p = ctx.enter_context(tc.tile_pool(name="const", bufs=1))
            sup_p = ctx.enter_context(tc.tile_pool(name="sup", bufs=1))
            sc_p = ctx.enter_context(tc.tile_pool(name="sc", bufs=1))
            fus_p = ctx.enter_context(tc.tile_pool(name="fus", bufs=1))
            fu_p = ctx.enter_context(tc.tile_pool(name="fu", bufs=3))
            acc_p = ctx.enter_context(tc.tile_pool(name="acc", bufs=3, space="PSUM"))
            psx_p = ctx.enter_context(tc.tile_pool(name="psx", bufs=2, space="PSUM"))
            tr_p = ctx.enter_context(tc.tile_pool(name="tr", bufs=2, space="PSUM"))
            dram_p = ctx.enter_context(tc.tile_pool(name="dram", bufs=1, space="DRAM"))

            # ---- constants / memsets ----
            id128 = const_p.tile([128, 128], f16, tag="id")
            masks.make_identity(nc, id128[:])
            ones40 = const_p.tile([1, 40], f16, tag="ones")
            nc.vector.memset(ones40[:], 1.0)
            # W_f-GEMM stationary [q, col, kt]; cols 8-31 are never DMA'd
            # -> zero them once, early (garbage would NaN the psum)
            gt_all = const_p.tile([128, 40, KT], f16, tag="gt")
            nc.gpsimd.memset(gt_all[:], 0.0)

            # ---- input DMAs: smalls first; supports split SP/Pool so the
            # last support lands by ~9.5us ----
            sm16 = const_p.tile([128, _SM16], f16, tag="sm16")
            nc.sync.dma_start(sm16[:], sm16_d[:])
            sup_tiles = []
            for s in range(9):
                supb = sup_p.tile([128, 4, N], f16, tag=f"sup{s}")
                sup_tiles.append(supb)
            for s in (0, 2, 4, 6, 8):
                nc.sync.dma_start(sup_tiles[s][:], sup_d[s // 3][:, s % 3])
            for s in (1, 3, 5, 7):
                nc.gpsimd.dma_start(sup_tiles[s][:], sup_d[s // 3][:, s % 3])

            # W_f shard: SP streams most of it through the branch phase;
            # ACT's chunk is scheduled into the collective window. Pool
            # carries none (it does late-branch copies + the collective).
            wt_all = const_p.tile([128, KT, JS], f16, tag="wt")
            nc.sync.dma_start(wt_all[:, 0:40, :], wt_d[:, 0:40, :])
            nc.sync.dma_start(wt_all[:, 40:44, :], wt_d[:, 40:44, :])
            with tc.tile_wait_until(0.032):
                nc.scalar.dma_start(wt_all[:, 44:56, :], wt_d[:, 44:56, :])
            with tc.tile_wait_until(0.0545):
                nc.gpsimd.dma_start(wt_all[:, 56:64, :], wt_d[:, 56:64, :])

            # packed-small views
            x0m_all = sm16[:, _OFF_X0M : _OFF_X0M + 64]

            def x0m_ap(m):
                return sm16[:, _OFF_X0M + m * FL : _OFF_X0M + (m + 1) * FL]

            def wa1_ap(t, k=128):
                return sm16[0:k, _OFF_WA1 + t * U : _OFF_WA1 + (t + 1) * U]

            wd1_ap = sm16[0:48, _OFF_WD1 : _OFF_WD1 + U]

            def wa2_ap(t, k=128):
                return sm16[0:k, _OFF_WA2 + t * FL : _OFF_WA2 + (t + 1) * FL]

            def wd2_ap(t, k=128):
                return sm16[0:k, _OFF_WD2 + t * FL : _OFF_WD2 + (t + 1) * FL]

            def bf_ap(lo, hi):
                return sm16[0:1, _OFF_BF + lo : _OFF_BF + hi]

            b1a = sm16[0:1, _OFF_B1A : _OFF_B1A + U]
            b1d = sm16[0:1, _OFF_B1D : _OFF_B1D + U]
            b2a = sm16[0:1, _OFF_B2A : _OFF_B2A + FL]
            b2d = sm16[0:1, _OFF_B2D : _OFF_B2D + FL]
            ones128 = sm16[0:1, _OFF_ONES : _OFF_ONES + 128]

            # node-major mat stacks [128, J, m, fin]; J: 0=x0/c1, then
            # x1_s at 1+2s, x2_s at 2+2s (the reference concat order)
            nm1a = const_p.tile([128, 4, 17, FL], f16, tag="nm1a")
            nm1d = const_p.tile([128, 4, 3, FL], f16, tag="nm1d")
            nm2a = const_p.tile([128, 4, 17, U], f16, tag="nm2a")
            nm2d = const_p.tile([128, 4, 3, U], f16, tag="nm2d")

            nc.vector.tensor_copy(nm1a[:, :, 0, :], x0m_all)
            nc.scalar.copy(nm1d[:, :, 0, :], x0m_all)

            def supT_ap(s, m, j):
                # S_s^T[m-block, j-block] = stationary for out node-block j
                return sup_tiles[s][:, m, j * 128 : (j + 1) * 128]

            # copy/stt helpers: 0 = DVE, 1 = ACT (copy only), 2 = Pool
            def cp(which, dst, src):
                if which == 1:
                    nc.scalar.copy(dst, src)
                elif which == 2:
                    nc.gpsimd.tensor_copy(dst, src)
                else:
                    nc.vector.tensor_copy(dst, src)

            def cheb_nm(fin, nm, s_list, x_src, pool_tag, cp_rot):
                """x1 = S@x, x2 = 2*S@x1 - x in node-major form.

                x_src(s): [128, 4, fin] node-major input (x0 or c1).
                Writes nm[:, 1+2i, :, :] and nm[:, 2+2i, :, :] for s_list[i].
                """
                # pass A: x1 for every support
                for i, s in enumerate(s_list):
                    psb = acc_p.tile([128, 4, U], f32, tag="ps")
                    ps1 = psb[:, :, 0:fin]
                    for j in range(4):
                        for m in range(4):
                            nc.tensor.matmul(
                                ps1[:, j, :], supT_ap(s, m, j),
                                x_src(s)[:, m, :],
                                start=(m == 0), stop=(m == 3),
                                skip_group_check=True,
                            )
                    cp(cp_rot[i % len(cp_rot)], nm[s][:, :, 1 + 2 * i, :], ps1[:])
                # pass B: x2 = 2*(S@x1) - x
                for i, s in enumerate(s_list):
                    psb = acc_p.tile([128, 4, U], f32, tag="ps")
                    ps2 = psb[:, :, 0:fin]
                    for j in range(4):
                        for m in range(4):
                            nc.tensor.matmul(
                                ps2[:, j, :], supT_ap(s, m, j),
                                nm[s][:, m, 1 + 2 * i, :],
                                start=(m == 0), stop=(m == 3),
                                skip_group_check=True,
                            )
                    cp(cp_rot[(i + 1) % len(cp_rot)],
                       nm[s][:, :, 2 + 2 * i, :], ps2[:])

            def fm_transpose(nm_ap_fn, rows, tag, eng):
                """Batched FM flip: nm cols (J..J+g, fin) of each m-block ->
                fm tile [rows<=128, 512] (k = J*fin+f, node-major cols)."""
                fmb = sc_p.tile([128, N], f16, tag=tag)
                fm = fmb[0:rows, :]
                ptr = tr_p.tile([128, N], f16, tag="ptr")
                for m in range(4):
                    nc.tensor.transpose(
                        ptr[0:rows, m * 128 : (m + 1) * 128],
                        nm_ap_fn(m),
                        id128[:],
                    )
                cp(eng, fm, ptr[0:rows, :])
                return fm

            # ---- Layer 1 (fin=16) ----
            adv = list(range(8))
            cheb_nm(FL, {s: nm1a for s in adv}, adv,
                    lambda s: nm1a[:, :, 0, :], "psL1", (0, 1))
            cheb_nm(FL, {8: nm1d}, [8],
                    lambda s: nm1d[:, :, 0, :], "psL1", (1, 0))

            # FM stationaries for the L1 GEMM: adv tiles (8+8+1 mats),
            # diff tile (3 mats)
            fm1a = []
            for t in range(2):
                fm1a.append(fm_transpose(
                    lambda m, t=t: nm1a[:, m, 8 * t : 8 * t + 8, :],
                    128, f"fmA{t}", t % 2,
                ))
            fm1a.append(fm_transpose(
                lambda m: nm1a[:, m, 16, :], FL, "fmA2", 0))
            fm1d = fm_transpose(
                lambda m: nm1d[:, m, 0:3, :], 48, "fmA3", 1)

            # L1 GEMM (weights moving): c1 = tanh(xs @ W1 + b1), node-major
            pc1a = acc_p.tile([128, 4, U], f32, tag="ps")
            pc1d = acc_p.tile([128, 4, U], f32, tag="ps")
            for j in range(4):
                for t in range(3):
                    kk = 128 if t < 2 else FL
                    nc.tensor.matmul(
                        pc1a[:, j, :], fm1a[t][0:kk, j * 128 : (j + 1) * 128],
                        wa1_ap(t, kk),
                        start=(t == 0), stop=False, skip_group_check=True,
                    )
                nc.tensor.matmul(
                    pc1a[:, j, :], ones128, b1a,
                    start=False, stop=True, skip_group_check=True,
                )
                nc.tensor.matmul(
                    pc1d[:, j, :], fm1d[:, j * 128 : (j + 1) * 128], wd1_ap,
                    start=True, stop=False, skip_group_check=True,
                )
                nc.tensor.matmul(
                    pc1d[:, j, :], ones128, b1d,
                    start=False, stop=True, skip_group_check=True,
                )
            nc.scalar.activation(nm2a[:, :, 0, :], pc1a[:], AF.Tanh)
            nc.scalar.activation(nm2d[:, :, 0, :], pc1d[:], AF.Tanh)

            # ---- Layer 2 (fin=64); diff first so its grad chain overlaps ----
            cheb_nm(U, {8: nm2d}, [8],
                    lambda s: nm2d[:, :, 0, :], "psL2", (1, 0))
            cheb_nm(U, {s: nm2a for s in adv}, adv,
                    lambda s: nm2a[:, :, 0, :], "psL2", (0, 1))

            # FM stationaries for the L2 GEMM: diff (2 tiles), adv (9 tiles)
            fm2d = []
            fm2d.append(fm_transpose(
                lambda m: nm2d[:, m, 0:2, :], 128, "fmA0", 0))
            fm2d.append(fm_transpose(
                lambda m: nm2d[:, m, 2, :], U, "fmA1", 1))
            fm2a = []
            for t in range(8):
                fm2a.append(fm_transpose(
                    lambda m, t=t: nm2a[:, m, 2 * t : 2 * t + 2, :],
                    128, f"fmA{t+2}", (0, 1)[t % 2],
                ))
            fm2a.append(fm_transpose(
                lambda m: nm2a[:, m, 16, :], U, "fmA10", 1))

            # L2 GEMMs -> grads node-major [128, 4, 16] (the agin layout).
            # W2/b2 are host-negated so tanh lands the sign; diff still
            # needs the 0.1 coefficient.
            g_st = fus_p.tile([128, 2, 4, FL], f16, tag="gst")
            pgdb = acc_p.tile([128, 4, U], f32, tag="ps")
            pgd = pgdb[:, :, 0:FL]
            for j in range(4):
                for t in range(2):
                    kk = 128 if t < 1 else U
                    nc.tensor.matmul(
                        pgd[:, j, :], fm2d[t][0:kk, j * 128 : (j + 1) * 128],
                        wd2_ap(t, kk),
                        start=(t == 0), stop=False, skip_group_check=True,
                    )
                nc.tensor.matmul(
                    pgd[:, j, :], ones128, b2d,
                    start=False, stop=True, skip_group_check=True,
                )
            gd_t = sc_p.tile([128, 4, FL], f16, tag="gdt")
            nc.scalar.activation(gd_t[:], pgd[:], AF.Tanh)
            nc.vector.tensor_scalar_mul(g_st[:, 0, :, :], gd_t[:], COEFF)

            pgab = acc_p.tile([128, 4, U], f32, tag="ps")
            pga = pgab[:, :, 0:FL]
            for j in range(4):
                for t in range(9):
                    kk = 128 if t < 8 else U
                    nc.tensor.matmul(
                        pga[:, j, :], fm2a[t][0:kk, j * 128 : (j + 1) * 128],
                        wa2_ap(t, kk),
                        start=(t == 0), stop=False, skip_group_check=True,
                    )
                nc.tensor.matmul(
                    pga[:, j, :], ones128, b2a,
                    start=False, stop=True, skip_group_check=True,
                )
            nc.scalar.activation(g_st[:, 1, :, :], pga[:], AF.Tanh)

            # ---- AllGather of node-major grads: agin[r, p, m, f] ----
            agin = dram_p.tile([2, 128, 4, FL], f16)
            agout = dram_p.tile([NCORES, 2, 128, 4, FL], f16)
            nc.scalar.dma_start(agin[0].rearrange("p m f -> p (m f)"),
                                g_st[:, 0].rearrange("p m f -> p (m f)"))
            nc.scalar.dma_start(agin[1].rearrange("p m f -> p (m f)"),
                                g_st[:, 1].rearrange("p m f -> p (m f)"))
            if collective:
                nc.gpsimd.collective_compute(
                    "AllGather",
                    ALU.bypass,
                    replica_groups=[list(range(NCORES))],
                    ins=[agin.opt()],
                    outs=[agout.opt()],
                )
            else:
                for r in range(NCORES):
                    nc.gpsimd.dma_start(agout[r], agin[:])

            # ---- W_f phase ----
            # Gathered grads land directly in the stationary layout:
            # gt_all[q, col, kt] with kt = m*16+f <-> k = (m*128+q)*16+f;
            # wt is host-permuted to the same enumeration. Diff grads ->
            # cols 0-7, adv -> cols 32-39 (3-dim APs, contiguous last dim).
            nc.sync.dma_start(
                gt_all[:, 0:8, :],
                agout[:, 0].rearrange("c p m f -> p c (m f)"),
            )
            nc.gpsimd.dma_start(
                gt_all[:, 32:40, :],
                agout[:, 1].rearrange("c p m f -> p c (m f)"),
            )

            # Half 1's GEMM completes first so its fusion chain runs under
            # half 2's GEMM; each half is 64 matmuls + a bias row.
            def fusion(ps, h):
                # only one PSUM operand allowed per DVE op -> stage X_adv
                xa = fu_p.tile([B, 512], f16, tag="fu")
                nc.scalar.copy(xa[:], ps[32 : 32 + B, :])
                ssum = fu_p.tile([B, 512], f16, tag="fu")
                nc.vector.tensor_add(ssum[:], ps[0:B, :], xa[:])
                d = fu_p.tile([B, 512], f16, tag="fu")
                nc.vector.tensor_sub(d[:], ps[0:B, :], xa[:])
                z = fu_p.tile([B, 512], f16, tag="fu")
                nc.scalar.activation(z[:], ssum[:], AF.Sigmoid)
                zd = fu_p.tile([B, 512], f16, tag="fu")
                nc.vector.tensor_mul(zd[:], z[:], d[:])
                o = fus_p.tile([B, 512], f32, tag="fo")
                nc.vector.tensor_add(o[:], zd[:], ps[32 : 32 + B, :])
                nc.sync.dma_start(out_d[:, h * 512 : (h + 1) * 512], o[:])

            def fusion_q(ps, q):
                c0 = q * 256
                sl = slice(c0, c0 + 256)
                xa = fu_p.tile([B, 256], f16, tag="fuq")
                nc.scalar.copy(xa[:], ps[32 : 32 + B, sl])
                ssum = fu_p.tile([B, 256], f16, tag="fuq")
                nc.vector.tensor_add(ssum[:], ps[0:B, sl], xa[:])
                d = fu_p.tile([B, 256], f16, tag="fuq")
                nc.vector.tensor_sub(d[:], ps[0:B, sl], xa[:])
                z = fu_p.tile([B, 256], f16, tag="fuq")
                nc.scalar.activation(z[:], ssum[:], AF.Sigmoid)
                zd = fu_p.tile([B, 256], f16, tag="fuq")
                nc.vector.tensor_mul(zd[:], z[:], d[:])
                o = fus_p.tile([B, 256], f32, tag=f"foq{q}")
                nc.vector.tensor_add(o[:], zd[:], ps[32 : 32 + B, sl])
                nc.sync.dma_start(out_d[:, 512 + c0 : 512 + c0 + 256], o[:])

            psX1 = psx_p.tile([40, 512], f32, tag="psX")
            psX2 = psx_p.tile([40, 512], f32, tag="psX")
            for kt in range(KT):
                nc.tensor.matmul(
                    psX1[:], gt_all[:, :, kt], wt_all[:, kt, 0:512],
                    start=(kt == 0), stop=False, skip_group_check=True,
                )
            nc.tensor.matmul(
                psX1[:], ones40[:], bf_ap(0, 512),
                start=False, stop=True, skip_group_check=True,
            )
            fusion(psX1, 0)
            for q, (lo, hi) in enumerate(((512, 768), (768, 1024))):
                for kt in range(KT):
                    nc.tensor.matmul(
                        psX2[:, q * 256 : (q + 1) * 256], gt_all[:, :, kt],
                        wt_all[:, kt, lo:hi],
                        start=(kt == 0), stop=False, skip_group_check=True,
                    )
                nc.tensor.matmul(
                    psX2[:, q * 256 : (q + 1) * 256], ones40[:], bf_ap(lo, hi),
                    start=False, stop=True, skip_group_check=True,
                )
                fusion_q(psX2, q)

    _split_excess_waits(nc)
    return nc


def _prep_in_maps(inputs: dict) -> list[dict]:
    y = np.asarray(inputs["y"], np.float32)
    sd = np.asarray(inputs["supports_diff"], np.float32)
    sa = np.asarray(inputs["supports_adv"], np.float32)
    W_d1 = np.asarray(inputs["W_d1"], np.float32)
    W_d2 = -np.asarray(inputs["W_d2"], np.float32)
    W_a1 = np.asarray(inputs["W_a1"], np.float32)
    W_a2 = -np.asarray(inputs["W_a2"], np.float32)
    W_f = np.asarray(inputs["W_f"], np.float32)
    b_f = np.asarray(inputs["b_f"], np.float32)


    def cheb_fold(W, fin, M):
        # mats become [x0, x1_s, y2_s=S@x1_s]: W'[x0] -= sum W[x2_s];
        # W'[y2_s] = 2 W[x2_s]
        Wf = W.reshape(fin, M, -1).copy()
        for j in range(2, M, 2):
            Wf[:, 0, :] -= Wf[:, j, :]
            Wf[:, j, :] *= 2.0
        return Wf.reshape(fin * M, -1)

    W_a1 = cheb_fold(W_a1, FL, 17)
    W_d1 = cheb_fold(W_d1, FL, 3)
    W_a2 = cheb_fold(W_a2, U, 17)
    W_d2 = cheb_fold(W_d2, U, 3)

    # supports, transposed, node-tile-major, one per tile:
    # supT[b, p, si, m, n] = S_{3b+si}.T[m*128+p, n]
    supT = np.empty((3, 128, 3, 4, N), np.float16)
    for s in range(9):
        Ssrc = sa[s] if s < 8 else sd[0]
        st = Ssrc.T.astype(np.float16)  # [m, n]
        supT[s // 3, :, s % 3] = st.reshape(4, 128, N).transpose(1, 0, 2)

    def perm_pad(W, fin, M, fout, ntiles):
        # reference row (f, m) -> packed row m*fin+f, zero-padded to tiles
        Wp = W.reshape(fin, M, fout).transpose(1, 0, 2).reshape(fin * M, fout)
        pad = np.zeros((ntiles * 128, fout), np.float16)
        pad[: fin * M] = Wp.astype(np.float16)
        return pad.reshape(ntiles, 128, fout)

    wa1 = perm_pad(W_a1, FL, 17, U, 3)
    wd1 = perm_pad(W_d1, FL, 3, U, 1)
    wa2 = perm_pad(W_a2, U, 17, FL, 9)
    wd2 = perm_pad(W_d2, U, 3, FL, 2)

    # wt[q, m*16+f, j] = W_f.T[(m*128+q)*FL + f, c*JS+j]  (kt = m*16+f)
    WT = W_f.T.astype(np.float16)  # [k_orig = n*FL+f, j_global]
    in_maps = []
    for c in range(NCORES):
        x0 = y[c].reshape(N, FL)  # [node, f]
        x0m = x0.reshape(4, 128, FL).transpose(1, 0, 2).astype(np.float16)

        sm16 = np.zeros((128, _SM16), np.float16)
        sm16[:, _OFF_X0M : _OFF_X0M + 64] = x0m.reshape(128, 64)
        sm16[:, _OFF_WA1 : _OFF_WA1 + 3 * U] = wa1.transpose(1, 0, 2).reshape(
            128, 3 * U
        )
        sm16[:, _OFF_WD1 : _OFF_WD1 + U] = wd1[0]
        sm16[:, _OFF_WA2 : _OFF_WA2 + 9 * FL] = wa2.transpose(1, 0, 2).reshape(
            128, 9 * FL
        )
        sm16[:, _OFF_WD2 : _OFF_WD2 + 2 * FL] = wd2.transpose(1, 0, 2).reshape(
            128, 2 * FL
        )
        sm16[0, _OFF_BF : _OFF_BF + JS] = b_f[c * JS : (c + 1) * JS].astype(
            np.float16
        )
        sm16[0, _OFF_B1A : _OFF_B1A + U] = np.asarray(inputs["b_a1"], np.float16)
        sm16[0, _OFF_B1D : _OFF_B1D + U] = np.asarray(inputs["b_d1"], np.float16)
        sm16[0, _OFF_B2A : _OFF_B2A + FL] = -np.asarray(
            inputs["b_a2"], np.float16
        )
        sm16[0, _OFF_B2D : _OFF_B2D + FL] = -np.asarray(
            inputs["b_d2"], np.float16
        )
        sm16[0, _OFF_ONES : _OFF_ONES + 128] = 1.0

        # [(m q f), j] -> [q, m, f, j] -> [q, m*16+f, j]
        wt = np.ascontiguousarray(
            WT[:, c * JS : (c + 1) * JS]
            .reshape(4, 128, FL, JS)
            .transpose(1, 0, 2, 3)
            .reshape(128, KT, JS)
        )
        in_maps.append({"sm16": sm16, "supT": supT, "wt": wt})
    return in_maps


_CACHE: dict = {}


def _get_nc() -> bass.Bass:
    if "nc" not in _CACHE:
        _CACHE["nc"] = _build()
    return _CACHE["nc"]


def run(inputs: dict, trace: bool = False):
    """Run on the 8 cores; returns (full_output, BassKernelResults)."""
    in_maps = _prep_in_maps(inputs)
    nc = _get_nc()
    kw = {}
    if trace:
        kw = dict(trace=True, trace_cores=list(range(NCORES)), stitch_traces=False)
    res = run_bass_kernel_spmd(nc, in_maps, core_ids=list(range(NCORES)), **kw)
    out = np.concatenate(
        [res.results[c]["out"] for c in range(NCORES)], axis=1
    ).astype(np.float32)
    return out, res


def kernel(**inputs) -> np.ndarray:
    out, _ = run(inputs)
    return out


# revision 33
# speedup vs baseline: 1.2261x; 1.0291x over previous
"""Trainium2 Bass kernel for nn_ODEFunc (gnn_message_passing, 8 cores).

Strategy (cost model: matmul = out-free-rows; DMA = free-dim bytes per
queue, 3 queues; collective = 15us + gathered bytes / 40GBps):
  - Batch-parallel branches: core b computes batch b's diff+adv gconv
    branches. All Chebyshev mats are built NODE-major with the support
    as the matmul *stationary* ([128,128] S^T blocks), so each x1/x2
    costs only 16 matmuls x fin rows instead of streaming the 512-wide
    support as moving data.
  - Node-major mats are packed per layer into nm stacks [128, J, 4, fin]
    (mat index J on the free dim -> no partition-start issues). The
    layer GEMM needs feature-major stationaries: batched PE transposes
    flip 8 (L1) / 2 (L2) mats per 128-row k-tile in one psum bank.
  - Layer GEMMs run with the (host-permuted) weights as *moving* data:
    out = c1/grads node-major, 64/16 rows per matmul. Biases are added
    with a ones-row rank-1 matmul into the same psum group.
  - W_d2/b_d2, W_a2/b_a2 are host-negated so tanh emits the grad sign;
    the diff 0.1 coefficient is one DVE op. Grads come out node-major,
    exactly the AllGather staging layout (no grad transposes).
  - AllGather [2,128,4,16] fp16 per core; gathered grads land straight
    in the W_f stationary gt_all[128, 40, KT] via two 3-dim strided
    DMAs (kt = m*16+f; wt is host-permuted to the same k enumeration).
    psX[40, 512]: X_diff rows 0-7, X_adv rows 32-39 (cols 8-31 are
    memset-zero lanes).
  - W_f shard (fp16, 16 MB) is split across the 3 DMA queues sized to
    each queue's idle windows (ACT's chunk is scheduled into the
    collective window via tile_wait_until).
  - GEMM half 1 finishes before half 2 starts so its gated-fusion chain
    hides under half 2's matmuls.
"""

import sys

sys.path.insert(0, "/opt/trn_rl_repo")

import numpy as np

import concourse.bass as bass
import concourse.mybir as mybir
from concourse import masks
from concourse.bass_utils import run_bass_kernel_spmd
from concourse.tile import TileContext
from concourse.vector_clock import ScopedClock

N = 512          # nodes
FL = 16          # latent
U = 64           # units
B = 8            # batch
HID = N * FL     # 8192
COEFF = 0.1
NCORES = 8
JS = HID // NCORES  # 1024 output columns per core
KT = HID // 128     # 64 contraction tiles for the W_f GEMM

f16 = mybir.dt.float16
f32 = mybir.dt.float32
AF = mybir.ActivationFunctionType
ALU = mybir.AluOpType

# smalls_f16 packed free-dim offsets (elements)
_OFF_X0M = 0             # [128, 4*16] x0 node-major
_OFF_WA1 = 64            # 3 tiles [128, 64]
_OFF_WD1 = 256           # [48(->128), 64]
_OFF_WA2 = 320           # 9 tiles [128, 16]
_OFF_WD2 = 464           # 2 tiles [128, 16]
_OFF_BF = 496            # [1, 1024]
_OFF_B1A = 1520          # [1, 64]
_OFF_B1D = 1584          # [1, 64]
_OFF_B2A = 1648          # [1, 16]
_OFF_B2D = 1664          # [1, 16]
_OFF_ONES = 1680         # [1, 128] ones
_SM16 = 1808


class PatchedTileContext(TileContext):
    """Tail drain with at most one sem wait per instruction.

    The walrus build here rejects Drain instructions carrying >2 sync
    waits ("Too many sync wait commands"). Spread the global-clock waits
    over individual SP nops ahead of the drain.
    """

    def _drain_and_barrier(self, tick_clock, wait_clock):
        nc = self.nc
        probe = nc.sync.nop(nofuse=True)
        wait_clock.add_sem_waits(
            probe.ins, ScopedClock({None: tick_clock.global_clock})
        )
        si = probe.ins.sync_info
        ws = list(si.on_wait) if si is not None else []
        if len(ws) > 1:
            probe.ins.sync_info = mybir.SyncInfo(
                on_wait=ws[:1], on_update=list(si.on_update)
            )
            for w in ws[1:]:
                n2 = nc.sync.nop(nofuse=True)
                n2.ins.sync_info = mybir.SyncInfo(on_wait=[w], on_update=[])
        nc.sync.drain()
        nc.all_engine_barrier()
        popped = nc._tile_sem_poison_stack.pop()
        assert popped is self._sem_poison
        nc.clear_and_free_semaphores(list(self.sems.allocated().values()))
        nc.all_engine_barrier()


_WAIT_LIMIT = 1


def _split_excess_waits(nc: bass.Bass) -> None:
    """Move sync waits beyond _WAIT_LIMIT onto same-engine NOPs inserted
    just before the carrying instruction (this walrus build has tiny
    setupSyncWait budgets for DMA/collective/drain instruction formats)."""
    for fn in nc.m.functions:
        for bb in fn.blocks:
            insts = bb.instructions
            i = 0
            while i < len(insts):
                inst = insts[i]
                si = inst.sync_info
                ws = list(si.on_wait) if si is not None and si.on_wait else []
                if len(ws) > _WAIT_LIMIT and type(inst).__name__ != "InstNoOp":
                    keep = ws[:_WAIT_LIMIT]
                    extra = ws[_WAIT_LIMIT:]
                    inst.sync_info = mybir.SyncInfo(
                        on_wait=keep, on_update=list(si.on_update)
                    )
                    for k, w in enumerate(extra):
                        nop = mybir.InstNoOp(
                            name=f"{inst.name}-w{k}",
                            engine=inst.engine,
                            bass_nofuse=True,
                            sync_info=mybir.SyncInfo(on_wait=[w], on_update=[]),
                        )
                        nc.register_instruction(nop, overwrite=True)
                        insts.insert(i, nop)
                        i += 1
                i += 1


def _build(collective: bool = True) -> bass.Bass:
    nc = bass.Bass(num_devices=NCORES)

    # ---- DRAM I/O (per-core values supplied via in_maps) ----
    sm16_d = nc.dram_tensor("sm16", [128, _SM16], f16, kind="ExternalInput")
    sup_d = nc.dram_tensor("supT", [3, 128, 3, 4, N], f16, kind="ExternalInput")
    wt_d = nc.dram_tensor("wt", [128, KT, JS], f16, kind="ExternalInput")
    out_d = nc.dram_tensor("out", [B, JS], f32, kind="ExternalOutput")

    with PatchedTileContext(nc) as tc:
        from contextlib import ExitStack

        with ExitStack() as ctx:
            const_p = ctx.enter_context(tc.tile_pool(name="const", bufs=1))
            sup_p = ctx.enter_context(tc.tile_pool(name="sup", bufs=1))
            sc_p = ctx.enter_context(tc.tile_pool(name="sc", bufs=1))
            fus_p = ctx.enter_context(tc.tile_pool(name="fus", bufs=1))
            fu_p = ctx.enter_context(tc.tile_pool(name="fu", bufs=3))
            acc_p = ctx.enter_context(tc.tile_pool(name="acc", bufs=3, space="PSUM"))
            psx_p = ctx.enter_context(tc.tile_pool(name="psx", bufs=1, space="PSUM"))
            tr_p = ctx.enter_context(tc.tile_pool(name="tr", bufs=2, space="PSUM"))
            dram_p = ctx.enter_context(tc.tile_pool(name="dram", bufs=1, space="DRAM"))

            # ---- constants / memsets ----
            id128 = const_p.tile([128, 128], f16, tag="id")
            masks.make_identity(nc, id128[:])
            ones40 = const_p.tile([1, 40], f16, tag="ones")
            nc.vector.memset(ones40[:], 1.0)
            # W_f-GEMM stationary [q, col, kt]; cols 8-31 are never DMA'd
            # -> zero them once, early (garbage would NaN the psum)
            gt_all = const_p.tile([128, 40, KT], f16, tag="gt")
            nc.gpsimd.memset(gt_all[:], 0.0)

            # ---- input DMAs: smalls first; supports split SP/Pool so the
            # last support lands by ~9.5us ----
            sm16 = const_p.tile([128, _SM16], f16, tag="sm16")
            nc.sync.dma_start(sm16[:], sm16_d[:])
            sup_tiles = []
            for s in range(9):
                supb = sup_p.tile([128, 4, N], f16, tag=f"sup{s}")
                sup_tiles.append(supb)
            for s in (0, 2, 4, 6, 8):
                nc.sync.dma_start(sup_tiles[s][:], sup_d[s // 3][:, s % 3])
            for s in (1, 3, 5, 7):
                nc.gpsimd.dma_start(sup_tiles[s][:], sup_d[s // 3][:, s % 3])

            # W_f shard: SP streams most of it through the branch phase;
            # ACT's chunk is scheduled into the collective window. Pool
            # carries none (it does late-branch copies + the collective).
            wt_all = const_p.tile([128, KT, JS], f16, tag="wt")
            nc.sync.dma_start(wt_all[:, 0:40, :], wt_d[:, 0:40, :])
            nc.sync.dma_start(wt_all[:, 40:44, :], wt_d[:, 40:44, :])
            with tc.tile_wait_until(0.032):
                nc.scalar.dma_start(wt_all[:, 44:56, :], wt_d[:, 44:56, :])
            with tc.tile_wait_until(0.0545):
                nc.gpsimd.dma_start(wt_all[:, 56:64, :], wt_d[:, 56:64, :])

            # packed-small views
            x0m_all = sm16[:, _OFF_X0M : _OFF_X0M + 64]

            def x0m_ap(m):
                return sm16[:, _OFF_X0M + m * FL : _OFF_X0M + (m + 1) * FL]

            def wa1_ap(t, k=128):
                return sm16[0:k, _OFF_WA1 + t * U : _OFF_WA1 + (t + 1) * U]

            wd1_ap = sm16[0:48, _OFF_WD1 : _OFF_WD1 + U]

            def wa2_ap(t, k=128):
                return sm16[0:k, _OFF_WA2 + t * FL : _OFF_WA2 + (t + 1) * FL]

            def wd2_ap(t, k=128):
                return sm16[0:k, _OFF_WD2 + t * FL : _OFF_WD2 + (t + 1) * FL]

            def bf_ap(lo, hi):
                return sm16[0:1, _OFF_BF + lo : _OFF_BF + hi]

            b1a = sm16[0:1, _OFF_B1A : _OFF_B1A + U]
            b1d = sm16[0:1, _OFF_B1D : _OFF_B1D + U]
            b2a = sm16[0:1, _OFF_B2A : _OFF_B2A + FL]
            b2d = sm16[0:1, _OFF_B2D : _OFF_B2D + FL]
            ones128 = sm16[0:1, _OFF_ONES : _OFF_ONES + 128]

            # node-major mat stacks [128, J, m, fin]; J: 0=x0/c1, then
            # x1_s at 1+2s, x2_s at 2+2s (the reference concat order)
            nm1a = const_p.tile([128, 4, 17, FL], f16, tag="nm1a")
            nm1d = const_p.tile([128, 4, 3, FL], f16, tag="nm1d")
            nm2a = const_p.tile([128, 4, 17, U], f16, tag="nm2a")
            nm2d = const_p.tile([128, 4, 3, U], f16, tag="nm2d")

            nc.vector.tensor_copy(nm1a[:, :, 0, :], x0m_all)
            nc.scalar.copy(nm1d[:, :, 0, :], x0m_all)

            def supT_ap(s, m, j):
                # S_s^T[m-block, j-block] = stationary for out node-block j
                return sup_tiles[s][:, m, j * 128 : (j + 1) * 128]

            # copy/stt helpers: 0 = DVE, 1 = ACT (copy only), 2 = Pool
            def cp(which, dst, src):
                if which == 1:
                    nc.scalar.copy(dst, src)
                elif which == 2:
                    nc.gpsimd.tensor_copy(dst, src)
                else:
                    nc.vector.tensor_copy(dst, src)

            def cheb_nm(fin, nm, s_list, x_src, pool_tag, cp_rot):
                """x1 = S@x, x2 = 2*S@x1 - x in node-major form.

                x_src(s): [128, 4, fin] node-major input (x0 or c1).
                Writes nm[:, 1+2i, :, :] and nm[:, 2+2i, :, :] for s_list[i].
                """
                # pass A: x1 for every support
                for i, s in enumerate(s_list):
                    psb = acc_p.tile([128, 4, U], f32, tag="ps")
                    ps1 = psb[:, :, 0:fin]
                    for j in range(4):
                        for m in range(4):
                            nc.tensor.matmul(
                                ps1[:, j, :], supT_ap(s, m, j),
                                x_src(s)[:, m, :],
                                start=(m == 0), stop=(m == 3),
                                skip_group_check=True,
                            )
                    cp(cp_rot[i % len(cp_rot)], nm[s][:, :, 1 + 2 * i, :], ps1[:])
                # pass B: x2 = 2*(S@x1) - x
                for i, s in enumerate(s_list):
                    psb = acc_p.tile([128, 4, U], f32, tag="ps")
                    ps2 = psb[:, :, 0:fin]
                    for j in range(4):
                        for m in range(4):
                            nc.tensor.matmul(
                                ps2[:, j, :], supT_ap(s, m, j),
                                nm[s][:, m, 1 + 2 * i, :],
                                start=(m == 0), stop=(m == 3),
                                skip_group_check=True,
                            )
                    cp(cp_rot[(i + 1) % len(cp_rot)],
                       nm[s][:, :, 2 + 2 * i, :], ps2[:])

            def fm_transpose(nm_ap_fn, rows, tag, eng):
                """Batched FM flip: nm cols (J..J+g, fin) of each m-block ->
                fm tile [rows<=128, 512] (k = J*fin+f, node-major cols)."""
                fmb = sc_p.tile([128, N], f16, tag=tag)
                fm = fmb[0:rows, :]
                ptr = tr_p.tile([128, N], f16, tag="ptr")
                for m in range(4):
                    nc.tensor.transpose(
                        ptr[0:rows, m * 128 : (m + 1) * 128],
                        nm_ap_fn(m),
                        id128[:],
                    )
                cp(eng, fm, ptr[0:rows, :])
                return fm

            # ---- Layer 1 (fin=16) ----
            adv = list(range(8))
            cheb_nm(FL, {s: nm1a for s in adv}, adv,
                    lambda s: nm1a[:, :, 0, :], "psL1", (0, 1))
            cheb_nm(FL, {8: nm1d}, [8],
                    lambda s: nm1d[:, :, 0, :], "psL1", (1, 0))

            # FM stationaries for the L1 GEMM: adv tiles (8+8+1 mats),
            # diff tile (3 mats)
            fm1a = []
            for t in range(2):
                fm1a.append(fm_transpose(
                    lambda m, t=t: nm1a[:, m, 8 * t : 8 * t + 8, :],
                    128, f"fmA{t}", t % 2,
                ))
            fm1a.append(fm_transpose(
                lambda m: nm1a[:, m, 16, :], FL, "fmA2", 0))
            fm1d = fm_transpose(
                lambda m: nm1d[:, m, 0:3, :], 48, "fmA3", 1)

            # L1 GEMM (weights moving): c1 = tanh(xs @ W1 + b1), node-major
            pc1a = acc_p.tile([128, 4, U], f32, tag="ps")
            pc1d = acc_p.tile([128, 4, U], f32, tag="ps")
            for j in range(4):
                for t in range(3):
                    kk = 128 if t < 2 else FL
                    nc.tensor.matmul(
                        pc1a[:, j, :], fm1a[t][0:kk, j * 128 : (j + 1) * 128],
                        wa1_ap(t, kk),
                        start=(t == 0), stop=False, skip_group_check=True,
                    )
                nc.tensor.matmul(
                    pc1a[:, j, :], ones128, b1a,
                    start=False, stop=True, skip_group_check=True,
                )
                nc.tensor.matmul(
                    pc1d[:, j, :], fm1d[:, j * 128 : (j + 1) * 128], wd1_ap,
                    start=True, stop=False, skip_group_check=True,
                )
                nc.tensor.matmul(
                    pc1d[:, j, :], ones128, b1d,
                    start=False, stop=True, skip_group_check=True,
                )
            nc.scalar.activation(nm2a[:, :, 0, :], pc1a[:], AF.Tanh)
            nc.scalar.activation(nm2d[:, :, 0, :], pc1d[:], AF.Tanh)

            # ---- Layer 2 (fin=64); diff first so its grad chain overlaps ----
            cheb_nm(U, {8: nm2d}, [8],
                    lambda s: nm2d[:, :, 0, :], "psL2", (1, 0))
            cheb_nm(U, {s: nm2a for s in adv}, adv,
                    lambda s: nm2a[:, :, 0, :], "psL2", (0, 1))

            # FM stationaries for the L2 GEMM: diff (2 tiles), adv (9 tiles)
            fm2d = []
            fm2d.append(fm_transpose(
                lambda m: nm2d[:, m, 0:2, :], 128, "fmA0", 0))
            fm2d.append(fm_transpose(
                lambda m: nm2d[:, m, 2, :], U, "fmA1", 1))
            fm2a = []
            for t in range(8):
                fm2a.append(fm_transpose(
                    lambda m, t=t: nm2a[:, m, 2 * t : 2 * t + 2, :],
                    128, f"fmA{t+2}", (0, 1)[t % 2],
                ))
            fm2a.append(fm_transpose(
                lambda m: nm2a[:, m, 16, :], U, "fmA10", 1))

            # L2 GEMMs -> grads node-major [128, 4, 16] (the agin layout).
            # W2/b2 are host-negated so tanh lands the sign; diff still
            # needs the 0.1 coefficient.
            g_st = fus_p.tile([128, 2, 4, FL], f16, tag="gst")
            pgdb = acc_p.tile([128, 4, U], f32, tag="ps")
            pgd = pgdb[:, :, 0:FL]
            for j in range(4):
                for t in range(2):
                    kk = 128 if t < 1 else U
                    nc.tensor.matmul(
                        pgd[:, j, :], fm2d[t][0:kk, j * 128 : (j + 1) * 128],
                        wd2_ap(t, kk),
                        start=(t == 0), stop=False, skip_group_check=True,
                    )
                nc.tensor.matmul(
                    pgd[:, j, :], ones128, b2d,
                    start=False, stop=True, skip_group_check=True,
                )
            gd_t = sc_p.tile([128, 4, FL], f16, tag="gdt")
            nc.scalar.activation(gd_t[:], pgd[:], AF.Tanh)
            nc.vector.tensor_scalar_mul(g_st[:, 0, :, :], gd_t[:], COEFF)

            pgab = acc_p.tile([128, 4, U], f32, tag="ps")
            pga = pgab[:, :, 0:FL]
            for j in range(4):
                for t in range(9):
                    kk = 128 if t < 8 else U
                    nc.tensor.matmul(
                        pga[:, j, :], fm2a[t][0:kk, j * 128 : (j + 1) * 128],
                        wa2_ap(t, kk),
                        start=(t == 0), stop=False, skip_group_check=True,
                    )
                nc.tensor.matmul(
                    pga[:, j, :], ones128, b2a,
                    start=False, stop=True, skip_group_check=True,
                )
            nc.scalar.activation(g_st[:, 1, :, :], pga[:], AF.Tanh)

            # ---- AllGather of node-major grads: agin[r, p, m, f] ----
            agin = dram_p.tile([2, 128, 4, FL], f16)
            agout = dram_p.tile([NCORES, 2, 128, 4, FL], f16)
            nc.scalar.dma_start(agin[0].rearrange("p m f -> p (m f)"),
                                g_st[:, 0].rearrange("p m f -> p (m f)"))
            nc.scalar.dma_start(agin[1].rearrange("p m f -> p (m f)"),
                                g_st[:, 1].rearrange("p m f -> p (m f)"))
            if collective:
                nc.gpsimd.collective_compute(
                    "AllGather",
                    ALU.bypass,
                    replica_groups=[list(range(NCORES))],
                    ins=[agin.opt()],
                    outs=[agout.opt()],
                )
            else:
                for r in range(NCORES):
                    nc.gpsimd.dma_start(agout[r], agin[:])

            # ---- W_f phase ----
            # Gathered grads land directly in the stationary layout:
            # gt_all[q, col, kt] with kt = m*16+f <-> k = (m*128+q)*16+f;
            # wt is host-permuted to the same enumeration. Diff grads ->
            # cols 0-7, adv -> cols 32-39 (3-dim APs, contiguous last dim).
            nc.sync.dma_start(
                gt_all[:, 0:8, :],
                agout[:, 0].rearrange("c p m f -> p c (m f)"),
            )
            nc.gpsimd.dma_start(
                gt_all[:, 32:40, :],
                agout[:, 1].rearrange("c p m f -> p c (m f)"),
            )

            # Half 1's GEMM completes first so its fusion chain runs under
            # half 2's GEMM; each half is 64 matmuls + a bias row.
            def fusion(ps, h):
                # only one PSUM operand allowed per DVE op -> stage X_adv
                xa = fu_p.tile([B, 512], f16, tag="fu")
                nc.scalar.copy(xa[:], ps[32 : 32 + B, :])
                ssum = fu_p.tile([B, 512], f16, tag="fu")
                nc.vector.tensor_add(ssum[:], ps[0:B, :], xa[:])
                d = fu_p.tile([B, 512], f16, tag="fu")
                nc.vector.tensor_sub(d[:], ps[0:B, :], xa[:])
                z = fu_p.tile([B, 512], f16, tag="fu")
                nc.scalar.activation(z[:], ssum[:], AF.Sigmoid)
                zd = fu_p.tile([B, 512], f16, tag="fu")
                nc.vector.tensor_mul(zd[:], z[:], d[:])
                o = fus_p.tile([B, 512], f32, tag="fo")
                nc.vector.tensor_add(o[:], zd[:], ps[32 : 32 + B, :])
                nc.sync.dma_start(out_d[:, h * 512 : (h + 1) * 512], o[:])

            def fusion_q(ps, q):
                c0 = q * 256
                sl = slice(0, 256)
                xa = fu_p.tile([B, 256], f16, tag="fuq")
                nc.scalar.copy(xa[:], ps[32 : 32 + B, sl])
                ssum = fu_p.tile([B, 256], f16, tag="fuq")
                nc.vector.tensor_add(ssum[:], ps[0:B, sl], xa[:])
                d = fu_p.tile([B, 256], f16, tag="fuq")
                nc.vector.tensor_sub(d[:], ps[0:B, sl], xa[:])
                z = fu_p.tile([B, 256], f16, tag="fuq")
                nc.scalar.activation(z[:], ssum[:], AF.Sigmoid)
                zd = fu_p.tile([B, 256], f16, tag="fuq")
                nc.vector.tensor_mul(zd[:], z[:], d[:])
                o = fus_p.tile([B, 256], f32, tag=f"foq{q}")
                nc.vector.tensor_add(o[:], zd[:], ps[32 : 32 + B, sl])
                nc.sync.dma_start(out_d[:, 512 + c0 : 512 + c0 + 256], o[:])

            psX1 = psx_p.tile([40, 512], f32, tag="psX1")
            psX2a = psx_p.tile([40, 256], f32, tag="psX2a")
            psX2b = psx_p.tile([40, 256], f32, tag="psX2b")
            for kt in range(KT):
                nc.tensor.matmul(
                    psX1[:], gt_all[:, :, kt], wt_all[:, kt, 0:512],
                    start=(kt == 0), stop=False, skip_group_check=True,
                )
            nc.tensor.matmul(
                psX1[:], ones40[:], bf_ap(0, 512),
                start=False, stop=True, skip_group_check=True,
            )
            fusion(psX1, 0)
            for q, ps2, (lo, hi) in ((0, psX2a, (512, 768)),
                                     (1, psX2b, (768, 1024))):
                for kt in range(KT):
                    nc.tensor.matmul(
                        ps2[:], gt_all[:, :, kt], wt_all[:, kt, lo:hi],
                        start=(kt == 0), stop=False, skip_group_check=True,
                    )
                nc.tensor.matmul(
                    ps2[:], ones40[:], bf_ap(lo, hi),
                    start=False, stop=True, skip_group_check=True,
                )
                fusion_q(ps2, q)

    _split_excess_waits(nc)
    return nc


def _prep_in_maps(inputs: dict) -> list[dict]:
    y = np.asarray(inputs["y"], np.float32)
    sd = np.asarray(inputs["supports_diff"], np.float32)
    sa = np.asarray(inputs["supports_adv"], np.float32)
    W_d1 = np.asarray(inputs["W_d1"], np.float32)
    W_d2 = -np.asarray(inputs["W_d2"], np.float32)
    W_a1 = np.asarray(inputs["W_a1"], np.float32)
    W_a2 = -np.asarray(inputs["W_a2"], np.float32)
    W_f = np.asarray(inputs["W_f"], np.float32)
    b_f = np.asarray(inputs["b_f"], np.float32)


    def cheb_fold(W, fin, M):
        # mats become [x0, x1_s, y2_s=S@x1_s]: W'[x0] -= sum W[x2_s];
        # W'[y2_s] = 2 W[x2_s]
        Wf = W.reshape(fin, M, -1).copy()
        for j in range(2, M, 2):
            Wf[:, 0, :] -= Wf[:, j, :]
            Wf[:, j, :] *= 2.0
        return Wf.reshape(fin * M, -1)

    W_a1 = cheb_fold(W_a1, FL, 17)
    W_d1 = cheb_fold(W_d1, FL, 3)
    W_a2 = cheb_fold(W_a2, U, 17)
    W_d2 = cheb_fold(W_d2, U, 3)

    # supports, transposed, node-tile-major, one per tile:
    # supT[b, p, si, m, n] = S_{3b+si}.T[m*128+p, n]
    supT = np.empty((3, 128, 3, 4, N), np.float16)
    for s in range(9):
        Ssrc = sa[s] if s < 8 else sd[0]
        st = Ssrc.T.astype(np.float16)  # [m, n]
        supT[s // 3, :, s % 3] = st.reshape(4, 128, N).transpose(1, 0, 2)

    def perm_pad(W, fin, M, fout, ntiles):
        # reference row (f, m) -> packed row m*fin+f, zero-padded to tiles
        Wp = W.reshape(fin, M, fout).transpose(1, 0, 2).reshape(fin * M, fout)
        pad = np.zeros((ntiles * 128, fout), np.float16)
        pad[: fin * M] = Wp.astype(np.float16)
        return pad.reshape(ntiles, 128, fout)

    wa1 = perm_pad(W_a1, FL, 17, U, 3)
    wd1 = perm_pad(W_d1, FL, 3, U, 1)
    wa2 = perm_pad(W_a2, U, 17, FL, 9)
    wd2 = perm_pad(W_d2, U, 3, FL, 2)

    # wt[q, m*16+f, j] = W_f.T[(m*128+q)*FL + f, c*JS+j]  (kt = m*16+f)
    WT = W_f.T.astype(np.float16)  # [k_orig = n*FL+f, j_global]
    in_maps = []
    for c in range(NCORES):
        x0 = y[c].reshape(N, FL)  # [node, f]
        x0m = x0.reshape(4, 128, FL).transpose(1, 0, 2).astype(np.float16)

        sm16 = np.zeros((128, _SM16), np.float16)
        sm16[:, _OFF_X0M : _OFF_X0M + 64] = x0m.reshape(128, 64)
        sm16[:, _OFF_WA1 : _OFF_WA1 + 3 * U] = wa1.transpose(1, 0, 2).reshape(
            128, 3 * U
        )
        sm16[:, _OFF_WD1 : _OFF_WD1 + U] = wd1[0]
        sm16[:, _OFF_WA2 : _OFF_WA2 + 9 * FL] = wa2.transpose(1, 0, 2).reshape(
            128, 9 * FL
        )
        sm16[:, _OFF_WD2 : _OFF_WD2 + 2 * FL] = wd2.transpose(1, 0, 2).reshape(
            128, 2 * FL
        )
        sm16[0, _OFF_BF : _OFF_BF + JS] = b_f[c * JS : (c + 1) * JS].astype(
            np.float16
        )
        sm16[0, _OFF_B1A : _OFF_B1A + U] = np.asarray(inputs["b_a1"], np.float16)
        sm16[0, _OFF_B1D : _OFF_B1D + U] = np.asarray(inputs["b_d1"], np.float16)
        sm16[0, _OFF_B2A : _OFF_B2A + FL] = -np.asarray(
            inputs["b_a2"], np.float16
        )
        sm16[0, _OFF_B2D : _OFF_B2D + FL] = -np.asarray(
            inputs["b_d2"], np.float16
        )
        sm16[0, _OFF_ONES : _OFF_ONES + 128] = 1.0

        # [(m q f), j] -> [q, m, f, j] -> [q, m*16+f, j]
        wt = np.ascontiguousarray(
            WT[:, c * JS : (c + 1) * JS]
            .reshape(4, 128, FL, JS)
            .transpose(1, 0, 2, 3)
            .reshape(128, KT, JS)
        )
        in_maps.append({"sm16": sm16, "supT": supT, "wt": wt})
    return in_maps


_CACHE: dict = {}


def _get_nc() -> bass.Bass:
    if "nc" not in _CACHE:
        _CACHE["nc"] = _build()
    return _CACHE["nc"]


def run(inputs: dict, trace: bool = False):
    """Run on the 8 cores; returns (full_output, BassKernelResults)."""
    in_maps = _prep_in_maps(inputs)
    nc = _get_nc()
    kw = {}
    if trace:
        kw = dict(trace=True, trace_cores=list(range(NCORES)), stitch_traces=False)
    res = run_bass_kernel_spmd(nc, in_maps, core_ids=list(range(NCORES)), **kw)
    out = np.concatenate(
        [res.results[c]["out"] for c in range(NCORES)], axis=1
    ).astype(np.float32)
    return out, res


def kernel(**inputs) -> np.ndarray:
    out, _ = run(inputs)
    return out


# revision 37
# speedup vs baseline: 1.3086x; 1.0672x over previous
"""Trainium2 Bass kernel for nn_ODEFunc (gnn_message_passing, 8 cores).

Strategy (cost model: matmul = out-free-rows; DMA = free-dim bytes per
queue, 3 queues; collective = 15us + gathered bytes / 40GBps):
  - Batch-parallel branches: core b computes batch b's diff+adv gconv
    branches. All Chebyshev mats are built NODE-major with the support
    as the matmul *stationary* ([128,128] S^T blocks), so each x1/x2
    costs only 16 matmuls x fin rows instead of streaming the 512-wide
    support as moving data.
  - Node-major mats are packed per layer into nm stacks [128, J, 4, fin]
    (mat index J on the free dim -> no partition-start issues). The
    layer GEMM needs feature-major stationaries: batched PE transposes
    flip 8 (L1) / 2 (L2) mats per 128-row k-tile in one psum bank.
  - Layer GEMMs run with the (host-permuted) weights as *moving* data:
    out = c1/grads node-major, 64/16 rows per matmul. Biases are added
    with a ones-row rank-1 matmul into the same psum group.
  - W_d2/b_d2, W_a2/b_a2 are host-negated so tanh emits the grad sign;
    the diff 0.1 coefficient is one DVE op. Grads come out node-major,
    exactly the AllGather staging layout (no grad transposes).
  - AllGather [2,128,4,16] fp16 per core; gathered grads land straight
    in the W_f stationary gt_all[128, 40, KT] via two 3-dim strided
    DMAs (kt = m*16+f; wt is host-permuted to the same k enumeration).
    psX[40, 512]: X_diff rows 0-7, X_adv rows 32-39 (cols 8-31 are
    memset-zero lanes).
  - W_f shard (fp16, 16 MB) is split across the 3 DMA queues sized to
    each queue's idle windows (ACT's chunk is scheduled into the
    collective window via tile_wait_until).
  - GEMM half 1 finishes before half 2 starts so its gated-fusion chain
    hides under half 2's matmuls.
"""

import sys

sys.path.insert(0, "/opt/trn_rl_repo")

import numpy as np

import concourse.bass as bass
import concourse.mybir as mybir
from concourse import masks
from concourse.bass_utils import run_bass_kernel_spmd
from concourse.tile import TileContext
from concourse.vector_clock import ScopedClock

N = 512          # nodes
FL = 16          # latent
U = 64           # units
B = 8            # batch
HID = N * FL     # 8192
COEFF = 0.1
NCORES = 8
JS = HID // NCORES  # 1024 output columns per core
KT = HID // 128     # 64 contraction tiles for the W_f GEMM

f16 = mybir.dt.float16
f32 = mybir.dt.float32
AF = mybir.ActivationFunctionType
ALU = mybir.AluOpType

# smalls_f16 packed free-dim offsets (elements)
_OFF_X0M = 0             # [128, 4*16] x0 node-major
_OFF_WA1 = 64            # 3 tiles [128, 64]
_OFF_WD1 = 256           # [48(->128), 64]
_OFF_WA2 = 320           # 9 tiles [128, 16]
_OFF_WD2 = 464           # 2 tiles [128, 16]
_OFF_BF = 496            # [1, 1024]
_OFF_B1A = 1520          # [1, 64]
_OFF_B1D = 1584          # [1, 64]
_OFF_B2A = 1648          # [1, 16]
_OFF_B2D = 1664          # [1, 16]
_OFF_ONES = 1680         # [1, 128] ones
_SM16 = 1808


class PatchedTileContext(TileContext):
    """Tail drain with at most one sem wait per instruction.

    The walrus build here rejects Drain instructions carrying >2 sync
    waits ("Too many sync wait commands"). Spread the global-clock waits
    over individual SP nops ahead of the drain.
    """

    def _drain_and_barrier(self, tick_clock, wait_clock):
        nc = self.nc
        probe = nc.sync.nop(nofuse=True)
        wait_clock.add_sem_waits(
            probe.ins, ScopedClock({None: tick_clock.global_clock})
        )
        si = probe.ins.sync_info
        ws = list(si.on_wait) if si is not None else []
        if len(ws) > 1:
            probe.ins.sync_info = mybir.SyncInfo(
                on_wait=ws[:1], on_update=list(si.on_update)
            )
            for w in ws[1:]:
                n2 = nc.sync.nop(nofuse=True)
                n2.ins.sync_info = mybir.SyncInfo(on_wait=[w], on_update=[])
        nc.sync.drain()
        nc.all_engine_barrier()
        popped = nc._tile_sem_poison_stack.pop()
        assert popped is self._sem_poison
        nc.clear_and_free_semaphores(list(self.sems.allocated().values()))
        nc.all_engine_barrier()


_WAIT_LIMIT = 1


def _split_excess_waits(nc: bass.Bass) -> None:
    """Move sync waits beyond _WAIT_LIMIT onto same-engine NOPs inserted
    just before the carrying instruction (this walrus build has tiny
    setupSyncWait budgets for DMA/collective/drain instruction formats)."""
    for fn in nc.m.functions:
        for bb in fn.blocks:
            insts = bb.instructions
            i = 0
            while i < len(insts):
                inst = insts[i]
                si = inst.sync_info
                ws = list(si.on_wait) if si is not None and si.on_wait else []
                if len(ws) > _WAIT_LIMIT and type(inst).__name__ != "InstNoOp":
                    keep = ws[:_WAIT_LIMIT]
                    extra = ws[_WAIT_LIMIT:]
                    inst.sync_info = mybir.SyncInfo(
                        on_wait=keep, on_update=list(si.on_update)
                    )
                    for k, w in enumerate(extra):
                        nop = mybir.InstNoOp(
                            name=f"{inst.name}-w{k}",
                            engine=inst.engine,
                            bass_nofuse=True,
                            sync_info=mybir.SyncInfo(on_wait=[w], on_update=[]),
                        )
                        nc.register_instruction(nop, overwrite=True)
                        insts.insert(i, nop)
                        i += 1
                i += 1


def _build(collective: bool = True) -> bass.Bass:
    nc = bass.Bass(num_devices=NCORES)

    # ---- DRAM I/O (per-core values supplied via in_maps) ----
    sm16_d = nc.dram_tensor("sm16", [128, _SM16], f16, kind="ExternalInput")
    sup_d = nc.dram_tensor("supT", [3, 128, 3, 4, N], f16, kind="ExternalInput")
    wt_d = nc.dram_tensor("wt", [128, KT, JS], f16, kind="ExternalInput")
    out_d = nc.dram_tensor("out", [B, JS], f32, kind="ExternalOutput")

    with PatchedTileContext(nc) as tc:
        from contextlib import ExitStack

        with ExitStack() as ctx:
            const_p = ctx.enter_context(tc.tile_pool(name="const", bufs=1))
            sup_p = ctx.enter_context(tc.tile_pool(name="sup", bufs=1))
            sc_p = ctx.enter_context(tc.tile_pool(name="sc", bufs=1))
            fus_p = ctx.enter_context(tc.tile_pool(name="fus", bufs=1))
            fu_p = ctx.enter_context(tc.tile_pool(name="fu", bufs=3))
            acc_p = ctx.enter_context(tc.tile_pool(name="acc", bufs=3, space="PSUM"))
            psx_p = ctx.enter_context(tc.tile_pool(name="psx", bufs=1, space="PSUM"))
            tr_p = ctx.enter_context(tc.tile_pool(name="tr", bufs=2, space="PSUM"))
            dram_p = ctx.enter_context(tc.tile_pool(name="dram", bufs=1, space="DRAM"))

            # ---- constants / memsets ----
            id128 = const_p.tile([128, 128], f16, tag="id")
            masks.make_identity(nc, id128[:])
            id128f = const_p.tile([128, 128], f32, tag="idf")
            masks.make_identity(nc, id128f[:])
            ones40 = const_p.tile([1, 40], f16, tag="ones")
            nc.vector.memset(ones40[:], 1.0)
            # W_f-GEMM moving data [q, col, kt]: diff grads cols 0-7,
            # adv cols 8-15 (every col is DMA'd; no zeroing needed)
            gt_all = const_p.tile([128, FL, KT], f16, tag="gt")

            # ---- input DMAs: smalls first; supports split SP/Pool so the
            # last support lands by ~9.5us ----
            sm16 = const_p.tile([128, _SM16], f16, tag="sm16")
            nc.sync.dma_start(sm16[:], sm16_d[:])
            sup_tiles = []
            for s in range(9):
                supb = sup_p.tile([128, 4, N], f16, tag=f"sup{s}")
                sup_tiles.append(supb)
            for s in (0, 2, 4, 6, 8):
                nc.sync.dma_start(sup_tiles[s][:], sup_d[s // 3][:, s % 3])
            for s in (1, 3, 5, 7):
                nc.gpsimd.dma_start(sup_tiles[s][:], sup_d[s // 3][:, s % 3])

            # W_f shard: SP streams most of it through the branch phase;
            # ACT's chunk is scheduled into the collective window. Pool
            # carries none (it does late-branch copies + the collective).
            wt_all = const_p.tile([128, KT, JS], f16, tag="wt")
            nc.sync.dma_start(wt_all[:, 0:40, :], wt_d[:, 0:40, :])
            with tc.tile_wait_until(0.029):
                nc.scalar.dma_start(wt_all[:, 40:64, :], wt_d[:, 40:64, :])

            # packed-small views
            x0m_all = sm16[:, _OFF_X0M : _OFF_X0M + 64]

            def x0m_ap(m):
                return sm16[:, _OFF_X0M + m * FL : _OFF_X0M + (m + 1) * FL]

            def wa1_ap(t, k=128):
                return sm16[0:k, _OFF_WA1 + t * U : _OFF_WA1 + (t + 1) * U]

            wd1_ap = sm16[0:48, _OFF_WD1 : _OFF_WD1 + U]

            def wa2_ap(t, k=128):
                return sm16[0:k, _OFF_WA2 + t * FL : _OFF_WA2 + (t + 1) * FL]

            def wd2_ap(t, k=128):
                return sm16[0:k, _OFF_WD2 + t * FL : _OFF_WD2 + (t + 1) * FL]

            def bf_ap(lo, hi):
                return sm16[0:1, _OFF_BF + lo : _OFF_BF + hi]

            b1a = sm16[0:1, _OFF_B1A : _OFF_B1A + U]
            b1d = sm16[0:1, _OFF_B1D : _OFF_B1D + U]
            b2a = sm16[0:1, _OFF_B2A : _OFF_B2A + FL]
            b2d = sm16[0:1, _OFF_B2D : _OFF_B2D + FL]
            ones128 = sm16[0:1, _OFF_ONES : _OFF_ONES + 128]

            # node-major mat stacks [128, J, m, fin]; J: 0=x0/c1, then
            # x1_s at 1+2s, x2_s at 2+2s (the reference concat order)
            nm1a = const_p.tile([128, 4, 17, FL], f16, tag="nm1a")
            nm1d = const_p.tile([128, 4, 3, FL], f16, tag="nm1d")
            nm2a = const_p.tile([128, 4, 17, U], f16, tag="nm2a")
            nm2d = const_p.tile([128, 4, 3, U], f16, tag="nm2d")

            nc.vector.tensor_copy(nm1a[:, :, 0, :], x0m_all)
            nc.scalar.copy(nm1d[:, :, 0, :], x0m_all)

            def supT_ap(s, m, j):
                # S_s^T[m-block, j-block] = stationary for out node-block j
                return sup_tiles[s][:, m, j * 128 : (j + 1) * 128]

            # copy/stt helpers: 0 = DVE, 1 = ACT (copy only), 2 = Pool
            def cp(which, dst, src):
                if which == 1:
                    nc.scalar.copy(dst, src)
                elif which == 2:
                    nc.gpsimd.tensor_copy(dst, src)
                else:
                    nc.vector.tensor_copy(dst, src)

            def cheb_nm(fin, nm, s_list, x_src, pool_tag, cp_rot):
                """x1 = S@x, x2 = 2*S@x1 - x in node-major form.

                x_src(s): [128, 4, fin] node-major input (x0 or c1).
                Writes nm[:, 1+2i, :, :] and nm[:, 2+2i, :, :] for s_list[i].
                """
                # pass A: x1 for every support
                for i, s in enumerate(s_list):
                    psb = acc_p.tile([128, 4, U], f32, tag="ps")
                    ps1 = psb[:, :, 0:fin]
                    for j in range(4):
                        for m in range(4):
                            nc.tensor.matmul(
                                ps1[:, j, :], supT_ap(s, m, j),
                                x_src(s)[:, m, :],
                                start=(m == 0), stop=(m == 3),
                                skip_group_check=True,
                            )
                    cp(cp_rot[i % len(cp_rot)], nm[s][:, :, 1 + 2 * i, :], ps1[:])
                # pass B: x2 = 2*(S@x1) - x
                for i, s in enumerate(s_list):
                    psb = acc_p.tile([128, 4, U], f32, tag="ps")
                    ps2 = psb[:, :, 0:fin]
                    for j in range(4):
                        for m in range(4):
                            nc.tensor.matmul(
                                ps2[:, j, :], supT_ap(s, m, j),
                                nm[s][:, m, 1 + 2 * i, :],
                                start=(m == 0), stop=(m == 3),
                                skip_group_check=True,
                            )
                    cp(cp_rot[(i + 1) % len(cp_rot)],
                       nm[s][:, :, 2 + 2 * i, :], ps2[:])

            def fm_transpose(nm_ap_fn, rows, tag, eng):
                """Batched FM flip: nm cols (J..J+g, fin) of each m-block ->
                fm tile [rows<=128, 512] (k = J*fin+f, node-major cols)."""
                fmb = sc_p.tile([128, N], f16, tag=tag)
                fm = fmb[0:rows, :]
                ptr = tr_p.tile([128, N], f16, tag="ptr")
                for m in range(4):
                    nc.tensor.transpose(
                        ptr[0:rows, m * 128 : (m + 1) * 128],
                        nm_ap_fn(m),
                        id128[:],
                    )
                cp(eng, fm, ptr[0:rows, :])
                return fm

            # ---- Layer 1 (fin=16) ----
            adv = list(range(8))
            cheb_nm(FL, {s: nm1a for s in adv}, adv,
                    lambda s: nm1a[:, :, 0, :], "psL1", (0, 1))
            cheb_nm(FL, {8: nm1d}, [8],
                    lambda s: nm1d[:, :, 0, :], "psL1", (1, 0))

            # FM stationaries for the L1 GEMM: adv tiles (8+8+1 mats),
            # diff tile (3 mats)
            fm1a = []
            for t in range(2):
                fm1a.append(fm_transpose(
                    lambda m, t=t: nm1a[:, m, 8 * t : 8 * t + 8, :],
                    128, f"fmA{t}", t % 2,
                ))
            fm1a.append(fm_transpose(
                lambda m: nm1a[:, m, 16, :], FL, "fmA2", 0))
            fm1d = fm_transpose(
                lambda m: nm1d[:, m, 0:3, :], 48, "fmA3", 1)

            # L1 GEMM (weights moving): c1 = tanh(xs @ W1 + b1), node-major
            pc1a = acc_p.tile([128, 4, U], f32, tag="ps")
            pc1d = acc_p.tile([128, 4, U], f32, tag="ps")
            for j in range(4):
                for t in range(3):
                    kk = 128 if t < 2 else FL
                    nc.tensor.matmul(
                        pc1a[:, j, :], fm1a[t][0:kk, j * 128 : (j + 1) * 128],
                        wa1_ap(t, kk),
                        start=(t == 0), stop=False, skip_group_check=True,
                    )
                nc.tensor.matmul(
                    pc1a[:, j, :], ones128, b1a,
                    start=False, stop=True, skip_group_check=True,
                )
                nc.tensor.matmul(
                    pc1d[:, j, :], fm1d[:, j * 128 : (j + 1) * 128], wd1_ap,
                    start=True, stop=False, skip_group_check=True,
                )
                nc.tensor.matmul(
                    pc1d[:, j, :], ones128, b1d,
                    start=False, stop=True, skip_group_check=True,
                )
            nc.scalar.activation(nm2a[:, :, 0, :], pc1a[:], AF.Tanh)
            nc.scalar.activation(nm2d[:, :, 0, :], pc1d[:], AF.Tanh)

            # ---- Layer 2 (fin=64); diff first so its grad chain overlaps ----
            cheb_nm(U, {8: nm2d}, [8],
                    lambda s: nm2d[:, :, 0, :], "psL2", (1, 0))
            cheb_nm(U, {s: nm2a for s in adv}, adv,
                    lambda s: nm2a[:, :, 0, :], "psL2", (0, 1))

            # FM stationaries for the L2 GEMM: diff (2 tiles), adv (9 tiles)
            fm2d = []
            fm2d.append(fm_transpose(
                lambda m: nm2d[:, m, 0:2, :], 128, "fmA0", 0))
            fm2d.append(fm_transpose(
                lambda m: nm2d[:, m, 2, :], U, "fmA1", 1))
            fm2a = []
            for t in range(8):
                fm2a.append(fm_transpose(
                    lambda m, t=t: nm2a[:, m, 2 * t : 2 * t + 2, :],
                    128, f"fmA{t+2}", (0, 1)[t % 2],
                ))
            fm2a.append(fm_transpose(
                lambda m: nm2a[:, m, 16, :], U, "fmA10", 1))

            # L2 GEMMs -> grads node-major [128, 4, 16] (the agin layout).
            # W2/b2 are host-negated so tanh lands the sign; diff still
            # needs the 0.1 coefficient.
            g_st = fus_p.tile([128, 2, 4, FL], f16, tag="gst")
            pgdb = acc_p.tile([128, 4, U], f32, tag="ps")
            pgd = pgdb[:, :, 0:FL]
            for j in range(4):
                for t in range(2):
                    kk = 128 if t < 1 else U
                    nc.tensor.matmul(
                        pgd[:, j, :], fm2d[t][0:kk, j * 128 : (j + 1) * 128],
                        wd2_ap(t, kk),
                        start=(t == 0), stop=False, skip_group_check=True,
                    )
                nc.tensor.matmul(
                    pgd[:, j, :], ones128, b2d,
                    start=False, stop=True, skip_group_check=True,
                )
            gd_t = sc_p.tile([128, 4, FL], f16, tag="gdt")
            nc.scalar.activation(gd_t[:], pgd[:], AF.Tanh)
            nc.vector.tensor_scalar_mul(g_st[:, 0, :, :], gd_t[:], COEFF)

            pgab = acc_p.tile([128, 4, U], f32, tag="ps")
            pga = pgab[:, :, 0:FL]
            for j in range(4):
                for t in range(9):
                    kk = 128 if t < 8 else U
                    nc.tensor.matmul(
                        pga[:, j, :], fm2a[t][0:kk, j * 128 : (j + 1) * 128],
                        wa2_ap(t, kk),
                        start=(t == 0), stop=False, skip_group_check=True,
                    )
                nc.tensor.matmul(
                    pga[:, j, :], ones128, b2a,
                    start=False, stop=True, skip_group_check=True,
                )
            nc.scalar.activation(g_st[:, 1, :, :], pga[:], AF.Tanh)

            # ---- AllGather of node-major grads: agin[r, p, m, f] ----
            agin = dram_p.tile([2, 128, 4, FL], f16)
            agout = dram_p.tile([NCORES, 2, 128, 4, FL], f16)
            nc.scalar.dma_start(agin[0].rearrange("p m f -> p (m f)"),
                                g_st[:, 0].rearrange("p m f -> p (m f)"))
            nc.scalar.dma_start(agin[1].rearrange("p m f -> p (m f)"),
                                g_st[:, 1].rearrange("p m f -> p (m f)"))
            if collective:
                nc.gpsimd.collective_compute(
                    "AllGather",
                    ALU.bypass,
                    replica_groups=[list(range(NCORES))],
                    ins=[agin.opt()],
                    outs=[agout.opt()],
                )
            else:
                for r in range(NCORES):
                    nc.gpsimd.dma_start(agout[r], agin[:])

            # ---- W_f phase ----
            # Gathered grads land directly in the stationary layout:
            # gt_all[q, col, kt] with kt = m*16+f <-> k = (m*128+q)*16+f;
            # wt is host-permuted to the same enumeration. Diff grads ->
            # cols 0-7, adv -> cols 32-39 (3-dim APs, contiguous last dim).
            nc.sync.dma_start(
                gt_all[:, 0:8, :],
                agout[:, 0].rearrange("c p m f -> p c (m f)"),
            )
            nc.gpsimd.dma_start(
                gt_all[:, 8:16, :],
                agout[:, 1].rearrange("c p m f -> p c (m f)"),
            )

            # W_f GEMM with wt as *stationary* and the 16 grad
            # columns as moving data: out = X^T[j, row], 16 rows per
            # matmul (the stationary load is free). 8 j-blocks x 64 kt
            # accumulate in one psum tile; the bias is a rank-1 matmul
            # (bf slice x ones16) closing each group.
            psT = psx_p.tile([128, 8, FL], f32, tag="psT")
            for jb in range(8):
                for kt in range(KT):
                    nc.tensor.matmul(
                        psT[:, jb, :],
                        wt_all[:, kt, jb * 128 : (jb + 1) * 128],
                        gt_all[:, :, kt],
                        start=(kt == 0), stop=False, skip_group_check=True,
                    )
                nc.tensor.matmul(
                    psT[:, jb, :], bf_ap(jb * 128, (jb + 1) * 128),
                    ones40[:, 0:FL],
                    start=False, stop=True, skip_group_check=True,
                )

            # gated fusion on the transposed layout (rows on the free dim)
            xall = fu_p.tile([128, 8, FL], f16, tag="fu")
            nc.scalar.copy(xall[:], psT[:])
            xd = xall[:, :, 0:8]
            xa = xall[:, :, 8:16]
            ssum = fu_p.tile([128, 8, 8], f16, tag="fus8")
            nc.vector.tensor_add(ssum[:], xd, xa)
            d = fu_p.tile([128, 8, 8], f16, tag="fus8")
            nc.vector.tensor_sub(d[:], xd, xa)
            z = fu_p.tile([128, 8, 8], f16, tag="fus8")
            nc.scalar.activation(z[:], ssum[:], AF.Sigmoid)
            zd = fu_p.tile([128, 8, 8], f16, tag="fus8")
            nc.vector.tensor_mul(zd[:], z[:], d[:])
            o = fus_p.tile([128, 8, 8], f32, tag="fo")
            nc.vector.tensor_add(o[:], zd[:], xa)

            # transpose [j, (jb b)] -> [(jb b), j-block] and DMA out
            po = tr_p.tile([64, 128], f32, tag="po")
            nc.tensor.transpose(
                po[:], o[:].rearrange("p jb b -> p (jb b)"), id128f[:]
            )
            po_sb = fus_p.tile([64, 128], f32, tag="posb")
            nc.vector.tensor_copy(po_sb[:], po[:])
            dma_eng = (nc.sync, nc.scalar, nc.gpsimd)
            for jb in range(8):
                dma_eng[jb % 3].dma_start(
                    out_d[:, jb * 128 : (jb + 1) * 128],
                    po_sb[jb * 8 : (jb + 1) * 8, :],
                )

    _split_excess_waits(nc)
    return nc


def _prep_in_maps(inputs: dict) -> list[dict]:
    y = np.asarray(inputs["y"], np.float32)
    sd = np.asarray(inputs["supports_diff"], np.float32)
    sa = np.asarray(inputs["supports_adv"], np.float32)
    W_d1 = np.asarray(inputs["W_d1"], np.float32)
    W_d2 = -np.asarray(inputs["W_d2"], np.float32)
    W_a1 = np.asarray(inputs["W_a1"], np.float32)
    W_a2 = -np.asarray(inputs["W_a2"], np.float32)
    W_f = np.asarray(inputs["W_f"], np.float32)
    b_f = np.asarray(inputs["b_f"], np.float32)


    def cheb_fold(W, fin, M):
        # mats become [x0, x1_s, y2_s=S@x1_s]: W'[x0] -= sum W[x2_s];
        # W'[y2_s] = 2 W[x2_s]
        Wf = W.reshape(fin, M, -1).copy()
        for j in range(2, M, 2):
            Wf[:, 0, :] -= Wf[:, j, :]
            Wf[:, j, :] *= 2.0
        return Wf.reshape(fin * M, -1)

    W_a1 = cheb_fold(W_a1, FL, 17)
    W_d1 = cheb_fold(W_d1, FL, 3)
    W_a2 = cheb_fold(W_a2, U, 17)
    W_d2 = cheb_fold(W_d2, U, 3)

    # supports, transposed, node-tile-major, one per tile:
    # supT[b, p, si, m, n] = S_{3b+si}.T[m*128+p, n]
    supT = np.empty((3, 128, 3, 4, N), np.float16)
    for s in range(9):
        Ssrc = sa[s] if s < 8 else sd[0]
        st = Ssrc.T.astype(np.float16)  # [m, n]
        supT[s // 3, :, s % 3] = st.reshape(4, 128, N).transpose(1, 0, 2)

    def perm_pad(W, fin, M, fout, ntiles):
        # reference row (f, m) -> packed row m*fin+f, zero-padded to tiles
        Wp = W.reshape(fin, M, fout).transpose(1, 0, 2).reshape(fin * M, fout)
        pad = np.zeros((ntiles * 128, fout), np.float16)
        pad[: fin * M] = Wp.astype(np.float16)
        return pad.reshape(ntiles, 128, fout)

    wa1 = perm_pad(W_a1, FL, 17, U, 3)
    wd1 = perm_pad(W_d1, FL, 3, U, 1)
    wa2 = perm_pad(W_a2, U, 17, FL, 9)
    wd2 = perm_pad(W_d2, U, 3, FL, 2)

    # wt[q, m*16+f, j] = W_f.T[(m*128+q)*FL + f, c*JS+j]  (kt = m*16+f)
    WT = W_f.T.astype(np.float16)  # [k_orig = n*FL+f, j_global]
    in_maps = []
    for c in range(NCORES):
        x0 = y[c].reshape(N, FL)  # [node, f]
        x0m = x0.reshape(4, 128, FL).transpose(1, 0, 2).astype(np.float16)

        sm16 = np.zeros((128, _SM16), np.float16)
        sm16[:, _OFF_X0M : _OFF_X0M + 64] = x0m.reshape(128, 64)
        sm16[:, _OFF_WA1 : _OFF_WA1 + 3 * U] = wa1.transpose(1, 0, 2).reshape(
            128, 3 * U
        )
        sm16[:, _OFF_WD1 : _OFF_WD1 + U] = wd1[0]
        sm16[:, _OFF_WA2 : _OFF_WA2 + 9 * FL] = wa2.transpose(1, 0, 2).reshape(
            128, 9 * FL
        )
        sm16[:, _OFF_WD2 : _OFF_WD2 + 2 * FL] = wd2.transpose(1, 0, 2).reshape(
            128, 2 * FL
        )
        sm16[0, _OFF_BF : _OFF_BF + JS] = b_f[c * JS : (c + 1) * JS].astype(
            np.float16
        )
        sm16[0, _OFF_B1A : _OFF_B1A + U] = np.asarray(inputs["b_a1"], np.float16)
        sm16[0, _OFF_B1D : _OFF_B1D + U] = np.asarray(inputs["b_d1"], np.float16)
        sm16[0, _OFF_B2A : _OFF_B2A + FL] = -np.asarray(
            inputs["b_a2"], np.float16
        )
        sm16[0, _OFF_B2D : _OFF_B2D + FL] = -np.asarray(
            inputs["b_d2"], np.float16
        )
        sm16[0, _OFF_ONES : _OFF_ONES + 128] = 1.0

        # [(m q f), j] -> [q, m, f, j] -> [q, m*16+f, j]
        wt = np.ascontiguousarray(
            WT[:, c * JS : (c + 1) * JS]
            .reshape(4, 128, FL, JS)
            .transpose(1, 0, 2, 3)
            .reshape(128, KT, JS)
        )
        in_maps.append({"sm16": sm16, "supT": supT, "wt": wt})
    return in_maps


_CACHE: dict = {}


def _get_nc() -> bass.Bass:
    if "nc" not in _CACHE:
        _CACHE["nc"] = _build()
    return _CACHE["nc"]


def run(inputs: dict, trace: bool = False):
    """Run on the 8 cores; returns (full_output, BassKernelResults)."""
    in_maps = _prep_in_maps(inputs)
    nc = _get_nc()
    kw = {}
    if trace:
        kw = dict(trace=True, trace_cores=list(range(NCORES)), stitch_traces=False)
    res = run_bass_kernel_spmd(nc, in_maps, core_ids=list(range(NCORES)), **kw)
    out = np.concatenate(
        [res.results[c]["out"] for c in range(NCORES)], axis=1
    ).astype(np.float32)
    return out, res


def kernel(**inputs) -> np.ndarray:
    out, _ = run(inputs)
    return out


# revision 38
# speedup vs baseline: 1.3384x; 1.0228x over previous
"""Trainium2 Bass kernel for nn_ODEFunc (gnn_message_passing, 8 cores).

Strategy (cost model: matmul = out-free-rows; DMA = free-dim bytes per
queue, 3 queues; collective = 15us + gathered bytes / 40GBps):
  - Batch-parallel branches: core b computes batch b's diff+adv gconv
    branches. All Chebyshev mats are built NODE-major with the support
    as the matmul *stationary* ([128,128] S^T blocks), so each x1/x2
    costs only 16 matmuls x fin rows instead of streaming the 512-wide
    support as moving data.
  - Node-major mats are packed per layer into nm stacks [128, J, 4, fin]
    (mat index J on the free dim -> no partition-start issues). The
    layer GEMM needs feature-major stationaries: batched PE transposes
    flip 8 (L1) / 2 (L2) mats per 128-row k-tile in one psum bank.
  - Layer GEMMs run with the (host-permuted) weights as *moving* data:
    out = c1/grads node-major, 64/16 rows per matmul. Biases are added
    with a ones-row rank-1 matmul into the same psum group.
  - W_d2/b_d2, W_a2/b_a2 are host-negated so tanh emits the grad sign;
    the diff 0.1 coefficient is one DVE op. Grads come out node-major,
    exactly the AllGather staging layout (no grad transposes).
  - AllGather [2,128,4,16] fp16 per core; gathered grads land straight
    in the W_f stationary gt_all[128, 40, KT] via two 3-dim strided
    DMAs (kt = m*16+f; wt is host-permuted to the same k enumeration).
    psX[40, 512]: X_diff rows 0-7, X_adv rows 32-39 (cols 8-31 are
    memset-zero lanes).
  - W_f shard (fp16, 16 MB) is split across the 3 DMA queues sized to
    each queue's idle windows (ACT's chunk is scheduled into the
    collective window via tile_wait_until).
  - GEMM half 1 finishes before half 2 starts so its gated-fusion chain
    hides under half 2's matmuls.
"""

import sys

sys.path.insert(0, "/opt/trn_rl_repo")

import numpy as np

import concourse.bass as bass
import concourse.mybir as mybir
from concourse import masks
from concourse.bass_utils import run_bass_kernel_spmd
from concourse.tile import TileContext
from concourse.vector_clock import ScopedClock

N = 512          # nodes
FL = 16          # latent
U = 64           # units
B = 8            # batch
HID = N * FL     # 8192
COEFF = 0.1
NCORES = 8
JS = HID // NCORES  # 1024 output columns per core
KT = HID // 128     # 64 contraction tiles for the W_f GEMM

f16 = mybir.dt.float16
f32 = mybir.dt.float32
AF = mybir.ActivationFunctionType
ALU = mybir.AluOpType

# smalls_f16 packed free-dim offsets (elements)
_OFF_X0M = 0             # [128, 4*16] x0 node-major
_OFF_WA1 = 64            # 3 tiles [128, 64]
_OFF_WD1 = 256           # [48(->128), 64]
_OFF_WA2 = 320           # 9 tiles [128, 16]
_OFF_WD2 = 464           # 2 tiles [128, 16]
_OFF_BF = 496            # [1, 1024]
_OFF_B1A = 1520          # [1, 64]
_OFF_B1D = 1584          # [1, 64]
_OFF_B2A = 1648          # [1, 16]
_OFF_B2D = 1664          # [1, 16]
_OFF_ONES = 1680         # [1, 128] ones
_SM16 = 1808


class PatchedTileContext(TileContext):
    """Tail drain with at most one sem wait per instruction.

    The walrus build here rejects Drain instructions carrying >2 sync
    waits ("Too many sync wait commands"). Spread the global-clock waits
    over individual SP nops ahead of the drain.
    """

    def _drain_and_barrier(self, tick_clock, wait_clock):
        nc = self.nc
        probe = nc.sync.nop(nofuse=True)
        wait_clock.add_sem_waits(
            probe.ins, ScopedClock({None: tick_clock.global_clock})
        )
        si = probe.ins.sync_info
        ws = list(si.on_wait) if si is not None else []
        if len(ws) > 1:
            probe.ins.sync_info = mybir.SyncInfo(
                on_wait=ws[:1], on_update=list(si.on_update)
            )
            for w in ws[1:]:
                n2 = nc.sync.nop(nofuse=True)
                n2.ins.sync_info = mybir.SyncInfo(on_wait=[w], on_update=[])
        nc.sync.drain()
        nc.all_engine_barrier()
        popped = nc._tile_sem_poison_stack.pop()
        assert popped is self._sem_poison
        nc.clear_and_free_semaphores(list(self.sems.allocated().values()))
        nc.all_engine_barrier()


_WAIT_LIMIT = 1


def _split_excess_waits(nc: bass.Bass) -> None:
    """Move sync waits beyond _WAIT_LIMIT onto same-engine NOPs inserted
    just before the carrying instruction (this walrus build has tiny
    setupSyncWait budgets for DMA/collective/drain instruction formats)."""
    for fn in nc.m.functions:
        for bb in fn.blocks:
            insts = bb.instructions
            i = 0
            while i < len(insts):
                inst = insts[i]
                si = inst.sync_info
                ws = list(si.on_wait) if si is not None and si.on_wait else []
                if len(ws) > _WAIT_LIMIT and type(inst).__name__ != "InstNoOp":
                    keep = ws[:_WAIT_LIMIT]
                    extra = ws[_WAIT_LIMIT:]
                    inst.sync_info = mybir.SyncInfo(
                        on_wait=keep, on_update=list(si.on_update)
                    )
                    for k, w in enumerate(extra):
                        nop = mybir.InstNoOp(
                            name=f"{inst.name}-w{k}",
                            engine=inst.engine,
                            bass_nofuse=True,
                            sync_info=mybir.SyncInfo(on_wait=[w], on_update=[]),
                        )
                        nc.register_instruction(nop, overwrite=True)
                        insts.insert(i, nop)
                        i += 1
                i += 1


def _build(collective: bool = True) -> bass.Bass:
    nc = bass.Bass(num_devices=NCORES)

    # ---- DRAM I/O (per-core values supplied via in_maps) ----
    sm16_d = nc.dram_tensor("sm16", [128, _SM16], f16, kind="ExternalInput")
    sup_d = nc.dram_tensor("supT", [3, 128, 3, 4, N], f16, kind="ExternalInput")
    wt_d = nc.dram_tensor("wt", [128, KT, JS], f16, kind="ExternalInput")
    out_d = nc.dram_tensor("out", [B, JS], f32, kind="ExternalOutput")

    with PatchedTileContext(nc) as tc:
        from contextlib import ExitStack

        with ExitStack() as ctx:
            const_p = ctx.enter_context(tc.tile_pool(name="const", bufs=1))
            sup_p = ctx.enter_context(tc.tile_pool(name="sup", bufs=1))
            sc_p = ctx.enter_context(tc.tile_pool(name="sc", bufs=1))
            fus_p = ctx.enter_context(tc.tile_pool(name="fus", bufs=1))
            fu_p = ctx.enter_context(tc.tile_pool(name="fu", bufs=3))
            acc_p = ctx.enter_context(tc.tile_pool(name="acc", bufs=3, space="PSUM"))
            psx_p = ctx.enter_context(tc.tile_pool(name="psx", bufs=1, space="PSUM"))
            tr_p = ctx.enter_context(tc.tile_pool(name="tr", bufs=2, space="PSUM"))
            dram_p = ctx.enter_context(tc.tile_pool(name="dram", bufs=1, space="DRAM"))

            # ---- constants / memsets ----
            id128 = const_p.tile([128, 128], f16, tag="id")
            masks.make_identity(nc, id128[:])
            id128f = const_p.tile([128, 128], f32, tag="idf")
            masks.make_identity(nc, id128f[:])
            ones40 = const_p.tile([1, 40], f16, tag="ones")
            nc.vector.memset(ones40[:], 1.0)
            # W_f-GEMM moving data [q, col, kt]: diff grads cols 0-7,
            # adv cols 8-15 (every col is DMA'd; no zeroing needed)
            gt_all = const_p.tile([128, FL, KT], f16, tag="gt")

            # ---- input DMAs: smalls first; supports split SP/Pool so the
            # last support lands by ~9.5us ----
            sm16 = const_p.tile([128, _SM16], f16, tag="sm16")
            nc.sync.dma_start(sm16[:], sm16_d[:])
            sup_tiles = []
            for s in range(9):
                supb = sup_p.tile([128, 4, N], f16, tag=f"sup{s}")
                sup_tiles.append(supb)
            for s in (0, 2, 4, 6, 8):
                nc.sync.dma_start(sup_tiles[s][:], sup_d[s // 3][:, s % 3])
            for s in (1, 3, 5, 7):
                nc.gpsimd.dma_start(sup_tiles[s][:], sup_d[s // 3][:, s % 3])

            # W_f shard: SP streams most of it through the branch phase;
            # ACT's chunk is scheduled into the collective window. Pool
            # carries none (it does late-branch copies + the collective).
            wt_all = const_p.tile([128, KT, JS], f16, tag="wt")
            nc.sync.dma_start(wt_all[:, 0:40, :], wt_d[:, 0:40, :])
            with tc.tile_wait_until(0.026):
                nc.scalar.dma_start(wt_all[:, 40:64, :], wt_d[:, 40:64, :])

            # packed-small views
            x0m_all = sm16[:, _OFF_X0M : _OFF_X0M + 64]

            def x0m_ap(m):
                return sm16[:, _OFF_X0M + m * FL : _OFF_X0M + (m + 1) * FL]

            def wa1_ap(t, k=128):
                return sm16[0:k, _OFF_WA1 + t * U : _OFF_WA1 + (t + 1) * U]

            wd1_ap = sm16[0:48, _OFF_WD1 : _OFF_WD1 + U]

            def wa2_ap(t, k=128):
                return sm16[0:k, _OFF_WA2 + t * FL : _OFF_WA2 + (t + 1) * FL]

            def wd2_ap(t, k=128):
                return sm16[0:k, _OFF_WD2 + t * FL : _OFF_WD2 + (t + 1) * FL]

            def bf_ap(lo, hi):
                return sm16[0:1, _OFF_BF + lo : _OFF_BF + hi]

            b1a = sm16[0:1, _OFF_B1A : _OFF_B1A + U]
            b1d = sm16[0:1, _OFF_B1D : _OFF_B1D + U]
            b2a = sm16[0:1, _OFF_B2A : _OFF_B2A + FL]
            b2d = sm16[0:1, _OFF_B2D : _OFF_B2D + FL]
            ones128 = sm16[0:1, _OFF_ONES : _OFF_ONES + 128]

            # node-major mat stacks [128, J, m, fin]; J: 0=x0/c1, then
            # x1_s at 1+2s, x2_s at 2+2s (the reference concat order)
            nm1a = const_p.tile([128, 4, 17, FL], f16, tag="nm1a")
            nm1d = const_p.tile([128, 4, 3, FL], f16, tag="nm1d")
            nm2a = const_p.tile([128, 4, 17, U], f16, tag="nm2a")
            nm2d = const_p.tile([128, 4, 3, U], f16, tag="nm2d")

            nc.vector.tensor_copy(nm1a[:, :, 0, :], x0m_all)
            nc.scalar.copy(nm1d[:, :, 0, :], x0m_all)

            def supT_ap(s, m, j):
                # S_s^T[m-block, j-block] = stationary for out node-block j
                return sup_tiles[s][:, m, j * 128 : (j + 1) * 128]

            # copy/stt helpers: 0 = DVE, 1 = ACT (copy only), 2 = Pool
            def cp(which, dst, src):
                if which == 1:
                    nc.scalar.copy(dst, src)
                elif which == 2:
                    nc.gpsimd.tensor_copy(dst, src)
                else:
                    nc.vector.tensor_copy(dst, src)

            def cheb_nm(fin, nm, s_list, x_src, pool_tag, cp_rot):
                """x1 = S@x, x2 = 2*S@x1 - x in node-major form.

                x_src(s): [128, 4, fin] node-major input (x0 or c1).
                Writes nm[:, 1+2i, :, :] and nm[:, 2+2i, :, :] for s_list[i].
                """
                # pass A: x1 for every support
                for i, s in enumerate(s_list):
                    psb = acc_p.tile([128, 4, U], f32, tag="ps")
                    ps1 = psb[:, :, 0:fin]
                    for j in range(4):
                        for m in range(4):
                            nc.tensor.matmul(
                                ps1[:, j, :], supT_ap(s, m, j),
                                x_src(s)[:, m, :],
                                start=(m == 0), stop=(m == 3),
                                skip_group_check=True,
                            )
                    cp(cp_rot[i % len(cp_rot)], nm[s][:, :, 1 + 2 * i, :], ps1[:])
                # pass B: x2 = 2*(S@x1) - x
                for i, s in enumerate(s_list):
                    psb = acc_p.tile([128, 4, U], f32, tag="ps")
                    ps2 = psb[:, :, 0:fin]
                    for j in range(4):
                        for m in range(4):
                            nc.tensor.matmul(
                                ps2[:, j, :], supT_ap(s, m, j),
                                nm[s][:, m, 1 + 2 * i, :],
                                start=(m == 0), stop=(m == 3),
                                skip_group_check=True,
                            )
                    cp(cp_rot[(i + 1) % len(cp_rot)],
                       nm[s][:, :, 2 + 2 * i, :], ps2[:])

            def fm_transpose(nm_ap_fn, rows, tag, eng):
                """Batched FM flip: nm cols (J..J+g, fin) of each m-block ->
                fm tile [rows<=128, 512] (k = J*fin+f, node-major cols)."""
                fmb = sc_p.tile([128, N], f16, tag=tag)
                fm = fmb[0:rows, :]
                ptr = tr_p.tile([128, N], f16, tag="ptr")
                for m in range(4):
                    nc.tensor.transpose(
                        ptr[0:rows, m * 128 : (m + 1) * 128],
                        nm_ap_fn(m),
                        id128[:],
                    )
                cp(eng, fm, ptr[0:rows, :])
                return fm

            # ---- Layer 1 (fin=16) ----
            adv = list(range(8))
            cheb_nm(FL, {s: nm1a for s in adv}, adv,
                    lambda s: nm1a[:, :, 0, :], "psL1", (0, 1))
            cheb_nm(FL, {8: nm1d}, [8],
                    lambda s: nm1d[:, :, 0, :], "psL1", (1, 0))

            # FM stationaries for the L1 GEMM: adv tiles (8+8+1 mats),
            # diff tile (3 mats)
            fm1a = []
            for t in range(2):
                fm1a.append(fm_transpose(
                    lambda m, t=t: nm1a[:, m, 8 * t : 8 * t + 8, :],
                    128, f"fmA{t}", t % 2,
                ))
            fm1a.append(fm_transpose(
                lambda m: nm1a[:, m, 16, :], FL, "fmA2", 0))
            fm1d = fm_transpose(
                lambda m: nm1d[:, m, 0:3, :], 48, "fmA3", 1)

            # L1 GEMM (weights moving): c1 = tanh(xs @ W1 + b1), node-major
            pc1a = acc_p.tile([128, 4, U], f32, tag="ps")
            pc1d = acc_p.tile([128, 4, U], f32, tag="ps")
            for j in range(4):
                for t in range(3):
                    kk = 128 if t < 2 else FL
                    nc.tensor.matmul(
                        pc1a[:, j, :], fm1a[t][0:kk, j * 128 : (j + 1) * 128],
                        wa1_ap(t, kk),
                        start=(t == 0), stop=False, skip_group_check=True,
                    )
                nc.tensor.matmul(
                    pc1a[:, j, :], ones128, b1a,
                    start=False, stop=True, skip_group_check=True,
                )
                nc.tensor.matmul(
                    pc1d[:, j, :], fm1d[:, j * 128 : (j + 1) * 128], wd1_ap,
                    start=True, stop=False, skip_group_check=True,
                )
                nc.tensor.matmul(
                    pc1d[:, j, :], ones128, b1d,
                    start=False, stop=True, skip_group_check=True,
                )
            nc.scalar.activation(nm2a[:, :, 0, :], pc1a[:], AF.Tanh)
            nc.scalar.activation(nm2d[:, :, 0, :], pc1d[:], AF.Tanh)

            # ---- Layer 2 (fin=64); diff first so its grad chain overlaps ----
            cheb_nm(U, {8: nm2d}, [8],
                    lambda s: nm2d[:, :, 0, :], "psL2", (1, 0))
            cheb_nm(U, {s: nm2a for s in adv}, adv,
                    lambda s: nm2a[:, :, 0, :], "psL2", (0, 1))

            # FM stationaries for the L2 GEMM: diff (2 tiles), adv (9 tiles)
            fm2d = []
            fm2d.append(fm_transpose(
                lambda m: nm2d[:, m, 0:2, :], 128, "fmA0", 0))
            fm2d.append(fm_transpose(
                lambda m: nm2d[:, m, 2, :], U, "fmA1", 1))
            fm2a = []
            for t in range(8):
                fm2a.append(fm_transpose(
                    lambda m, t=t: nm2a[:, m, 2 * t : 2 * t + 2, :],
                    128, f"fmA{t+2}", (0, 1)[t % 2],
                ))
            fm2a.append(fm_transpose(
                lambda m: nm2a[:, m, 16, :], U, "fmA10", 1))

            # L2 GEMMs -> grads node-major [128, 4, 16] (the agin layout).
            # W2/b2 are host-negated so tanh lands the sign; diff still
            # needs the 0.1 coefficient.
            g_st = fus_p.tile([128, 2, 4, FL], f16, tag="gst")
            pgdb = acc_p.tile([128, 4, U], f32, tag="ps")
            pgd = pgdb[:, :, 0:FL]
            for j in range(4):
                for t in range(2):
                    kk = 128 if t < 1 else U
                    nc.tensor.matmul(
                        pgd[:, j, :], fm2d[t][0:kk, j * 128 : (j + 1) * 128],
                        wd2_ap(t, kk),
                        start=(t == 0), stop=False, skip_group_check=True,
                    )
                nc.tensor.matmul(
                    pgd[:, j, :], ones128, b2d,
                    start=False, stop=True, skip_group_check=True,
                )
            gd_t = sc_p.tile([128, 4, FL], f16, tag="gdt")
            nc.scalar.activation(gd_t[:], pgd[:], AF.Tanh)
            nc.vector.tensor_scalar_mul(g_st[:, 0, :, :], gd_t[:], COEFF)

            pgab = acc_p.tile([128, 4, U], f32, tag="ps")
            pga = pgab[:, :, 0:FL]
            for j in range(4):
                for t in range(9):
                    kk = 128 if t < 8 else U
                    nc.tensor.matmul(
                        pga[:, j, :], fm2a[t][0:kk, j * 128 : (j + 1) * 128],
                        wa2_ap(t, kk),
                        start=(t == 0), stop=False, skip_group_check=True,
                    )
                nc.tensor.matmul(
                    pga[:, j, :], ones128, b2a,
                    start=False, stop=True, skip_group_check=True,
                )
            nc.scalar.activation(g_st[:, 1, :, :], pga[:], AF.Tanh)

            # ---- AllGather of node-major grads: agin[r, p, m, f] ----
            agin = dram_p.tile([2, 128, 4, FL], f16)
            agout = dram_p.tile([NCORES, 2, 128, 4, FL], f16)
            nc.gpsimd.dma_start(agin[0].rearrange("p m f -> p (m f)"),
                                g_st[:, 0].rearrange("p m f -> p (m f)"))
            nc.gpsimd.dma_start(agin[1].rearrange("p m f -> p (m f)"),
                                g_st[:, 1].rearrange("p m f -> p (m f)"))
            if collective:
                nc.gpsimd.collective_compute(
                    "AllGather",
                    ALU.bypass,
                    replica_groups=[list(range(NCORES))],
                    ins=[agin.opt()],
                    outs=[agout.opt()],
                )
            else:
                for r in range(NCORES):
                    nc.gpsimd.dma_start(agout[r], agin[:])

            # ---- W_f phase ----
            # Gathered grads land directly in the stationary layout:
            # gt_all[q, col, kt] with kt = m*16+f <-> k = (m*128+q)*16+f;
            # wt is host-permuted to the same enumeration. Diff grads ->
            # cols 0-7, adv -> cols 32-39 (3-dim APs, contiguous last dim).
            nc.sync.dma_start(
                gt_all[:, 0:8, :],
                agout[:, 0].rearrange("c p m f -> p c (m f)"),
            )
            nc.gpsimd.dma_start(
                gt_all[:, 8:16, :],
                agout[:, 1].rearrange("c p m f -> p c (m f)"),
            )

            # W_f GEMM with wt as *stationary* and the 16 grad
            # columns as moving data: out = X^T[j, row], 16 rows per
            # matmul (the stationary load is free). 8 j-blocks x 64 kt
            # accumulate in one psum tile; the bias is a rank-1 matmul
            # (bf slice x ones16) closing each group.
            psT = psx_p.tile([128, 8, FL], f32, tag="psT")
            for jb in range(8):
                for kt in range(KT):
                    nc.tensor.matmul(
                        psT[:, jb, :],
                        wt_all[:, kt, jb * 128 : (jb + 1) * 128],
                        gt_all[:, :, kt],
                        start=(kt == 0), stop=False, skip_group_check=True,
                    )
                nc.tensor.matmul(
                    psT[:, jb, :], bf_ap(jb * 128, (jb + 1) * 128),
                    ones40[:, 0:FL],
                    start=False, stop=True, skip_group_check=True,
                )

            # gated fusion on the transposed layout (rows on the free dim)
            xall = fu_p.tile([128, 8, FL], f16, tag="fu")
            nc.scalar.copy(xall[:], psT[:])
            xd = xall[:, :, 0:8]
            xa = xall[:, :, 8:16]
            ssum = fu_p.tile([128, 8, 8], f16, tag="fus8")
            nc.vector.tensor_add(ssum[:], xd, xa)
            d = fu_p.tile([128, 8, 8], f16, tag="fus8")
            nc.vector.tensor_sub(d[:], xd, xa)
            z = fu_p.tile([128, 8, 8], f16, tag="fus8")
            nc.scalar.activation(z[:], ssum[:], AF.Sigmoid)
            zd = fu_p.tile([128, 8, 8], f16, tag="fus8")
            nc.vector.tensor_mul(zd[:], z[:], d[:])
            o = fus_p.tile([128, 8, 8], f32, tag="fo")
            nc.vector.tensor_add(o[:], zd[:], xa)

            # transpose [j, (jb b)] -> [(jb b), j-block] and DMA out
            po = tr_p.tile([64, 128], f32, tag="po")
            nc.tensor.transpose(
                po[:], o[:].rearrange("p jb b -> p (jb b)"), id128f[:]
            )
            po_sb = fus_p.tile([64, 128], f32, tag="posb")
            nc.vector.tensor_copy(po_sb[:], po[:])
            dma_eng = (nc.sync, nc.scalar, nc.gpsimd)
            for jb in range(8):
                dma_eng[jb % 3].dma_start(
                    out_d[:, jb * 128 : (jb + 1) * 128],
                    po_sb[jb * 8 : (jb + 1) * 8, :],
                )

    _split_excess_waits(nc)
    return nc


def _prep_in_maps(inputs: dict) -> list[dict]:
    y = np.asarray(inputs["y"], np.float32)
    sd = np.asarray(inputs["supports_diff"], np.float32)
    sa = np.asarray(inputs["supports_adv"], np.float32)
    W_d1 = np.asarray(inputs["W_d1"], np.float32)
    W_d2 = -np.asarray(inputs["W_d2"], np.float32)
    W_a1 = np.asarray(inputs["W_a1"], np.float32)
    W_a2 = -np.asarray(inputs["W_a2"], np.float32)
    W_f = np.asarray(inputs["W_f"], np.float32)
    b_f = np.asarray(inputs["b_f"], np.float32)


    def cheb_fold(W, fin, M):
        # mats become [x0, x1_s, y2_s=S@x1_s]: W'[x0] -= sum W[x2_s];
        # W'[y2_s] = 2 W[x2_s]
        Wf = W.reshape(fin, M, -1).copy()
        for j in range(2, M, 2):
            Wf[:, 0, :] -= Wf[:, j, :]
            Wf[:, j, :] *= 2.0
        return Wf.reshape(fin * M, -1)

    W_a1 = cheb_fold(W_a1, FL, 17)
    W_d1 = cheb_fold(W_d1, FL, 3)
    W_a2 = cheb_fold(W_a2, U, 17)
    W_d2 = cheb_fold(W_d2, U, 3)

    # supports, transposed, node-tile-major, one per tile:
    # supT[b, p, si, m, n] = S_{3b+si}.T[m*128+p, n]
    supT = np.empty((3, 128, 3, 4, N), np.float16)
    for s in range(9):
        Ssrc = sa[s] if s < 8 else sd[0]
        st = Ssrc.T.astype(np.float16)  # [m, n]
        supT[s // 3, :, s % 3] = st.reshape(4, 128, N).transpose(1, 0, 2)

    def perm_pad(W, fin, M, fout, ntiles):
        # reference row (f, m) -> packed row m*fin+f, zero-padded to tiles
        Wp = W.reshape(fin, M, fout).transpose(1, 0, 2).reshape(fin * M, fout)
        pad = np.zeros((ntiles * 128, fout), np.float16)
        pad[: fin * M] = Wp.astype(np.float16)
        return pad.reshape(ntiles, 128, fout)

    wa1 = perm_pad(W_a1, FL, 17, U, 3)
    wd1 = perm_pad(W_d1, FL, 3, U, 1)
    wa2 = perm_pad(W_a2, U, 17, FL, 9)
    wd2 = perm_pad(W_d2, U, 3, FL, 2)

    # wt[q, m*16+f, j] = W_f.T[(m*128+q)*FL + f, c*JS+j]  (kt = m*16+f)
    WT = W_f.T.astype(np.float16)  # [k_orig = n*FL+f, j_global]
    in_maps = []
    for c in range(NCORES):
        x0 = y[c].reshape(N, FL)  # [node, f]
        x0m = x0.reshape(4, 128, FL).transpose(1, 0, 2).astype(np.float16)

        sm16 = np.zeros((128, _SM16), np.float16)
        sm16[:, _OFF_X0M : _OFF_X0M + 64] = x0m.reshape(128, 64)
        sm16[:, _OFF_WA1 : _OFF_WA1 + 3 * U] = wa1.transpose(1, 0, 2).reshape(
            128, 3 * U
        )
        sm16[:, _OFF_WD1 : _OFF_WD1 + U] = wd1[0]
        sm16[:, _OFF_WA2 : _OFF_WA2 + 9 * FL] = wa2.transpose(1, 0, 2).reshape(
            128, 9 * FL
        )
        sm16[:, _OFF_WD2 : _OFF_WD2 + 2 * FL] = wd2.transpose(1, 0, 2).reshape(
            128, 2 * FL
        )
        sm16[0, _OFF_BF : _OFF_BF + JS] = b_f[c * JS : (c + 1) * JS].astype(
            np.float16
        )
        sm16[0, _OFF_B1A : _OFF_B1A + U] = np.asarray(inputs["b_a1"], np.float16)
        sm16[0, _OFF_B1D : _OFF_B1D + U] = np.asarray(inputs["b_d1"], np.float16)
        sm16[0, _OFF_B2A : _OFF_B2A + FL] = -np.asarray(
            inputs["b_a2"], np.float16
        )
        sm16[0, _OFF_B2D : _OFF_B2D + FL] = -np.asarray(
            inputs["b_d2"], np.float16
        )
        sm16[0, _OFF_ONES : _OFF_ONES + 128] = 1.0

        # [(m q f), j] -> [q, m, f, j] -> [q, m*16+f, j]
        wt = np.ascontiguousarray(
            WT[:, c * JS : (c + 1) * JS]
            .reshape(4, 128, FL, JS)
            .transpose(1, 0, 2, 3)
            .reshape(128, KT, JS)
        )
        in_maps.append({"sm16": sm16, "supT": supT, "wt": wt})
    return in_maps


_CACHE: dict = {}


def _get_nc() -> bass.Bass:
    if "nc" not in _CACHE:
        _CACHE["nc"] = _build()
    return _CACHE["nc"]


def run(inputs: dict, trace: bool = False):
    """Run on the 8 cores; returns (full_output, BassKernelResults)."""
    in_maps = _prep_in_maps(inputs)
    nc = _get_nc()
    kw = {}
    if trace:
        kw = dict(trace=True, trace_cores=list(range(NCORES)), stitch_traces=False)
    res = run_bass_kernel_spmd(nc, in_maps, core_ids=list(range(NCORES)), **kw)
    out = np.concatenate(
        [res.results[c]["out"] for c in range(NCORES)], axis=1
    ).astype(np.float32)
    return out, res


def kernel(**inputs) -> np.ndarray:
    out, _ = run(inputs)
    return out


# revision 39
# speedup vs baseline: 1.7731x; 1.3248x over previous
"""Trainium2 Bass kernel for nn_ODEFunc (gnn_message_passing, 8 cores).

Strategy (cost model: matmul = out-free-rows; DMA = free-dim bytes per
queue, 3 queues; collective = 15us + gathered bytes / 40GBps):
  - Batch-parallel branches: core b computes batch b's diff+adv gconv
    branches. All Chebyshev mats are built NODE-major with the support
    as the matmul *stationary* ([128,128] S^T blocks), so each x1/x2
    costs only 16 matmuls x fin rows instead of streaming the 512-wide
    support as moving data.
  - Node-major mats are packed per layer into nm stacks [128, J, 4, fin]
    (mat index J on the free dim -> no partition-start issues). The
    layer GEMM needs feature-major stationaries: batched PE transposes
    flip 8 (L1) / 2 (L2) mats per 128-row k-tile in one psum bank.
  - Layer GEMMs run with the (host-permuted) weights as *moving* data:
    out = c1/grads node-major, 64/16 rows per matmul. Biases are added
    with a ones-row rank-1 matmul into the same psum group.
  - W_d2/b_d2, W_a2/b_a2 are host-negated so tanh emits the grad sign;
    the diff 0.1 coefficient is one DVE op. Grads come out node-major,
    exactly the AllGather staging layout (no grad transposes).
  - AllGather [2,128,4,16] fp16 per core; gathered grads land straight
    in the W_f stationary gt_all[128, 40, KT] via two 3-dim strided
    DMAs (kt = m*16+f; wt is host-permuted to the same k enumeration).
    psX[40, 512]: X_diff rows 0-7, X_adv rows 32-39 (cols 8-31 are
    memset-zero lanes).
  - W_f shard (fp16, 16 MB) is split across the 3 DMA queues sized to
    each queue's idle windows (ACT's chunk is scheduled into the
    collective window via tile_wait_until).
  - GEMM half 1 finishes before half 2 starts so its gated-fusion chain
    hides under half 2's matmuls.
"""

import sys

sys.path.insert(0, "/opt/trn_rl_repo")

import numpy as np

import concourse.bass as bass
import concourse.mybir as mybir
from concourse import masks
from concourse.bass_utils import run_bass_kernel_spmd
from concourse.tile import TileContext
from concourse.vector_clock import ScopedClock

N = 512          # nodes
FL = 16          # latent
U = 64           # units
B = 8            # batch
HID = N * FL     # 8192
COEFF = 0.1
NCORES = 8
JS = HID // NCORES  # 1024 output columns per core
KT = HID // 128     # 64 contraction tiles for the W_f GEMM

f16 = mybir.dt.float16
f32 = mybir.dt.float32
AF = mybir.ActivationFunctionType
ALU = mybir.AluOpType

# smalls_f16 packed free-dim offsets (elements)
_OFF_X0M = 0             # [128, 4*16] x0 node-major
_OFF_WA1 = 64            # 3 tiles [128, 64]
_OFF_WD1 = 256           # [48(->128), 64]
_OFF_WA2 = 320           # 9 tiles [128, 16]
_OFF_WD2 = 464           # 2 tiles [128, 16]
_OFF_BF = 496            # [1, 1024]
_OFF_B1A = 1520          # [1, 64]
_OFF_B1D = 1584          # [1, 64]
_OFF_B2A = 1648          # [1, 16]
_OFF_B2D = 1664          # [1, 16]
_OFF_ONES = 1680         # [1, 128] ones
_SM16 = 1808


class PatchedTileContext(TileContext):
    """Tail drain with at most one sem wait per instruction.

    The walrus build here rejects Drain instructions carrying >2 sync
    waits ("Too many sync wait commands"). Spread the global-clock waits
    over individual SP nops ahead of the drain.
    """

    def _drain_and_barrier(self, tick_clock, wait_clock):
        nc = self.nc
        probe = nc.sync.nop(nofuse=True)
        wait_clock.add_sem_waits(
            probe.ins, ScopedClock({None: tick_clock.global_clock})
        )
        si = probe.ins.sync_info
        ws = list(si.on_wait) if si is not None else []
        if len(ws) > 1:
            probe.ins.sync_info = mybir.SyncInfo(
                on_wait=ws[:1], on_update=list(si.on_update)
            )
            for w in ws[1:]:
                n2 = nc.sync.nop(nofuse=True)
                n2.ins.sync_info = mybir.SyncInfo(on_wait=[w], on_update=[])
        nc.sync.drain()
        nc.all_engine_barrier()
        popped = nc._tile_sem_poison_stack.pop()
        assert popped is self._sem_poison
        nc.clear_and_free_semaphores(list(self.sems.allocated().values()))
        nc.all_engine_barrier()


_WAIT_LIMIT = 1


def _split_excess_waits(nc: bass.Bass) -> None:
    """Move sync waits beyond _WAIT_LIMIT onto same-engine NOPs inserted
    just before the carrying instruction (this walrus build has tiny
    setupSyncWait budgets for DMA/collective/drain instruction formats)."""
    for fn in nc.m.functions:
        for bb in fn.blocks:
            insts = bb.instructions
            i = 0
            while i < len(insts):
                inst = insts[i]
                si = inst.sync_info
                ws = list(si.on_wait) if si is not None and si.on_wait else []
                if len(ws) > _WAIT_LIMIT and type(inst).__name__ != "InstNoOp":
                    keep = ws[:_WAIT_LIMIT]
                    extra = ws[_WAIT_LIMIT:]
                    inst.sync_info = mybir.SyncInfo(
                        on_wait=keep, on_update=list(si.on_update)
                    )
                    for k, w in enumerate(extra):
                        nop = mybir.InstNoOp(
                            name=f"{inst.name}-w{k}",
                            engine=inst.engine,
                            bass_nofuse=True,
                            sync_info=mybir.SyncInfo(on_wait=[w], on_update=[]),
                        )
                        nc.register_instruction(nop, overwrite=True)
                        insts.insert(i, nop)
                        i += 1
                i += 1


def _build(collective: bool = True) -> bass.Bass:
    nc = bass.Bass(num_devices=NCORES)

    # ---- DRAM I/O (per-core values supplied via in_maps) ----
    sm16_d = nc.dram_tensor("sm16", [128, _SM16], f16, kind="ExternalInput")
    sup_d = nc.dram_tensor("supT", [3, 128, 3, 4, N], f16, kind="ExternalInput")
    wt_d = nc.dram_tensor("wt", [128, KT, JS], f16, kind="ExternalInput")
    out_d = nc.dram_tensor("out", [B, JS], f32, kind="ExternalOutput")

    with PatchedTileContext(nc) as tc:
        from contextlib import ExitStack

        with ExitStack() as ctx:
            const_p = ctx.enter_context(tc.tile_pool(name="const", bufs=1))
            sup_p = ctx.enter_context(tc.tile_pool(name="sup", bufs=1))
            sc_p = ctx.enter_context(tc.tile_pool(name="sc", bufs=1))
            fus_p = ctx.enter_context(tc.tile_pool(name="fus", bufs=1))
            fu_p = ctx.enter_context(tc.tile_pool(name="fu", bufs=3))
            acc_p = ctx.enter_context(tc.tile_pool(name="acc", bufs=3, space="PSUM"))
            psx_p = ctx.enter_context(tc.tile_pool(name="psx", bufs=1, space="PSUM"))
            tr_p = ctx.enter_context(tc.tile_pool(name="tr", bufs=2, space="PSUM"))
            dram_p = ctx.enter_context(tc.tile_pool(name="dram", bufs=1, space="DRAM"))

            # ---- constants / memsets ----
            id128 = const_p.tile([128, 128], f16, tag="id")
            masks.make_identity(nc, id128[:])
            id128f = const_p.tile([128, 128], f32, tag="idf")
            masks.make_identity(nc, id128f[:])
            ones40 = const_p.tile([1, 40], f16, tag="ones")
            nc.vector.memset(ones40[:], 1.0)
            # W_f-GEMM moving data [q, col, kt]: diff grads cols 0-7,
            # adv cols 8-15 (every col is DMA'd; no zeroing needed)
            gt_all = const_p.tile([128, FL, KT], f16, tag="gt")

            # ---- input DMAs: smalls first; supports split SP/Pool so the
            # last support lands by ~9.5us ----
            sm16 = const_p.tile([128, _SM16], f16, tag="sm16")
            nc.sync.dma_start(sm16[:], sm16_d[:])
            sup_tiles = []
            for s in range(9):
                supb = sup_p.tile([128, 4, N], f16, tag=f"sup{s}")
                sup_tiles.append(supb)
            for s in (0, 2, 4, 6, 8):
                nc.sync.dma_start(sup_tiles[s][:], sup_d[s // 3][:, s % 3])
            for s in (1, 3, 5, 7):
                nc.gpsimd.dma_start(sup_tiles[s][:], sup_d[s // 3][:, s % 3])

            # W_f shard: SP streams most of it through the branch phase;
            # ACT's chunk is scheduled into the collective window. Pool
            # carries none (it does late-branch copies + the collective).
            wt_all = const_p.tile([128, KT, JS], f16, tag="wt")
            nc.sync.dma_start(wt_all[:, 0:44, :], wt_d[:, 0:44, :])
            with tc.tile_wait_until(0.030):
                nc.scalar.dma_start(wt_all[:, 44:64, :], wt_d[:, 44:64, :])

            # packed-small views
            x0m_all = sm16[:, _OFF_X0M : _OFF_X0M + 64]

            def x0m_ap(m):
                return sm16[:, _OFF_X0M + m * FL : _OFF_X0M + (m + 1) * FL]

            def wa1_ap(t, k=128):
                return sm16[0:k, _OFF_WA1 + t * U : _OFF_WA1 + (t + 1) * U]

            wd1_ap = sm16[0:48, _OFF_WD1 : _OFF_WD1 + U]

            def wa2_ap(t, k=128):
                return sm16[0:k, _OFF_WA2 + t * FL : _OFF_WA2 + (t + 1) * FL]

            def wd2_ap(t, k=128):
                return sm16[0:k, _OFF_WD2 + t * FL : _OFF_WD2 + (t + 1) * FL]

            def bf_ap(lo, hi):
                return sm16[0:1, _OFF_BF + lo : _OFF_BF + hi]

            b1a = sm16[0:1, _OFF_B1A : _OFF_B1A + U]
            b1d = sm16[0:1, _OFF_B1D : _OFF_B1D + U]
            b2a = sm16[0:1, _OFF_B2A : _OFF_B2A + FL]
            b2d = sm16[0:1, _OFF_B2D : _OFF_B2D + FL]
            ones128 = sm16[0:1, _OFF_ONES : _OFF_ONES + 128]

            # node-major mat stacks [128, J, m, fin]; J: 0=x0/c1, then
            # x1_s at 1+2s, x2_s at 2+2s (the reference concat order)
            nm1a = const_p.tile([128, 4, 17, FL], f16, tag="nm1a")
            nm1d = const_p.tile([128, 4, 3, FL], f16, tag="nm1d")
            nm2a = const_p.tile([128, 4, 17, U], f16, tag="nm2a")
            nm2d = const_p.tile([128, 4, 3, U], f16, tag="nm2d")

            nc.vector.tensor_copy(nm1a[:, :, 0, :], x0m_all)
            nc.scalar.copy(nm1d[:, :, 0, :], x0m_all)

            def supT_ap(s, m, j):
                # S_s^T[m-block, j-block] = stationary for out node-block j
                return sup_tiles[s][:, m, j * 128 : (j + 1) * 128]

            # copy/stt helpers: 0 = DVE, 1 = ACT (copy only), 2 = Pool
            def cp(which, dst, src):
                if which == 1:
                    nc.scalar.copy(dst, src)
                elif which == 2:
                    nc.gpsimd.tensor_copy(dst, src)
                else:
                    nc.vector.tensor_copy(dst, src)

            def cheb_nm(fin, nm, s_list, x_src, pool_tag, cp_rot):
                """x1 = S@x, x2 = 2*S@x1 - x in node-major form.

                x_src(s): [128, 4, fin] node-major input (x0 or c1).
                Writes nm[:, 1+2i, :, :] and nm[:, 2+2i, :, :] for s_list[i].
                """
                # pass A: x1 for every support
                for i, s in enumerate(s_list):
                    psb = acc_p.tile([128, 4, U], f32, tag="ps")
                    ps1 = psb[:, :, 0:fin]
                    for j in range(4):
                        for m in range(4):
                            nc.tensor.matmul(
                                ps1[:, j, :], supT_ap(s, m, j),
                                x_src(s)[:, m, :],
                                start=(m == 0), stop=(m == 3),
                                skip_group_check=True,
                            )
                    cp(cp_rot[i % len(cp_rot)], nm[s][:, :, 1 + 2 * i, :], ps1[:])
                # pass B: x2 = 2*(S@x1) - x
                for i, s in enumerate(s_list):
                    psb = acc_p.tile([128, 4, U], f32, tag="ps")
                    ps2 = psb[:, :, 0:fin]
                    for j in range(4):
                        for m in range(4):
                            nc.tensor.matmul(
                                ps2[:, j, :], supT_ap(s, m, j),
                                nm[s][:, m, 1 + 2 * i, :],
                                start=(m == 0), stop=(m == 3),
                                skip_group_check=True,
                            )
                    cp(cp_rot[(i + 1) % len(cp_rot)],
                       nm[s][:, :, 2 + 2 * i, :], ps2[:])

            def fm_transpose(nm_ap_fn, rows, tag, eng):
                """Batched FM flip: nm cols (J..J+g, fin) of each m-block ->
                fm tile [rows<=128, 512] (k = J*fin+f, node-major cols)."""
                fmb = sc_p.tile([128, N], f16, tag=tag)
                fm = fmb[0:rows, :]
                ptr = tr_p.tile([128, N], f16, tag="ptr")
                for m in range(4):
                    nc.tensor.transpose(
                        ptr[0:rows, m * 128 : (m + 1) * 128],
                        nm_ap_fn(m),
                        id128[:],
                    )
                cp(eng, fm, ptr[0:rows, :])
                return fm

            # ---- Layer 1 (fin=16) ----
            adv = list(range(8))
            cheb_nm(FL, {s: nm1a for s in adv}, adv,
                    lambda s: nm1a[:, :, 0, :], "psL1", (0, 1))
            cheb_nm(FL, {8: nm1d}, [8],
                    lambda s: nm1d[:, :, 0, :], "psL1", (1, 0))

            # FM stationaries for the L1 GEMM: adv tiles (8+8+1 mats),
            # diff tile (3 mats)
            fm1a = []
            for t in range(2):
                fm1a.append(fm_transpose(
                    lambda m, t=t: nm1a[:, m, 8 * t : 8 * t + 8, :],
                    128, f"fmA{t}", t % 2,
                ))
            fm1a.append(fm_transpose(
                lambda m: nm1a[:, m, 16, :], FL, "fmA2", 0))
            fm1d = fm_transpose(
                lambda m: nm1d[:, m, 0:3, :], 48, "fmA3", 1)

            # L1 GEMM (weights moving): c1 = tanh(xs @ W1 + b1), node-major
            pc1a = acc_p.tile([128, 4, U], f32, tag="ps")
            pc1d = acc_p.tile([128, 4, U], f32, tag="ps")
            for j in range(4):
                for t in range(3):
                    kk = 128 if t < 2 else FL
                    nc.tensor.matmul(
                        pc1a[:, j, :], fm1a[t][0:kk, j * 128 : (j + 1) * 128],
                        wa1_ap(t, kk),
                        start=(t == 0), stop=False, skip_group_check=True,
                    )
                nc.tensor.matmul(
                    pc1a[:, j, :], ones128, b1a,
                    start=False, stop=True, skip_group_check=True,
                )
                nc.tensor.matmul(
                    pc1d[:, j, :], fm1d[:, j * 128 : (j + 1) * 128], wd1_ap,
                    start=True, stop=False, skip_group_check=True,
                )
                nc.tensor.matmul(
                    pc1d[:, j, :], ones128, b1d,
                    start=False, stop=True, skip_group_check=True,
                )
            nc.scalar.activation(nm2a[:, :, 0, :], pc1a[:], AF.Tanh)
            nc.scalar.activation(nm2d[:, :, 0, :], pc1d[:], AF.Tanh)

            # ---- Layer 2 (fin=64); diff first so its grad chain overlaps ----
            cheb_nm(U, {8: nm2d}, [8],
                    lambda s: nm2d[:, :, 0, :], "psL2", (1, 0))
            cheb_nm(U, {s: nm2a for s in adv}, adv,
                    lambda s: nm2a[:, :, 0, :], "psL2", (0, 1))

            # FM stationaries for the L2 GEMM: diff (2 tiles), adv (9 tiles)
            fm2d = []
            fm2d.append(fm_transpose(
                lambda m: nm2d[:, m, 0:2, :], 128, "fmA0", 0))
            fm2d.append(fm_transpose(
                lambda m: nm2d[:, m, 2, :], U, "fmA1", 1))
            fm2a = []
            for t in range(8):
                fm2a.append(fm_transpose(
                    lambda m, t=t: nm2a[:, m, 2 * t : 2 * t + 2, :],
                    128, f"fmA{t+2}", (0, 1)[t % 2],
                ))
            fm2a.append(fm_transpose(
                lambda m: nm2a[:, m, 16, :], U, "fmA10", 1))

            # L2 GEMMs -> grads node-major [128, 4, 16] (the agin layout).
            # W2/b2 are host-negated so tanh lands the sign; diff still
            # needs the 0.1 coefficient.
            g_st = fus_p.tile([128, 2, 4, FL], f16, tag="gst")
            pgdb = acc_p.tile([128, 4, U], f32, tag="ps")
            pgd = pgdb[:, :, 0:FL]
            for j in range(4):
                for t in range(2):
                    kk = 128 if t < 1 else U
                    nc.tensor.matmul(
                        pgd[:, j, :], fm2d[t][0:kk, j * 128 : (j + 1) * 128],
                        wd2_ap(t, kk),
                        start=(t == 0), stop=False, skip_group_check=True,
                    )
                nc.tensor.matmul(
                    pgd[:, j, :], ones128, b2d,
                    start=False, stop=True, skip_group_check=True,
                )
            gd_t = sc_p.tile([128, 4, FL], f16, tag="gdt")
            nc.scalar.activation(gd_t[:], pgd[:], AF.Tanh)
            nc.vector.tensor_scalar_mul(g_st[:, 0, :, :], gd_t[:], COEFF)

            pgab = acc_p.tile([128, 4, U], f32, tag="ps")
            pga = pgab[:, :, 0:FL]
            for j in range(4):
                for t in range(9):
                    kk = 128 if t < 8 else U
                    nc.tensor.matmul(
                        pga[:, j, :], fm2a[t][0:kk, j * 128 : (j + 1) * 128],
                        wa2_ap(t, kk),
                        start=(t == 0), stop=False, skip_group_check=True,
                    )
                nc.tensor.matmul(
                    pga[:, j, :], ones128, b2a,
                    start=False, stop=True, skip_group_check=True,
                )
            nc.scalar.activation(g_st[:, 1, :, :], pga[:], AF.Tanh)

            # ---- AllGather of node-major grads: agin[r, p, m, f] ----
            agin = dram_p.tile([2, 128, 4, FL], f16)
            agout = dram_p.tile([NCORES, 2, 128, 4, FL], f16)
            nc.gpsimd.dma_start(agin[0].rearrange("p m f -> p (m f)"),
                                g_st[:, 0].rearrange("p m f -> p (m f)"))
            nc.gpsimd.dma_start(agin[1].rearrange("p m f -> p (m f)"),
                                g_st[:, 1].rearrange("p m f -> p (m f)"))
            if collective:
                nc.gpsimd.collective_compute(
                    "AllGather",
                    ALU.bypass,
                    replica_groups=[list(range(NCORES))],
                    ins=[agin.opt()],
                    outs=[agout.opt()],
                )
            else:
                for r in range(NCORES):
                    nc.gpsimd.dma_start(agout[r], agin[:])

            # ---- W_f phase ----
            # Gathered grads land directly in the stationary layout:
            # gt_all[q, col, kt] with kt = m*16+f <-> k = (m*128+q)*16+f;
            # wt is host-permuted to the same enumeration. Diff grads ->
            # cols 0-7, adv -> cols 32-39 (3-dim APs, contiguous last dim).
            nc.sync.dma_start(
                gt_all[:, 0:8, :],
                agout[:, 0].rearrange("c p m f -> p c (m f)"),
            )
            nc.gpsimd.dma_start(
                gt_all[:, 8:16, :],
                agout[:, 1].rearrange("c p m f -> p c (m f)"),
            )

            # W_f GEMM with wt as *stationary* and the 16 grad
            # columns as moving data: out = X^T[j, row], 16 rows per
            # matmul (the stationary load is free). 8 j-blocks x 64 kt
            # accumulate in one psum tile; the bias is a rank-1 matmul
            # (bf slice x ones16) closing each group.
            psT = psx_p.tile([128, 8, FL], f32, tag="psT")
            for jb in range(8):
                for kt in range(KT):
                    nc.tensor.matmul(
                        psT[:, jb, :],
                        wt_all[:, kt, jb * 128 : (jb + 1) * 128],
                        gt_all[:, :, kt],
                        start=(kt == 0), stop=False, skip_group_check=True,
                    )
                nc.tensor.matmul(
                    psT[:, jb, :], bf_ap(jb * 128, (jb + 1) * 128),
                    ones40[:, 0:FL],
                    start=False, stop=True, skip_group_check=True,
                )

            # gated fusion on the transposed layout (rows on the free dim)
            xall = fu_p.tile([128, 8, FL], f16, tag="fu")
            nc.scalar.copy(xall[:], psT[:])
            xd = xall[:, :, 0:8]
            xa = xall[:, :, 8:16]
            ssum = fu_p.tile([128, 8, 8], f16, tag="fus8")
            nc.vector.tensor_add(ssum[:], xd, xa)
            d = fu_p.tile([128, 8, 8], f16, tag="fus8")
            nc.vector.tensor_sub(d[:], xd, xa)
            z = fu_p.tile([128, 8, 8], f16, tag="fus8")
            nc.scalar.activation(z[:], ssum[:], AF.Sigmoid)
            zd = fu_p.tile([128, 8, 8], f16, tag="fus8")
            nc.vector.tensor_mul(zd[:], z[:], d[:])
            o = fus_p.tile([128, 8, 8], f32, tag="fo")
            nc.vector.tensor_add(o[:], zd[:], xa)

            # transpose [j, (jb b)] -> [(jb b), j-block] and DMA out
            po = tr_p.tile([64, 128], f32, tag="po")
            nc.tensor.transpose(
                po[:], o[:].rearrange("p jb b -> p (jb b)"), id128f[:]
            )
            po_sb = fus_p.tile([64, 128], f32, tag="posb")
            nc.vector.tensor_copy(po_sb[:], po[:])
            dma_eng = (nc.sync, nc.scalar, nc.gpsimd)
            for jb in range(8):
                dma_eng[jb % 3].dma_start(
                    out_d[:, jb * 128 : (jb + 1) * 128],
                    po_sb[jb * 8 : (jb + 1) * 8, :],
                )

    _split_excess_waits(nc)
    return nc


def _prep_in_maps(inputs: dict) -> list[dict]:
    y = np.asarray(inputs["y"], np.float32)
    sd = np.asarray(inputs["supports_diff"], np.float32)
    sa = np.asarray(inputs["supports_adv"], np.float32)
    W_d1 = np.asarray(inputs["W_d1"], np.float32)
    W_d2 = -np.asarray(inputs["W_d2"], np.float32)
    W_a1 = np.asarray(inputs["W_a1"], np.float32)
    W_a2 = -np.asarray(inputs["W_a2"], np.float32)
    W_f = np.asarray(inputs["W_f"], np.float32)
    b_f = np.asarray(inputs["b_f"], np.float32)


    def cheb_fold(W, fin, M):
        # mats become [x0, x1_s, y2_s=S@x1_s]: W'[x0] -= sum W[x2_s];
        # W'[y2_s] = 2 W[x2_s]
        Wf = W.reshape(fin, M, -1).copy()
        for j in range(2, M, 2):
            Wf[:, 0, :] -= Wf[:, j, :]
            Wf[:, j, :] *= 2.0
        return Wf.reshape(fin * M, -1)

    W_a1 = cheb_fold(W_a1, FL, 17)
    W_d1 = cheb_fold(W_d1, FL, 3)
    W_a2 = cheb_fold(W_a2, U, 17)
    W_d2 = cheb_fold(W_d2, U, 3)

    # supports, transposed, node-tile-major, one per tile:
    # supT[b, p, si, m, n] = S_{3b+si}.T[m*128+p, n]
    supT = np.empty((3, 128, 3, 4, N), np.float16)
    for s in range(9):
        Ssrc = sa[s] if s < 8 else sd[0]
        st = Ssrc.T.astype(np.float16)  # [m, n]
        supT[s // 3, :, s % 3] = st.reshape(4, 128, N).transpose(1, 0, 2)

    def perm_pad(W, fin, M, fout, ntiles):
        # reference row (f, m) -> packed row m*fin+f, zero-padded to tiles
        Wp = W.reshape(fin, M, fout).transpose(1, 0, 2).reshape(fin * M, fout)
        pad = np.zeros((ntiles * 128, fout), np.float16)
        pad[: fin * M] = Wp.astype(np.float16)
        return pad.reshape(ntiles, 128, fout)

    wa1 = perm_pad(W_a1, FL, 17, U, 3)
    wd1 = perm_pad(W_d1, FL, 3, U, 1)
    wa2 = perm_pad(W_a2, U, 17, FL, 9)
    wd2 = perm_pad(W_d2, U, 3, FL, 2)

    # wt[q, m*16+f, j] = W_f.T[(m*128+q)*FL + f, c*JS+j]  (kt = m*16+f)
    WT = W_f.T.astype(np.float16)  # [k_orig = n*FL+f, j_global]
    in_maps = []
    for c in range(NCORES):
        x0 = y[c].reshape(N, FL)  # [node, f]
        x0m = x0.reshape(4, 128, FL).transpose(1, 0, 2).astype(np.float16)

        sm16 = np.zeros((128, _SM16), np.float16)
        sm16[:, _OFF_X0M : _OFF_X0M + 64] = x0m.reshape(128, 64)
        sm16[:, _OFF_WA1 : _OFF_WA1 + 3 * U] = wa1.transpose(1, 0, 2).reshape(
            128, 3 * U
        )
        sm16[:, _OFF_WD1 : _OFF_WD1 + U] = wd1[0]
        sm16[:, _OFF_WA2 : _OFF_WA2 + 9 * FL] = wa2.transpose(1, 0, 2).reshape(
            128, 9 * FL
        )
        sm16[:, _OFF_WD2 : _OFF_WD2 + 2 * FL] = wd2.transpose(1, 0, 2).reshape(
            128, 2 * FL
        )
        sm16[0, _OFF_BF : _OFF_BF + JS] = b_f[c * JS : (c + 1) * JS].astype(
            np.float16
        )
        sm16[0, _OFF_B1A : _OFF_B1A + U] = np.asarray(inputs["b_a1"], np.float16)
        sm16[0, _OFF_B1D : _OFF_B1D + U] = np.asarray(inputs["b_d1"], np.float16)
        sm16[0, _OFF_B2A : _OFF_B2A + FL] = -np.asarray(
            inputs["b_a2"], np.float16
        )
        sm16[0, _OFF_B2D : _OFF_B2D + FL] = -np.asarray(
            inputs["b_d2"], np.float16
        )
        sm16[0, _OFF_ONES : _OFF_ONES + 128] = 1.0

        # [(m q f), j] -> [q, m, f, j] -> [q, m*16+f, j]
        wt = np.ascontiguousarray(
            WT[:, c * JS : (c + 1) * JS]
            .reshape(4, 128, FL, JS)
            .transpose(1, 0, 2, 3)
            .reshape(128, KT, JS)
        )
        in_maps.append({"sm16": sm16, "supT": supT, "wt": wt})
    return in_maps


_CACHE: dict = {}


def _get_nc() -> bass.Bass:
    if "nc" not in _CACHE:
        _CACHE["nc"] = _build()
    return _CACHE["nc"]


def run(inputs: dict, trace: bool = False):
    """Run on the 8 cores; returns (full_output, BassKernelResults)."""
    in_maps = _prep_in_maps(inputs)
    nc = _get_nc()
    kw = {}
    if trace:
        kw = dict(trace=True, trace_cores=list(range(NCORES)), stitch_traces=False)
    res = run_bass_kernel_spmd(nc, in_maps, core_ids=list(range(NCORES)), **kw)
    out = np.concatenate(
        [res.results[c]["out"] for c in range(NCORES)], axis=1
    ).astype(np.float32)
    return out, res


def kernel(**inputs) -> np.ndarray:
    out, _ = run(inputs)
    return out
